# revision 1
# baseline (speedup 1.0000x reference)
"""BiLSTM dual-pathway + CRF NLL kernel for 8 Trainium2 NeuronCores.

Sharding: data-parallel over batch (B=64 -> 8 per core). Each core runs the
full network on its batch shard entirely on-device and emits a partial sum of
(denom - num) over its 8 sequences; host sums and divides by 64.

On-chip layout is feature-major: features on SBUF partitions, (t*BL + b) on the
free axis. LSTM gate order is permuted to [i, f, o, g] so sigmoid applies to one
contiguous span. Matmul operands are fp16 (FWL weight loads), state/CRF fp32.
Backward-direction time reversal is done with negative-stride access patterns,
never materialized.
"""

import sys

sys.path.insert(0, "/opt/trn_rl_repo")

import numpy as np

import concourse.bass as bass
import concourse.mybir as mybir
from concourse import bacc
from concourse.bass import ds
from concourse.masks import make_identity
from concourse.tile import TileContext
from concourse.bass_utils import run_bass_kernel_spmd

F16 = mybir.dt.float16
F32 = mybir.dt.float32
AF = mybir.ActivationFunctionType

B, T, V, K = 64, 512, 40, 15
NC_N = 8
BL = B // NC_N          # 8 sequences per core
TB = T * BL             # 4096 free columns
UNROLL = 16

# directions: (name, Dk chunks of input, source kind, reverse, hs slot)
DIRS = [
    ("c0f", 1, "ce", False, "l0f"), ("c0b", 1, "ce", True, "l0b"),
    ("c1f", 4, "ch0", False, "c1f"), ("c1b", 4, "ch0", True, "c1b"),
    ("w0f", 6, "we", False, "l0f"), ("w0b", 6, "we", True, "l0b"),
    ("w1f", 4, "wh0", False, "w1f"), ("w1b", 4, "wh0", True, "w1b"),
]

_BUILD_CACHE = {}


def _seq_ap(tile, k, col_lo, n_steps, reverse):
    """AP over tile[:, k, :]: n_steps time-blocks of BL cols, fwd or reversed."""
    p_step = tile.ap[0][0]
    W = tile.ap[2][1]
    off = tile.offset + k * W + col_lo
    step = -BL if reverse else BL
    return bass.AP(tensor=tile.tensor, offset=off,
                   ap=[[p_step, 128], [step, n_steps], [1, BL]])


def _build_nc():
    if "nc" in _BUILD_CACHE:
        return _BUILD_CACHE["nc"]
    nc = bacc.Bacc(target_bir_lowering=False)

    # ---- external parameters -------------------------------------------------
    ceT_ext = nc.declare_dram_parameter("ceT", [128, 1, TB], F16, isOutput=False)
    weT_ext = nc.declare_dram_parameter("weT", [128, 6, TB], F16, isOutput=False)
    wih_ext, whh_ext = {}, {}
    for nm, dk, _, _, _ in DIRS:
        wih_ext[nm] = nc.declare_dram_parameter(f"wih_{nm}", [128, dk * 8 * 128], F16, isOutput=False)
        whh_ext[nm] = nc.declare_dram_parameter(f"whh_{nm}", [128, 2 * 8 * 128], F16, isOutput=False)
    biasall_ext = nc.declare_dram_parameter("biasall", [128, 8, 8], F32, isOutput=False)
    cls1_ext = nc.declare_dram_parameter("cls1", [128, 8 * 4 * 128], F16, isOutput=False)
    clsb1_ext = nc.declare_dram_parameter("clsb1", [128, 4], F32, isOutput=False)
    cls2_ext = nc.declare_dram_parameter("cls2", [128, 4 * 15], F16, isOutput=False)
    clsb2_ext = nc.declare_dram_parameter("clsb2", [15, 1], F32, isOutput=False)
    trans_ext = nc.declare_dram_parameter("trans", [15, 15], F32, isOutput=False)
    start_ext = nc.declare_dram_parameter("crfstart", [15, 1], F32, isOutput=False)
    end_ext = nc.declare_dram_parameter("crfend", [15, 1], F32, isOutput=False)
    tago_ext = nc.declare_dram_parameter("tagoneT", [15, TB], F16, isOutput=False)
    out_ext = nc.declare_dram_parameter("out", [1, 1], F32, isOutput=True)

    # ---- internal DRAM: pre-activation gate inputs per direction -------------
    xg_dram = {nm: nc.dram_tensor(f"xg_{nm}", [128, 8, TB], F16) for nm, _, _, _, _ in DIRS}

    with TileContext(nc) as tc:
        with (
            tc.tile_pool(name="consts", bufs=1) as consts,
            tc.tile_pool(name="seqs", bufs=1) as seqs,
            tc.tile_pool(name="wpool", bufs=1) as wpool,
            tc.tile_pool(name="work", bufs=2) as work,
            tc.tile_pool(name="stage", bufs=3) as stagep,
            tc.tile_pool(name="ps_big", bufs=2, space="PSUM") as ps_big,
            tc.tile_pool(name="ps_rec", bufs=2, space="PSUM") as ps_rec,
            tc.tile_pool(name="ps_small", bufs=3, space="PSUM") as ps_small,
        ):
            ident = consts.tile([128, 128], F32, tag="ident")
            make_identity(nc, ident)

            ceT = consts.tile([128, 1, TB], F16, tag="ceT")
            nc.sync.dma_start(out=ceT, in_=ceT_ext[:, :, :])
            cls1 = consts.tile([128, 8, 4, 128], F16, tag="cls1")
            nc.sync.dma_start(out=cls1, in_=cls1_ext.ap().rearrange("p (k m c) -> p k m c", k=8, m=4))
            clsb1 = consts.tile([128, 4], F32, tag="clsb1")
            nc.sync.dma_start(out=clsb1, in_=clsb1_ext[:, :])
            cls2 = consts.tile([128, 4, 15], F16, tag="cls2")
            nc.sync.dma_start(out=cls2, in_=cls2_ext.ap().rearrange("p (k j) -> p k j", k=4))
            clsb2 = consts.tile([15, 1], F32, tag="clsb2")
            nc.sync.dma_start(out=clsb2, in_=clsb2_ext[:, :])
            trans = consts.tile([15, 15], F32, tag="trans")
            nc.sync.dma_start(out=trans, in_=trans_ext[:, :])
            crfstart = consts.tile([15, 1], F32, tag="crfstart")
            nc.sync.dma_start(out=crfstart, in_=start_ext[:, :])
            crfend = consts.tile([15, 1], F32, tag="crfend")
            nc.sync.dma_start(out=crfend, in_=end_ext[:, :])
            tago = consts.tile([15, TB], F16, tag="tago")
            nc.sync.dma_start(out=tago, in_=tago_ext[:, :])
            biasall = consts.tile([128, 8, 8], F32, tag="biasall")
            nc.sync.dma_start(out=biasall, in_=biasall_ext[:, :, :])

            # sequence buffers: [128, 2, BL + T*BL] fp16; col (s+1)*BL holds h_s.
            # l0f/l0b slots are reused by the word pathway after the char L1
            # inputs have been consumed.
            hs_slot = {}
            for slot in ("l0f", "l0b", "c1f", "c1b", "w1f", "w1b"):
                hs_slot[slot] = seqs.tile([128, 2, BL + TB], F16, tag=f"hs_{slot}",
                                          name=f"hs_{slot}")
            hs = {nm: hs_slot[slot] for nm, _, _, _, slot in DIRS}

            def xg_rhs_ap(src_tile, dk, ns, reverse, width_steps, col_base):
                """rhs AP (128 x 64*BL) for xg matmul: source chunk dk, s-tile ns."""
                if not reverse:
                    return _seq_ap(src_tile, dk, col_base + ns * 64 * BL, 64, False)
                top = width_steps - 1 - ns * 64
                return _seq_ap(src_tile, dk, col_base + top * BL, 64, True)

            def xg_phase(di, nm, dk_n, src_kind, reverse):
                wih = wpool.tile([128, 6, 8, 128], F16, tag="wih")
                nc.sync.dma_start(
                    out=wih[:, :dk_n],
                    in_=wih_ext[nm].ap().rearrange("p (k m c) -> p k m c", k=dk_n, m=8),
                )
                for ns in range(8):
                    if src_kind == "we":
                        wxs = work.tile([128, 6, 64 * BL], F16, tag="wxs", bufs=1)
                        blk = (7 - ns) if reverse else ns
                        nc.sync.dma_start(out=wxs, in_=weT_ext[:, :, ds(blk * 64 * BL, 64 * BL)])
                    for m in range(8):
                        ps = ps_big.tile([128, 64, BL], F32, tag="xgps")
                        for dk in range(dk_n):
                            if src_kind == "ce":
                                rhs = xg_rhs_ap(ceT, dk, ns, reverse, T, 0)
                            elif src_kind == "we":
                                rhs = xg_rhs_ap(wxs, dk, 0, reverse, 64, 0)
                            else:
                                pre = "c0" if src_kind == "ch0" else "w0"
                                base = hs[pre + ("f" if dk < 2 else "b")]
                                krev = reverse if dk < 2 else (not reverse)
                                rhs = xg_rhs_ap(base, dk % 2, ns, krev, T, BL)
                            nc.tensor.matmul(ps, wih[:, dk, m], rhs,
                                             start=(dk == 0), stop=(dk == dk_n - 1))
                        st = stagep.tile([128, 64 * BL], F16, tag="xgstage")
                        nc.vector.tensor_scalar_add(st, ps, biasall[:, di, m : m + 1])
                        nc.sync.dma_start(out=xg_dram[nm][:, m, ds(ns * 64 * BL, 64 * BL)], in_=st)

            def lstm_rec(nm):
                hst = hs[nm]
                whh = work.tile([128, 2, 8, 128], F16, tag="whh")
                nc.sync.dma_start(
                    out=whh, in_=whh_ext[nm].ap().rearrange("p (k m c) -> p k m c", k=2, m=8)
                )
                cst = work.tile([128, 2, BL], F32, tag="cstate")
                nc.vector.memset(cst, 0.0)
                nc.vector.memset(hst[:, :, 0:BL], 0.0)
                with tc.For_i(0, T, UNROLL) as tv:
                    xgs = stagep.tile([128, 8, UNROLL * BL], F16, tag="xgs")
                    nc.sync.dma_start(out=xgs, in_=xg_dram[nm][:, :, ds(tv * BL, UNROLL * BL)])
                    for j in range(UNROLL):
                        ps = ps_rec.tile([128, 8, BL], F32, tag="recps")
                        for m in range(8):
                            for k in range(2):
                                nc.tensor.matmul(
                                    ps[:, m], whh[:, k, m],
                                    hst[:, k, ds(tv * BL + j * BL, BL)],
                                    start=(k == 0), stop=(k == 1),
                                )
                        g = stagep.tile([128, 8, BL], F32, tag="g")
                        nc.vector.tensor_add(g, ps, xgs[:, :, j * BL : (j + 1) * BL])
                        sig = stagep.tile([128, 6, BL], F32, tag="sig")
                        nc.scalar.activation(sig, g[:, 0:6], AF.Sigmoid)
                        tgg = stagep.tile([128, 2, BL], F32, tag="tgg")
                        nc.scalar.activation(tgg, g[:, 6:8], AF.Tanh)
                        tmp = stagep.tile([128, 2, BL], F32, tag="tmpig")
                        nc.vector.tensor_mul(tmp, sig[:, 0:2], tgg)
                        nc.vector.tensor_mul(cst, cst, sig[:, 2:4])
                        nc.vector.tensor_add(cst, cst, tmp)
                        tch = stagep.tile([128, 2, BL], F32, tag="tch")
                        nc.scalar.activation(tch, cst, AF.Tanh)
                        nc.vector.tensor_mul(
                            hst[:, :, ds(tv * BL + j * BL + BL, BL)], sig[:, 4:6], tch
                        )

            for di, (nm, dk_n, src, rev, _) in enumerate(DIRS):
                xg_phase(di, nm, dk_n, src, rev)
                lstm_rec(nm)

            # ---- classifier + logits --------------------------------------
            logits = seqs.tile([15, TB], F32, tag="logits")

            def comb_rhs(kk, ns):
                names = ["c1f", "c1b", "w1f", "w1b"]
                base = hs[names[kk // 2]]
                rev = (kk // 2) % 2 == 1
                return xg_rhs_ap(base, kk % 2, ns, rev, T, BL)

            for ns in range(8):
                hmt = []
                for m in range(4):
                    ps = ps_big.tile([128, 64, BL], F32, tag="xgps")
                    for kk in range(8):
                        nc.tensor.matmul(ps, cls1[:, kk, m], comb_rhs(kk, ns),
                                         start=(kk == 0), stop=(kk == 7))
                    hm = stagep.tile([128, 64 * BL], F16, tag="hm", bufs=4, name=f"hm{m}")
                    nc.scalar.activation(hm, ps, AF.Relu, bias=clsb1[:, m : m + 1])
                    hmt.append(hm)
                ps2 = ps_small.tile([15, 64 * BL], F32, tag="small")
                for m in range(4):
                    nc.tensor.matmul(ps2, cls2[:, m], hmt[m], start=(m == 0), stop=(m == 3))
                nc.vector.tensor_scalar_add(logits[:, ds(ns * 64 * BL, 64 * BL)], ps2, clsb2)

            # fold CRF start/end into first/last emission columns
            nc.vector.tensor_scalar_add(logits[:, 0:BL], logits[:, 0:BL], crfstart)
            nc.vector.tensor_scalar_add(logits[:, TB - BL : TB], logits[:, TB - BL : TB], crfend)

            # ---- CRF numerator --------------------------------------------
            # emission part: sum(logits * onehot); transition part via
            # V = trans^T @ onehot, shifted dot with onehot.
            racc = work.tile([15, 16], F32, tag="racc")
            nc.vector.memset(racc, 0.0)
            trans16 = consts.tile([15, 15], F16, tag="trans16")
            nc.vector.tensor_copy(trans16, trans)
            for ns in range(8):
                psv = ps_small.tile([15, 64 * BL], F32, tag="small")
                nc.tensor.matmul(psv, trans16, tago[:, ds(ns * 64 * BL, 64 * BL)], start=True, stop=True)
                w = 64 * BL if ns < 7 else 64 * BL - BL
                pr = stagep.tile([15, 64 * BL], F32, tag="prodns")
                nc.vector.tensor_mul(pr[:, :w], psv[:, :w], tago[:, ds(ns * 64 * BL + BL, w)])
                nc.vector.tensor_reduce(racc[:, ns : ns + 1], pr[:, :w],
                                        axis=mybir.AxisListType.X, op=mybir.AluOpType.add)
                pr2 = stagep.tile([15, 64 * BL], F32, tag="prodns")
                nc.vector.tensor_mul(pr2, logits[:, ds(ns * 64 * BL, 64 * BL)],
                                     tago[:, ds(ns * 64 * BL, 64 * BL)])
                nc.vector.tensor_reduce(racc[:, 8 + ns : 9 + ns], pr2,
                                        axis=mybir.AxisListType.X, op=mybir.AluOpType.add)
            nv = stagep.tile([15, 1], F32, tag="nv")
            nc.vector.tensor_reduce(nv, racc, axis=mybir.AxisListType.X, op=mybir.AluOpType.add)
            ones15 = consts.tile([15, 1], F32, tag="ones15")
            nc.vector.memset(ones15, 1.0)
            psn = ps_small.tile([1, 1], F32, tag="small")
            nc.tensor.matmul(psn, ones15, nv, start=True, stop=True)
            num11 = work.tile([1, 1], F32, tag="num11")
            nc.vector.tensor_copy(num11, psn)

            # ---- CRF forward scan (layout: [tags(15) x batch(8)]) ----------
            mexp = consts.tile([15, 15], F32, tag="mexp")
            nc.scalar.activation(mexp, trans, AF.Exp)
            ones115 = consts.tile([1, 15], F32, tag="ones115")
            nc.vector.memset(ones115, 1.0)
            alpha = seqs.tile([15, BL], F32, tag="alpha")
            nc.vector.tensor_copy(alpha, logits[:, 0:BL])
            off_r = seqs.tile([1, BL], F32, tag="off_r")
            nc.vector.memset(off_r, 0.0)

            def crf_step(e_ap):
                p8 = stagep.tile([15, BL], F32, tag="crfp")
                nc.scalar.activation(p8, alpha, AF.Exp)
                z = ps_small.tile([15, BL], F32, tag="small")
                nc.tensor.matmul(z, mexp, p8, start=True, stop=True)
                lnz = stagep.tile([15, BL], F32, tag="crflnz")
                nc.scalar.activation(lnz, z, AF.Ln)
                nc.vector.tensor_add(alpha, lnz, e_ap)

            def crf_renorm():
                pt = ps_small.tile([BL, 15], F32, tag="small")
                nc.tensor.transpose(pt, alpha, ident[:15, :15])
                a8 = stagep.tile([BL, 15], F32, tag="crfa8")
                nc.vector.tensor_copy(a8, pt)
                negm = stagep.tile([BL, 1], F32, tag="crfnegm")
                nc.vector.tensor_reduce(negm, a8, axis=mybir.AxisListType.X,
                                        op=mybir.AluOpType.max, negate=True)
                ptm = ps_small.tile([1, BL], F32, tag="small")
                nc.tensor.transpose(ptm, negm, ident[:BL, :BL])
                nr = stagep.tile([1, BL], F32, tag="crfnr")
                nc.vector.tensor_copy(nr, ptm)
                bps = ps_small.tile([15, BL], F32, tag="small")
                nc.tensor.matmul(bps, ones115, nr, start=True, stop=True)
                nc.vector.tensor_add(alpha, alpha, bps)
                nc.vector.tensor_sub(off_r, off_r, nr)

            with tc.For_i(0, 496, UNROLL) as tv:
                for j in range(UNROLL):
                    crf_step(logits[:, ds(tv * BL + (j + 1) * BL, BL)])
                crf_renorm()
            for t in range(497, T):
                crf_step(logits[:, t * BL : (t + 1) * BL])

            # ---- denominator + output -------------------------------------
            ptf = ps_small.tile([BL, 15], F32, tag="small")
            nc.tensor.transpose(ptf, alpha, ident[:15, :15])
            af8 = stagep.tile([BL, 15], F32, tag="af8")
            nc.vector.tensor_copy(af8, ptf)
            negm2 = stagep.tile([BL, 1], F32, tag="negm2")
            nc.vector.tensor_reduce(negm2, af8, axis=mybir.AxisListType.X,
                                    op=mybir.AluOpType.max, negate=True)
            e8 = stagep.tile([BL, 15], F32, tag="e8")
            s8 = stagep.tile([BL, 1], F32, tag="s8")
            nc.scalar.activation(e8, af8, AF.Exp, bias=negm2, accum_out=s8)
            l8 = stagep.tile([BL, 1], F32, tag="l8")
            nc.scalar.activation(l8, s8, AF.Ln)
            den8 = stagep.tile([BL, 1], F32, tag="den8")
            nc.vector.tensor_sub(den8, l8, negm2)
            pso = ps_small.tile([BL, 1], F32, tag="small")
            nc.tensor.transpose(pso, off_r, ident[:1, :1])
            o8 = stagep.tile([BL, 1], F32, tag="o8")
            nc.vector.tensor_copy(o8, pso)
            nc.vector.tensor_add(den8, den8, o8)
            ones8 = consts.tile([BL, 1], F32, tag="ones8")
            nc.vector.memset(ones8, 1.0)
            psd = ps_small.tile([1, 1], F32, tag="small")
            nc.tensor.matmul(psd, ones8, den8, start=True, stop=True)
            den11 = work.tile([1, 1], F32, tag="den11")
            nc.vector.tensor_copy(den11, psd)
            res = work.tile([1, 1], F32, tag="res")
            nc.vector.tensor_sub(res, den11, num11)
            nc.sync.dma_start(out=out_ext[:, :], in_=res)

    nc.finalize()
    _BUILD_CACHE["nc"] = nc
    return nc


# ---- host-side input prep ---------------------------------------------------

_GPERM = np.concatenate([np.arange(0, 512), np.arange(768, 1024), np.arange(512, 768)])


def _wih_prep(W, dk_n):
    # lhsT tiles: [p, dk, m, c] = W[gperm[m*128+c], dk*128+p]
    Wp = W[_GPERM]
    return np.ascontiguousarray(
        Wp.reshape(8, 128, dk_n, 128).transpose(3, 2, 0, 1).reshape(128, dk_n * 8 * 128)
    ).astype(np.float16)


def _common_inputs(inputs):
    out = {}
    bias_cols = []
    for key in ("c0", "c1", "w0", "w1"):
        Wih = np.asarray(inputs[f"{key}_Wih"], np.float32)
        Whh = np.asarray(inputs[f"{key}_Whh"], np.float32)
        bih = np.asarray(inputs[f"{key}_bih"], np.float32)
        bhh = np.asarray(inputs[f"{key}_bhh"], np.float32)
        dk_n = Wih.shape[2] // 128
        for r, sfx in ((0, "f"), (1, "b")):
            out[f"wih_{key}{sfx}"] = _wih_prep(Wih[r], dk_n)
            out[f"whh_{key}{sfx}"] = _wih_prep(Whh[r], 2)
            bb = (bih[r] + bhh[r])[_GPERM]
            bias_cols.append(bb.reshape(8, 128).T)  # (128, 8)
    # DIRS order is c0f,c0b,c1f,c1b,w0f,w0b,w1f,w1b == bias_cols order
    out["biasall"] = np.ascontiguousarray(np.stack(bias_cols, axis=1)).astype(np.float32)
    w1 = np.asarray(inputs["cls_w1"], np.float32)  # (512, 1024)
    out["cls1"] = np.ascontiguousarray(
        w1.reshape(4, 128, 8, 128).transpose(3, 2, 0, 1).reshape(128, 8 * 4 * 128)
    ).astype(np.float16)
    out["clsb1"] = np.ascontiguousarray(
        np.asarray(inputs["cls_b1"], np.float32).reshape(4, 128).T
    ).astype(np.float32)
    w2 = np.asarray(inputs["cls_w2"], np.float32)  # (15, 512)
    out["cls2"] = np.ascontiguousarray(
        w2.reshape(15, 4, 128).transpose(2, 1, 0).reshape(128, 4 * 15)
    ).astype(np.float16)
    out["clsb2"] = np.asarray(inputs["cls_b2"], np.float32).reshape(15, 1).copy()
    out["trans"] = np.asarray(inputs["crf_trans"], np.float32).copy()
    out["crfstart"] = np.asarray(inputs["crf_start"], np.float32).reshape(15, 1).copy()
    out["crfend"] = np.asarray(inputs["crf_end"], np.float32).reshape(15, 1).copy()
    return out


def kernel(**inputs):
    nc = _build_nc()
    common = _common_inputs(inputs)
    char_ids = np.asarray(inputs["char_ids"])
    tags = np.asarray(inputs["tags"])
    wemb = np.asarray(inputs["word_embeddings"], np.float32)
    emb = np.asarray(inputs["char_emb_table"], np.float32)

    in_maps = []
    for c in range(NC_N):
        lo, hi = c * BL, (c + 1) * BL
        m = dict(common)
        ce = emb[char_ids[lo:hi]]  # (BL, T, 128)
        m["ceT"] = np.ascontiguousarray(
            ce.transpose(2, 1, 0).reshape(128, 1, TB)
        ).astype(np.float16)
        m["weT"] = np.ascontiguousarray(
            wemb[lo:hi].reshape(BL, T, 6, 128).transpose(3, 2, 1, 0).reshape(128, 6, TB)
        ).astype(np.float16)
        oh = (np.arange(K)[:, None, None] == tags[lo:hi][None]).astype(np.float32)
        m["tagoneT"] = np.ascontiguousarray(oh.transpose(0, 2, 1).reshape(K, TB)).astype(np.float16)
        in_maps.append(m)

    res = run_bass_kernel_spmd(nc, in_maps, core_ids=list(range(NC_N)))
    total = sum(float(res.results[c]["out"][0, 0]) for c in range(NC_N))
    return np.float32(total / B)



# revision 8
# speedup vs baseline: 2.5854x; 2.5854x over previous
"""BiLSTM dual-pathway + CRF NLL kernel for 8 Trainium2 NeuronCores.

Sharding: direction-parallel for the LSTM recurrences, data-parallel for the
classifier/CRF. Phase 1 runs the four layer-0 directions (char fwd/bwd, word
fwd/bwd) on 8 cores as (direction x batch-half), batch 32 per core, so the
recurrent matmuls run at N=32 instead of N=8 and the sequential chain count
drops from 4096 to 1024 steps. A pairwise AllGather exchanges the L0 hidden
states between fwd/bwd cores, phase 2 runs the four layer-1 directions the
same way, then an 8-rank AllToAll redistributes hidden states to a
data-parallel layout (8 sequences per core) for the classifier and CRF.

SPMD uniformity: every core runs the identical program. Backward directions
receive host-time-reversed inputs; reversed reads of peer hidden states are
fixed negative-stride APs, with host-permuted weight columns absorbing the
f/b role differences. Peer-region selection after the AllGather uses per-core
0/1 blend masks delivered as input data.

The CRF forward scan runs in probability space with a constant per-step
prescale alpha folded into the emission exponentials (corrected analytically
on the host), so each step is one resident-weight 15x15 matmul plus one
vector multiply -- no per-step exp/ln activation-table swaps.
"""

import sys

sys.path.insert(0, "/opt/trn_rl_repo")

import numpy as np

import concourse.bass as bass
import concourse.mybir as mybir
from concourse import bacc
from concourse.bass import ds
from concourse.tile import TileContext
from concourse.bass_utils import run_bass_kernel_spmd

F16 = mybir.dt.float16
F32 = mybir.dt.float32
AF = mybir.ActivationFunctionType

B, T, V, K = 64, 512, 40, 15
NC_N = 8
BL2 = 32            # batch per core in phases 1-2
BL3 = 8             # sequences per core in phase 3
TB3 = T * BL3       # 4096 classifier/CRF columns per core
DK1, DK2 = 6, 4     # input chunks for L0 (word=768, char padded) and L1 (512)
UNROLL = 16
HCOL = T + 1        # h buffer columns per sequence (col 0 = zero init)

_BUILD_CACHE = {}


def _rec_phase(nc, tc, consts, work, stagep, ps_rec, ps_big, h_sb, cst,
               wih, whh, bias, xg_dram, dkn, gemm_src_ap, phase_tag):
    """Emit one GEMM pass (xg to DRAM) + one 512-step recurrence.

    gemm_src_ap(k, tv) -> AP of [128, UNROLL*BL2] input columns for chunk k,
    time block tv (may be a closure that DMAs/blends into SBUF and returns
    the tile AP).
    """
    CB = UNROLL * BL2  # columns per time block

    # ---- GEMM: xg[:, m, block] = sum_k wih[k, m]^T X[k, block] + bias ----
    with tc.For_i(0, T, UNROLL) as tv:
        srcs = gemm_src_ap(tv)
        for m in range(8):
            ps = ps_big.tile([128, CB], F32, tag="gemmps")
            for k in range(dkn):
                nc.tensor.matmul(ps, wih[:, k, m], srcs[k],
                                 start=(k == 0), stop=(k == dkn - 1))
            st = stagep.tile([128, CB], F16, tag="xgstage")
            nc.scalar.activation(st, ps, AF.Identity, bias=bias[:, m: m + 1])
            nc.sync.dma_start(out=xg_dram[:, m, ds(tv * BL2, CB)], in_=st)

    # ---- recurrence ----
    nc.vector.memset(cst, 0.0)
    nc.vector.memset(h_sb[:, :, :, 0:1], 0.0)

    def h_col(t_off):
        # [128, 2, 32] view of h at column t_off (b-stride = HCOL)
        p_step = h_sb.ap[0][0]
        return bass.AP(tensor=h_sb.tensor, offset=h_sb.offset + t_off,
                       ap=[[p_step, 128], [BL2 * HCOL, 2], [HCOL, BL2]])

    with tc.For_i(0, T, UNROLL) as tv:
        xgs = stagep.tile([128, 8, CB], F16, tag="xgs")
        nc.sync.dma_start(out=xgs, in_=xg_dram[:, :, ds(tv * BL2, CB)])
        for j in range(UNROLL):
            ps = ps_rec.tile([128, 8, BL2], F32, tag="recps")
            hin = h_col(tv + j)
            # m-order: i,f (0-3) first, then g~ (6,7), then o (4,5)
            for m in (0, 1, 2, 3, 6, 7, 4, 5):
                for k in range(2):
                    nc.tensor.matmul(ps[:, m], whh[:, k, m],
                                     hin[:, k], start=(k == 0), stop=(k == 1))
            g = stagep.tile([128, 8, BL2], F32, tag="g")
            xsl = xgs[:, :, ds(j * BL2, BL2)]
            nc.vector.tensor_add(g[:, 0:4], ps[:, 0:4], xsl[:, 0:4])
            sig = stagep.tile([128, 6, BL2], F32, tag="sig")
            nc.scalar.activation(sig[:, 0:4], g[:, 0:4], AF.Sigmoid)
            nc.vector.tensor_mul(cst, cst, sig[:, 2:4])
            nc.vector.tensor_add(g[:, 6:8], ps[:, 6:8], xsl[:, 6:8])
            tgg = stagep.tile([128, 2, BL2], F32, tag="tgg")
            nc.scalar.activation(tgg, g[:, 6:8], AF.Tanh)
            tmp = stagep.tile([128, 2, BL2], F32, tag="tmpig")
            nc.vector.tensor_mul(tmp, sig[:, 0:2], tgg)
            nc.vector.tensor_add(g[:, 4:6], ps[:, 4:6], xsl[:, 4:6])
            nc.scalar.activation(sig[:, 4:6], g[:, 4:6], AF.Sigmoid)
            nc.vector.tensor_add(cst, cst, tmp)
            tch = stagep.tile([128, 2, BL2], F32, tag="tch")
            nc.scalar.activation(tch, cst, AF.Tanh)
            hout = h_col(tv + j + 1)
            nc.vector.tensor_mul(hout, sig[:, 4:6], tch)


def _build_nc():
    if "nc" in _BUILD_CACHE:
        return _BUILD_CACHE["nc"]
    nc = bacc.Bacc(target_bir_lowering=False, num_devices=NC_N)

    # ---- external parameters -------------------------------------------------
    x1_ext = nc.declare_dram_parameter("x1", [128, DK1, T * BL2], F16, isOutput=False)
    wih1_ext = nc.declare_dram_parameter("wih1", [128, DK1 * 8 * 128], F16, isOutput=False)
    whh1_ext = nc.declare_dram_parameter("whh1", [128, 2 * 8 * 128], F16, isOutput=False)
    bias1_ext = nc.declare_dram_parameter("bias1", [128, 8], F32, isOutput=False)
    wih2_ext = nc.declare_dram_parameter("wih2", [128, DK2 * 8 * 128], F16, isOutput=False)
    whh2_ext = nc.declare_dram_parameter("whh2", [128, 2 * 8 * 128], F16, isOutput=False)
    bias2_ext = nc.declare_dram_parameter("bias2", [128, 8], F32, isOutput=False)
    masks_ext = nc.declare_dram_parameter("masks", [128, 2], F32, isOutput=False)
    cls1_ext = nc.declare_dram_parameter("cls1", [128, 8 * 4 * 128], F16, isOutput=False)
    clsb1_ext = nc.declare_dram_parameter("clsb1", [128, 4], F32, isOutput=False)
    cls2_ext = nc.declare_dram_parameter("cls2", [128, 4 * 15], F16, isOutput=False)
    clsb2_ext = nc.declare_dram_parameter("clsb2", [15, 1], F32, isOutput=False)
    mexp_ext = nc.declare_dram_parameter("mexp", [15, 15], F32, isOutput=False)
    transn_ext = nc.declare_dram_parameter("transn", [15, 15], F16, isOutput=False)
    start_ext = nc.declare_dram_parameter("crfstart", [15, 1], F32, isOutput=False)
    end_ext = nc.declare_dram_parameter("crfend", [15, 1], F32, isOutput=False)
    lna_ext = nc.declare_dram_parameter("lnalpha", [15, 1], F32, isOutput=False)
    tago_ext = nc.declare_dram_parameter("tagoneT", [15, TB3], F16, isOutput=False)
    out_ext = nc.declare_dram_parameter("out", [1, 1], F32, isOutput=True)

    # ---- internal DRAM -------------------------------------------------------
    HSZ = 2 * BL2 * T  # 32768 cols/partition of h (f16)
    xg1_dram = nc.dram_tensor("xg1", [128, 8, T * BL2], F16)
    xg2_dram = nc.dram_tensor("xg2", [128, 8, T * BL2], F16)
    hmine = nc.dram_tensor("hmine", [128, HSZ], F16)
    agout = nc.dram_tensor("agout", [2, 128, HSZ], F16)
    sendb = nc.dram_tensor("sendb", [8, 128, 2 * 4 * T], F16)
    recvb = nc.dram_tensor("recvb", [8, 128, 2 * 4 * T], F16)

    with TileContext(nc) as tc:
        with (
            tc.tile_pool(name="consts", bufs=1) as consts,
            tc.tile_pool(name="seqs", bufs=1) as seqs,
            tc.tile_pool(name="work", bufs=2) as work,
            tc.tile_pool(name="stage", bufs=2) as stagep,
            tc.tile_pool(name="ps_big", bufs=2, space="PSUM") as ps_big,
            tc.tile_pool(name="ps_rec", bufs=2, space="PSUM") as ps_rec,
            tc.tile_pool(name="ps_small", bufs=3, space="PSUM") as ps_small,
        ):
            # h buffer, seq-major: [128, chunk2, b32, T+1], reused by phases 1+2
            h_sb = seqs.tile([128, 2, BL2, HCOL], F16, tag="h_sb")
            cst = seqs.tile([128, 2, BL2], F32, tag="cstate")
            masks = consts.tile([128, 2], F32, tag="masks")
            nc.sync.dma_start(out=masks, in_=masks_ext[:, :])

            # ================= PHASE 1 (L0) =================
            wih1 = consts.tile([128, DK1, 8, 128], F16, tag="wbig")
            nc.sync.dma_start(
                out=wih1,
                in_=wih1_ext.ap().rearrange("p (k m c) -> p k m c", k=DK1, m=8))
            whh1 = consts.tile([128, 2, 8, 128], F16, tag="whh")
            nc.sync.dma_start(
                out=whh1,
                in_=whh1_ext.ap().rearrange("p (k m c) -> p k m c", k=2, m=8))
            bias1 = consts.tile([128, 8], F32, tag="bias1")
            nc.sync.dma_start(out=bias1, in_=bias1_ext[:, :])

            CB = UNROLL * BL2

            def gemm1_src(tv):
                xb = stagep.tile([128, DK1, CB], F16, tag="xb1")
                nc.sync.dma_start(out=xb, in_=x1_ext[:, :, ds(tv * BL2, CB)])
                return [xb[:, k] for k in range(DK1)]

            _rec_phase(nc, tc, consts, work, stagep, ps_rec, ps_big, h_sb, cst,
                       wih1, whh1, bias1, xg1_dram, DK1, gemm1_src, "p1")

            # h (cols 1..T+1) -> local DRAM, then pairwise AllGather
            hm_view = hmine.ap().rearrange("p (c b t) -> p c b t", c=2, b=BL2)
            for c in range(2):
                nc.sync.dma_start(out=hm_view[:, c], in_=h_sb[:, c, :, 1:HCOL])
            nc.gpsimd.collective_compute(
                "AllGather", mybir.AluOpType.bypass,
                replica_groups=[[0, 2], [1, 3], [4, 6], [5, 7]],
                ins=[hmine.ap()], outs=[agout.ap()],
            )

            # ================= PHASE 2 (L1) =================
            wih2 = consts.tile([128, DK2, 8, 128], F16, tag="wbig")
            nc.sync.dma_start(
                out=wih2,
                in_=wih2_ext.ap().rearrange("p (k m c) -> p k m c", k=DK2, m=8))
            whh2 = consts.tile([128, 2, 8, 128], F16, tag="whh")
            nc.sync.dma_start(
                out=whh2,
                in_=whh2_ext.ap().rearrange("p (k m c) -> p k m c", k=2, m=8))
            bias2 = consts.tile([128, 8], F32, tag="bias2")
            nc.sync.dma_start(out=bias2, in_=bias2_ext[:, :])

            def slot_ap(tensor_offset, tensor, c, tv, reverse):
                # [128, 32, UNROLL] chunk c of an h-layout DRAM region
                # (p, c2, b32, T), always read as an ascending t window;
                # reversed slots flip t in the SBUF-side rhs view instead.
                if not reverse:
                    off = tensor_offset + c * (BL2 * T) + tv
                else:
                    off = tensor_offset + c * (BL2 * T) + (T - UNROLL) - tv
                return bass.AP(tensor=tensor, offset=off,
                               ap=[[HSZ, 128], [T, BL2], [1, UNROLL]])

            def gemm2_src(tv):
                # slot tiles are b-major [128, c2, b32, t16] to match the
                # DRAM h layout; the matmul rhs view re-orders to (t, b)
                sa = stagep.tile([128, 2, BL2, UNROLL], F16, tag="slotA")
                for c in range(2):
                    nc.sync.dma_start(out=sa[:, c], in_=slot_ap(0, hmine, c, tv, False))
                # slotB: blend of the two AG regions, read time-reversed
                r0 = stagep.tile([128, 2, BL2, UNROLL], F16, tag="slotR0")
                r1 = stagep.tile([128, 2, BL2, UNROLL], F16, tag="slotR1")
                for c in range(2):
                    nc.sync.dma_start(out=r0[:, c], in_=slot_ap(0, agout, c, tv, True))
                    nc.sync.dma_start(out=r1[:, c], in_=slot_ap(128 * HSZ, agout, c, tv, True))
                sb = stagep.tile([128, 2, BL2, UNROLL], F16, tag="slotB")
                nc.vector.tensor_scalar_mul(sb, r0, masks[:, 0:1])
                nc.vector.tensor_scalar_mul(r1, r1, masks[:, 1:2])
                nc.vector.tensor_add(sb, sb, r1)
                saf = sa.rearrange("p c b t -> p c t b")

                def rev_t(tile, c):
                    # (t desc, b) view of chunk c: local step s reads t=15-s
                    p_step = tile.ap[0][0]
                    off = tile.offset + c * (BL2 * UNROLL) + (UNROLL - 1)
                    return bass.AP(tensor=tile.tensor, offset=off,
                                   ap=[[p_step, 128], [-1, UNROLL],
                                       [UNROLL, BL2]])

                return [saf[:, 0], saf[:, 1], rev_t(sb, 0), rev_t(sb, 1)]

            _rec_phase(nc, tc, consts, work, stagep, ps_rec, ps_big, h_sb, cst,
                       wih2, whh2, bias2, xg2_dram, DK2, gemm2_src, "p2")

            # ---- AllToAll redistribution to data-parallel layout ----
            sb_view = sendb.ap().rearrange("j p (c b t) -> j p c b t", c=2, b=4)
            for j in range(8):
                for c in range(2):
                    nc.sync.dma_start(out=sb_view[j, :, c],
                                      in_=h_sb[:, c, ds(4 * j, 4), 1:HCOL])
            nc.gpsimd.collective_compute(
                "AllToAll", mybir.AluOpType.bypass,
                replica_groups=[list(range(8))],
                ins=[sendb.ap()], outs=[recvb.ap()],
            )

            # ================= PHASE 3: classifier + CRF =================
            cls1 = consts.tile([128, 8, 4, 128], F16, tag="wbig")
            nc.sync.dma_start(
                out=cls1,
                in_=cls1_ext.ap().rearrange("p (k m c) -> p k m c", k=8, m=4))
            clsb1 = consts.tile([128, 4], F32, tag="clsb1")
            nc.sync.dma_start(out=clsb1, in_=clsb1_ext[:, :])
            cls2 = consts.tile([128, 4, 15], F16, tag="cls2")
            nc.sync.dma_start(
                out=cls2, in_=cls2_ext.ap().rearrange("p (k j) -> p k j", k=4))
            clsb2 = consts.tile([15, 1], F32, tag="clsb2")
            nc.sync.dma_start(out=clsb2, in_=clsb2_ext[:, :])
            mexp = consts.tile([15, 15], F32, tag="mexp")
            nc.sync.dma_start(out=mexp, in_=mexp_ext[:, :])
            transn = consts.tile([15, 15], F16, tag="transn")
            nc.sync.dma_start(out=transn, in_=transn_ext[:, :])
            crfstart = consts.tile([15, 1], F32, tag="crfstart")
            nc.sync.dma_start(out=crfstart, in_=start_ext[:, :])
            crfend = consts.tile([15, 1], F32, tag="crfend")
            nc.sync.dma_start(out=crfend, in_=end_ext[:, :])
            lnalpha = consts.tile([15, 1], F32, tag="lnalpha")
            nc.sync.dma_start(out=lnalpha, in_=lna_ext[:, :])
            tago = consts.tile([15, TB3], F16, tag="tago")
            nc.sync.dma_start(out=tago, in_=tago_ext[:, :])

            logits = seqs.tile([15, TB3], F32, tag="logits")

            NT = 64  # t-steps per classifier n-tile (NT*BL3 = 512 cols)
            SHSZ = 128 * 2 * 4 * T  # elements per recv shard

            def comb_ap(kk, half, ns):
                # [128, 4, NT]: dir kk//2, chunk kk%2, half-shard, n-tile ns;
                # always an ascending t window (reversal done in the rhs view)
                d, c = kk // 2, kk % 2
                rev = d in (1, 3)  # c1b, w1b stored time-reversed
                base = (2 * d + half) * SHSZ + c * (4 * T)
                if not rev:
                    off = base + ns * NT
                else:
                    off = base + (T - NT) - ns * NT
                return bass.AP(tensor=recvb, offset=off,
                               ap=[[2 * 4 * T, 128], [T, 4], [1, NT]])

            for ns in range(8):
                comb = stagep.tile([128, 8, BL3, NT], F16, tag="comb", bufs=2)
                for kk in range(8):
                    for half in range(2):
                        nc.sync.dma_start(out=comb[:, kk, ds(4 * half, 4)],
                                          in_=comb_ap(kk, half, ns))
                hmt = []
                for m in range(4):
                    ps = ps_big.tile([128, NT * BL3], F32, tag="gemmps")
                    for kk in range(8):
                        if kk // 2 in (1, 3):
                            p_step = comb.ap[0][0]
                            off = (comb.offset + kk * BL3 * NT + (NT - 1))
                            rhs = bass.AP(tensor=comb.tensor, offset=off,
                                          ap=[[p_step, 128], [-1, NT],
                                              [NT, BL3]])
                        else:
                            rhs = comb[:, kk].rearrange("p b t -> p t b")
                        nc.tensor.matmul(ps, cls1[:, kk, m], rhs,
                                         start=(kk == 0), stop=(kk == 7))
                    hm = stagep.tile([128, NT * BL3], F16, tag="hm", bufs=4,
                                     name=f"hm{m}")
                    nc.scalar.activation(hm, ps, AF.Relu, bias=clsb1[:, m: m + 1])
                    hmt.append(hm)
                ps2 = ps_small.tile([15, NT * BL3], F32, tag="small")
                for m in range(4):
                    nc.tensor.matmul(ps2, cls2[:, m], hmt[m],
                                     start=(m == 0), stop=(m == 3))
                nc.vector.tensor_scalar_add(
                    logits[:, ds(ns * NT * BL3, NT * BL3)], ps2, clsb2)

            # fold CRF start/end into first/last emission columns
            nc.vector.tensor_scalar_add(logits[:, 0:BL3], logits[:, 0:BL3], crfstart)
            nc.vector.tensor_scalar_add(logits[:, TB3 - BL3: TB3],
                                        logits[:, TB3 - BL3: TB3], crfend)

            # ---- CRF numerator ----
            racc = work.tile([15, 16], F32, tag="racc")
            nc.vector.memset(racc, 0.0)
            for ns in range(8):
                pre = stagep.tile([15, 512], F32, tag="prodns")
                nc.vector.tensor_mul(pre, logits[:, ds(ns * 512, 512)],
                                     tago[:, ds(ns * 512, 512)])
                nc.vector.tensor_reduce(racc[:, 8 + ns: 9 + ns], pre,
                                        axis=mybir.AxisListType.X,
                                        op=mybir.AluOpType.add)
                psv = ps_small.tile([15, 512], F32, tag="small")
                nc.tensor.matmul(psv, transn, tago[:, ds(ns * 512, 512)],
                                 start=True, stop=True)
                w = 512 if ns < 7 else 512 - BL3
                pr = stagep.tile([15, 512], F32, tag="prodns")
                nc.vector.tensor_mul(pr[:, :w], psv[:, :w],
                                     tago[:, ds(ns * 512 + BL3, w)])
                nc.vector.tensor_reduce(racc[:, ns: ns + 1], pr[:, :w],
                                        axis=mybir.AxisListType.X,
                                        op=mybir.AluOpType.add)
            nv = stagep.tile([15, 1], F32, tag="nv")
            nc.vector.tensor_reduce(nv, racc, axis=mybir.AxisListType.X,
                                    op=mybir.AluOpType.add)
            ones15 = consts.tile([15, 1], F32, tag="ones15")
            nc.vector.memset(ones15, 1.0)
            psn = ps_small.tile([1, 1], F32, tag="small")
            nc.tensor.matmul(psn, ones15, nv, start=True, stop=True)
            num11 = work.tile([1, 1], F32, tag="num11")
            nc.vector.tensor_copy(num11, psn)

            # ---- CRF forward scan, probability space ----
            # p0 first, then E = alpha*exp(logits) computed in place over
            # logits (last col already includes e^end)
            p = seqs.tile([15, BL3], F32, tag="pvec")
            nc.scalar.activation(p, logits[:, 0:BL3], AF.Exp)
            E = logits
            nc.scalar.activation(E, logits, AF.Exp, bias=lnalpha)

            def crf_step(t_col_ap):
                z = ps_small.tile([15, BL3], F32, tag="small")
                nc.tensor.matmul(z, mexp, p, start=True, stop=True)
                nc.vector.tensor_mul(p, z, t_col_ap)

            for t in range(1, 16):
                crf_step(E[:, t * BL3: (t + 1) * BL3])
            with tc.For_i(0, 496, UNROLL) as tv:
                for j in range(UNROLL):
                    crf_step(E[:, ds((16 + j) * BL3 + tv * BL3, BL3)])

            # ---- denominator + output ----
            psd = ps_small.tile([1, BL3], F32, tag="small")
            nc.tensor.matmul(psd, ones15, p, start=True, stop=True)
            ln8 = stagep.tile([1, BL3], F32, tag="ln8")
            nc.scalar.activation(ln8, psd, AF.Ln)
            den11 = work.tile([1, 1], F32, tag="den11")
            nc.vector.tensor_reduce(den11, ln8, axis=mybir.AxisListType.X,
                                    op=mybir.AluOpType.add)
            res = work.tile([1, 1], F32, tag="res")
            nc.vector.tensor_sub(res, den11, num11)
            nc.sync.dma_start(out=out_ext[:, :], in_=res)

    nc.finalize()
    _BUILD_CACHE["nc"] = nc
    return nc


# ---- host-side input prep ---------------------------------------------------

# gate perm [i(256), f(256), g(256), o(256)] -> [i, f, o, g~]
_GPERM = np.concatenate([np.arange(0, 512), np.arange(768, 1024), np.arange(512, 768)])

# core c -> (pathway, direction, half): 0..3 char f/f/b/b, 4..7 word
_ROLES = [("c", 0, 0), ("c", 0, 1), ("c", 1, 0), ("c", 1, 1),
          ("w", 0, 0), ("w", 0, 1), ("w", 1, 0), ("w", 1, 1)]


def _wih_prep(W, dk_n):
    Wp = W[_GPERM]
    return np.ascontiguousarray(
        Wp.reshape(8, 128, dk_n, 128).transpose(3, 2, 0, 1).reshape(128, dk_n * 8 * 128)
    ).astype(np.float16)


def _make_in_maps(inputs):
    char_ids = np.asarray(inputs["char_ids"])
    tags = np.asarray(inputs["tags"])
    wemb = np.asarray(inputs["word_embeddings"], np.float32)
    emb = np.asarray(inputs["char_emb_table"], np.float32)
    trans = np.asarray(inputs["crf_trans"], np.float32)

    alpha = 1.0 / (15.0 * float(np.exp(trans).mean()))
    common = {}
    w1 = np.asarray(inputs["cls_w1"], np.float32)
    common["cls1"] = np.ascontiguousarray(
        w1.reshape(4, 128, 8, 128).transpose(3, 2, 0, 1).reshape(128, 8 * 4 * 128)
    ).astype(np.float16)
    common["clsb1"] = np.ascontiguousarray(
        np.asarray(inputs["cls_b1"], np.float32).reshape(4, 128).T).astype(np.float32)
    w2 = np.asarray(inputs["cls_w2"], np.float32)
    common["cls2"] = np.ascontiguousarray(
        w2.reshape(15, 4, 128).transpose(2, 1, 0).reshape(128, 4 * 15)).astype(np.float16)
    common["clsb2"] = np.asarray(inputs["cls_b2"], np.float32).reshape(15, 1).copy()
    common["mexp"] = np.exp(trans).astype(np.float32)
    common["transn"] = trans.astype(np.float16)
    common["crfstart"] = np.asarray(inputs["crf_start"], np.float32).reshape(15, 1).copy()
    common["crfend"] = np.asarray(inputs["crf_end"], np.float32).reshape(15, 1).copy()
    common["lnalpha"] = np.full((15, 1), np.log(alpha), np.float32)

    in_maps = []
    for c in range(NC_N):
        pw, d, hf = _ROLES[c]
        lo, hi = hf * BL2, (hf + 1) * BL2
        m = dict(common)

        # phase-1 weights/input
        if pw == "c":
            Wih1 = np.zeros((1024, 768), np.float32)
            Wih1[:, :128] = np.asarray(inputs["c0_Wih"], np.float32)[d]
            Whh1 = np.asarray(inputs["c0_Whh"], np.float32)[d]
            b1 = (np.asarray(inputs["c0_bih"], np.float32)[d]
                  + np.asarray(inputs["c0_bhh"], np.float32)[d])
            ce = emb[char_ids[lo:hi]]  # (32, 512, 128)
            X = np.zeros((128, DK1, T, BL2), np.float32)
            X[:, 0] = ce.transpose(2, 1, 0)
            Wl1 = np.asarray(inputs["c1_Wih"], np.float32)[d]
            Whh2 = np.asarray(inputs["c1_Whh"], np.float32)[d]
            b2 = (np.asarray(inputs["c1_bih"], np.float32)[d]
                  + np.asarray(inputs["c1_bhh"], np.float32)[d])
        else:
            Wih1 = np.asarray(inputs["w0_Wih"], np.float32)[d]
            Whh1 = np.asarray(inputs["w0_Whh"], np.float32)[d]
            b1 = (np.asarray(inputs["w0_bih"], np.float32)[d]
                  + np.asarray(inputs["w0_bhh"], np.float32)[d])
            X = wemb[lo:hi].reshape(BL2, T, DK1, 128).transpose(3, 2, 1, 0)
            Wl1 = np.asarray(inputs["w1_Wih"], np.float32)[d]
            Whh2 = np.asarray(inputs["w1_Whh"], np.float32)[d]
            b2 = (np.asarray(inputs["w1_bih"], np.float32)[d]
                  + np.asarray(inputs["w1_bhh"], np.float32)[d])
        if d == 1:  # backward: reverse local time
            X = X[:, :, ::-1]
        m["x1"] = np.ascontiguousarray(X.reshape(128, DK1, T * BL2)).astype(np.float16)
        m["wih1"] = _wih_prep(Wih1, DK1)
        m["whh1"] = _wih_prep(Whh1, 2)
        m["bias1"] = np.ascontiguousarray(b1[_GPERM].reshape(8, 128).T).astype(np.float32)

        # phase-2 weights: columns [own(256) | peer(256)]
        if d == 1:
            Wl1 = Wl1[:, np.r_[256:512, 0:256]]
        m["wih2"] = _wih_prep(Wl1, DK2)
        m["whh2"] = _wih_prep(Whh2, 2)
        m["bias2"] = np.ascontiguousarray(b2[_GPERM].reshape(8, 128).T).astype(np.float32)
        # blend: f-core (d=0) picks AG region 1 (the b-core), b-core picks 0
        msk = np.zeros((128, 2), np.float32)
        msk[:, 1 - d] = 1.0
        m["masks"] = msk

        # phase-3 tags for this core's 8 sequences
        seqs3 = np.r_[4 * c: 4 * c + 4, 32 + 4 * c: 32 + 4 * c + 4]
        oh = (np.arange(K)[:, None, None] == tags[seqs3][None]).astype(np.float32)
        # (15, 8seq, 512t) -> (15, t, b)
        m["tagoneT"] = np.ascontiguousarray(
            oh.transpose(0, 2, 1).reshape(K, TB3)).astype(np.float16)
        in_maps.append(m)
    return in_maps, alpha


def kernel(**inputs):
    nc = _build_nc()
    in_maps, alpha = _make_in_maps(inputs)
    res = run_bass_kernel_spmd(nc, in_maps, core_ids=list(range(NC_N)))
    total = sum(float(res.results[c]["out"][0, 0]) for c in range(NC_N))
    total -= B * (T - 1) * np.log(alpha)
    return np.float32(total / B)


# revision 11
# speedup vs baseline: 3.4089x; 1.3186x over previous
"""BiLSTM dual-pathway + CRF NLL kernel for 8 Trainium2 NeuronCores.

Sharding: direction-parallel for the LSTM recurrences, data-parallel for the
classifier/CRF. Phase 1 runs the four layer-0 directions (char fwd/bwd, word
fwd/bwd) on 8 cores as (direction x batch-half), batch 32 per core, so the
recurrent matmuls run at N=32 instead of N=8 and the sequential chain count
drops from 4096 to 1024 steps. A pairwise AllGather exchanges the L0 hidden
states between fwd/bwd cores, phase 2 runs the four layer-1 directions the
same way, then an 8-rank AllToAll redistributes hidden states to a
data-parallel layout (8 sequences per core) for the classifier and CRF.

SPMD uniformity: every core runs the identical program. Backward directions
receive host-time-reversed inputs; reversed reads of peer hidden states are
fixed negative-stride APs, with host-permuted weight columns absorbing the
f/b role differences. Peer-region selection after the AllGather uses per-core
0/1 blend masks delivered as input data.

The CRF forward scan runs in probability space with a constant per-step
prescale alpha folded into the emission exponentials (corrected analytically
on the host), so each step is one resident-weight 15x15 matmul plus one
vector multiply -- no per-step exp/ln activation-table swaps.
"""

import sys

sys.path.insert(0, "/opt/trn_rl_repo")

import numpy as np

import concourse.bass as bass
import concourse.mybir as mybir
from concourse import bacc
from concourse.bass import ds
from concourse.tile import TileContext
from concourse.bass_utils import run_bass_kernel_spmd

F16 = mybir.dt.float16
F32 = mybir.dt.float32
AF = mybir.ActivationFunctionType

B, T, V, K = 64, 512, 40, 15
NC_N = 8
BL2 = 32            # batch per core in phases 1-2
BL3 = 8             # sequences per core in phase 3
TB3 = T * BL3       # 4096 classifier/CRF columns per core
DK1, DK2 = 6, 4     # input chunks for L0 (word=768, char padded) and L1 (512)
UNROLL = 16
HCOL = T + 1        # h buffer columns per sequence (col 0 = zero init)

_BUILD_CACHE = {}


def _rec_phase(nc, tc, consts, work, stagep, ps_rec, ps_big, h_sb, cst, hc,
               wih, whh, bias, xg_dram, dkn, gemm_src_ap, phase_tag):
    """Emit one GEMM pass (xg to DRAM) + one 512-step recurrence.

    Column order everywhere in the GEMM is b-major within a 16-step block
    (col = b*16 + t_local), so GEMM rhs and psum stay contiguous. The
    recurrence keeps its working h state in `hc` [128, 2, 17, 32] (t-major
    block ring, col 0 = carry-in) so the per-step matmul rhs is contiguous;
    a gpsimd copy per block mirrors h into the seq-major `h_sb`.

    gemm_src_ap(tv) -> list of dkn [128, 512]-column sources for block tv.
    """
    CB = UNROLL * BL2  # columns per time block

    # ---- GEMM: xg[:, m, block] = sum_k wih[k, m]^T X[k, block] + bias ----
    # two blocks per iteration with up-front DMAs so the PE never waits
    with tc.For_i(0, T, 2 * UNROLL) as tv:
        srcsA = gemm_src_ap(tv)
        srcsB = gemm_src_ap(tv + UNROLL)
        for half, srcs in ((0, srcsA), (1, srcsB)):
            for m in range(8):
                ps = ps_big.tile([128, CB], F32, tag="gemmps")
                for k in range(dkn):
                    nc.tensor.matmul(ps, wih[:, k, m], srcs[k],
                                     start=(k == 0), stop=(k == dkn - 1))
                st = stagep.tile([128, CB], F16, tag="xgstage")
                nc.scalar.activation(st, ps, AF.Identity, bias=bias[:, m: m + 1])
                nc.sync.dma_start(
                    out=xg_dram[:, m, ds((tv + half * UNROLL) * BL2, CB)], in_=st)

    # ---- recurrence ----
    nc.vector.memset(cst, 0.0)
    nc.vector.memset(hc[:, :, 0], 0.0)

    with tc.For_i(0, T, UNROLL) as tv:
        xgs = stagep.tile([128, 8, BL2, UNROLL], F16, tag="xgs")
        nc.sync.dma_start(out=xgs.rearrange("p m b t -> p m (b t)"),
                          in_=xg_dram[:, :, ds(tv * BL2, CB)])
        for j in range(UNROLL):
            ps = ps_rec.tile([128, 8, BL2], F32, tag="recps")
            # m-order: i,f (0-3) first, then g~ (6,7), then o (4,5)
            for m in (0, 1, 2, 3, 6, 7, 4, 5):
                for k in range(2):
                    nc.tensor.matmul(ps[:, m], whh[:, k, m],
                                     hc[:, k, j], start=(k == 0), stop=(k == 1))
            g = stagep.tile([128, 8, BL2], F32, tag="g")
            xsl = xgs[:, :, :, j]
            nc.vector.tensor_add(g[:, 0:4], ps[:, 0:4], xsl[:, 0:4])
            sig = stagep.tile([128, 6, BL2], F32, tag="sig")
            nc.scalar.activation(sig[:, 0:4], g[:, 0:4], AF.Sigmoid)
            nc.vector.tensor_mul(cst, cst, sig[:, 2:4])
            nc.vector.tensor_add(g[:, 6:8], ps[:, 6:8], xsl[:, 6:8])
            tgg = stagep.tile([128, 2, BL2], F32, tag="tgg")
            nc.scalar.activation(tgg, g[:, 6:8], AF.Tanh)
            tmp = stagep.tile([128, 2, BL2], F32, tag="tmpig")
            nc.vector.tensor_mul(tmp, sig[:, 0:2], tgg)
            nc.vector.tensor_add(g[:, 4:6], ps[:, 4:6], xsl[:, 4:6])
            nc.scalar.activation(sig[:, 4:6], g[:, 4:6], AF.Sigmoid)
            nc.vector.tensor_add(cst, cst, tmp)
            tch = stagep.tile([128, 2, BL2], F32, tag="tch")
            nc.scalar.activation(tch, cst, AF.Tanh)
            nc.vector.tensor_mul(hc[:, :, j + 1], sig[:, 4:6], tch)
        # mirror the block into the seq-major buffer; carry h into col 0
        nc.gpsimd.tensor_copy(
            h_sb[:, :, :, ds(tv + 1, UNROLL)],
            hc[:, :, 1: UNROLL + 1].rearrange("p c t b -> p c b t"))
        nc.gpsimd.tensor_copy(hc[:, :, 0], hc[:, :, UNROLL])


def _build_nc():
    if "nc" in _BUILD_CACHE:
        return _BUILD_CACHE["nc"]
    nc = bacc.Bacc(target_bir_lowering=False, num_devices=NC_N)

    # ---- external parameters -------------------------------------------------
    x1_ext = nc.declare_dram_parameter("x1", [128, DK1, T * BL2], F16, isOutput=False)
    wih1_ext = nc.declare_dram_parameter("wih1", [128, DK1 * 8 * 128], F16, isOutput=False)
    whh1_ext = nc.declare_dram_parameter("whh1", [128, 2 * 8 * 128], F16, isOutput=False)
    bias1_ext = nc.declare_dram_parameter("bias1", [128, 8], F32, isOutput=False)
    wih2_ext = nc.declare_dram_parameter("wih2", [128, DK2 * 8 * 128], F16, isOutput=False)
    whh2_ext = nc.declare_dram_parameter("whh2", [128, 2 * 8 * 128], F16, isOutput=False)
    bias2_ext = nc.declare_dram_parameter("bias2", [128, 8], F32, isOutput=False)
    masks_ext = nc.declare_dram_parameter("masks", [128, 2], F32, isOutput=False)
    cls1_ext = nc.declare_dram_parameter("cls1", [128, 8 * 4 * 128], F16, isOutput=False)
    clsb1_ext = nc.declare_dram_parameter("clsb1", [128, 4], F32, isOutput=False)
    cls2_ext = nc.declare_dram_parameter("cls2", [128, 4 * 15], F16, isOutput=False)
    clsb2_ext = nc.declare_dram_parameter("clsb2", [15, 1], F32, isOutput=False)
    mexp_ext = nc.declare_dram_parameter("mexp", [15, 15], F32, isOutput=False)
    transn_ext = nc.declare_dram_parameter("transn", [15, 15], F16, isOutput=False)
    start_ext = nc.declare_dram_parameter("crfstart", [15, 1], F32, isOutput=False)
    end_ext = nc.declare_dram_parameter("crfend", [15, 1], F32, isOutput=False)
    lna_ext = nc.declare_dram_parameter("lnalpha", [15, 1], F32, isOutput=False)
    tago_ext = nc.declare_dram_parameter("tagoneT", [15, TB3], F16, isOutput=False)
    out_ext = nc.declare_dram_parameter("out", [1, 1], F32, isOutput=True)

    # ---- internal DRAM -------------------------------------------------------
    HSZ = 2 * BL2 * T  # 32768 cols/partition of h (f16)
    xg1_dram = nc.dram_tensor("xg1", [128, 8, T * BL2], F16)
    xg2_dram = nc.dram_tensor("xg2", [128, 8, T * BL2], F16)
    hmine = nc.dram_tensor("hmine", [128, HSZ], F16)
    agout = nc.dram_tensor("agout", [2, 128, HSZ], F16)
    sendb = nc.dram_tensor("sendb", [8, 128, 2 * 4 * T], F16)
    recvb = nc.dram_tensor("recvb", [8, 128, 2 * 4 * T], F16)

    with TileContext(nc) as tc:
        with (
            tc.tile_pool(name="consts", bufs=1) as consts,
            tc.tile_pool(name="seqs", bufs=1) as seqs,
            tc.tile_pool(name="work", bufs=2) as work,
            tc.tile_pool(name="stage", bufs=2) as stagep,
            tc.tile_pool(name="ps_big", bufs=2, space="PSUM") as ps_big,
            tc.tile_pool(name="ps_rec", bufs=2, space="PSUM") as ps_rec,
            tc.tile_pool(name="ps_small", bufs=3, space="PSUM") as ps_small,
        ):
            # h buffer, seq-major: [128, chunk2, b32, T+1], reused by phases 1+2
            h_sb = seqs.tile([128, 2, BL2, HCOL], F16, tag="h_sb")
            hc = seqs.tile([128, 2, UNROLL + 1, BL2], F16, tag="hcomp")
            cst = seqs.tile([128, 2, BL2], F32, tag="cstate")
            masks = consts.tile([128, 2], F32, tag="masks")
            nc.sync.dma_start(out=masks, in_=masks_ext[:, :])

            # ================= PHASE 1 (L0) =================
            wih1 = consts.tile([128, DK1, 8, 128], F16, tag="wbig")
            nc.sync.dma_start(
                out=wih1,
                in_=wih1_ext.ap().rearrange("p (k m c) -> p k m c", k=DK1, m=8))
            whh1 = consts.tile([128, 2, 8, 128], F16, tag="whh")
            nc.sync.dma_start(
                out=whh1,
                in_=whh1_ext.ap().rearrange("p (k m c) -> p k m c", k=2, m=8))
            bias1 = consts.tile([128, 8], F32, tag="bias1")
            nc.sync.dma_start(out=bias1, in_=bias1_ext[:, :])

            CB = UNROLL * BL2

            def gemm1_src(tv):
                xb = stagep.tile([128, DK1, CB], F16, tag="xb1")
                nc.sync.dma_start(out=xb, in_=x1_ext[:, :, ds(tv * BL2, CB)])
                return [xb[:, k] for k in range(DK1)]  # cols already b-major

            _rec_phase(nc, tc, consts, work, stagep, ps_rec, ps_big, h_sb, cst,
                       hc, wih1, whh1, bias1, xg1_dram, DK1, gemm1_src, "p1")

            # h (cols 1..T+1) -> local DRAM, then pairwise AllGather
            hm_view = hmine.ap().rearrange("p (c b t) -> p c b t", c=2, b=BL2)
            for c in range(2):
                nc.sync.dma_start(out=hm_view[:, c], in_=h_sb[:, c, :, 1:HCOL])
            nc.gpsimd.collective_compute(
                "AllGather", mybir.AluOpType.bypass,
                replica_groups=[[0, 2], [1, 3], [4, 6], [5, 7]],
                ins=[hmine.ap()], outs=[agout.ap()],
            )

            # ================= PHASE 2 (L1) =================
            wih2 = consts.tile([128, DK2, 8, 128], F16, tag="wbig")
            nc.sync.dma_start(
                out=wih2,
                in_=wih2_ext.ap().rearrange("p (k m c) -> p k m c", k=DK2, m=8))
            whh2 = consts.tile([128, 2, 8, 128], F16, tag="whh")
            nc.sync.dma_start(
                out=whh2,
                in_=whh2_ext.ap().rearrange("p (k m c) -> p k m c", k=2, m=8))
            bias2 = consts.tile([128, 8], F32, tag="bias2")
            nc.sync.dma_start(out=bias2, in_=bias2_ext[:, :])

            def slot_ap(tensor_offset, tensor, c, tv, reverse):
                # [128, 32, UNROLL] chunk c of an h-layout DRAM region
                # (p, c2, b32, T), always read as an ascending t window;
                # reversed slots flip t in the SBUF-side rhs view instead.
                if not reverse:
                    off = tensor_offset + c * (BL2 * T) + tv
                else:
                    off = tensor_offset + c * (BL2 * T) + (T - UNROLL) - tv
                return bass.AP(tensor=tensor, offset=off,
                               ap=[[HSZ, 128], [T, BL2], [1, UNROLL]])

            def gemm2_src(tv):
                # slot tiles are b-major [128, c2, b32, t16] to match the
                # DRAM h layout; the matmul rhs view re-orders to (t, b)
                sa = stagep.tile([128, 2, BL2, UNROLL], F16, tag="slotA")
                for c in range(2):
                    nc.sync.dma_start(out=sa[:, c], in_=slot_ap(0, hmine, c, tv, False))
                # slotB: blend of the two AG regions, read time-reversed
                r0 = stagep.tile([128, 2, BL2, UNROLL], F16, tag="slotR0")
                r1 = stagep.tile([128, 2, BL2, UNROLL], F16, tag="slotR1")
                for c in range(2):
                    nc.sync.dma_start(out=r0[:, c], in_=slot_ap(0, agout, c, tv, True))
                    nc.sync.dma_start(out=r1[:, c], in_=slot_ap(128 * HSZ, agout, c, tv, True))
                sb = stagep.tile([128, 2, BL2, UNROLL], F16, tag="slotB")
                sbr = stagep.tile([128, 2, BL2, UNROLL], F16, tag="slotBr")
                nc.vector.tensor_scalar_mul(sb, r0, masks[:, 0:1])
                nc.vector.tensor_scalar_mul(r1, r1, masks[:, 1:2])
                # add, writing with t reversed within the block so the rhs
                # below is a plain contiguous forward read
                p_step = sbr.ap[0][0]
                rev = bass.AP(tensor=sbr.tensor,
                              offset=sbr.offset + (UNROLL - 1),
                              ap=[[p_step, 128], [BL2 * UNROLL, 2],
                                  [UNROLL, BL2], [-1, UNROLL]])
                nc.vector.tensor_add(rev, sb, r1)

                def flat(tile, c):
                    return tile[:, c].rearrange("p b t -> p (b t)")

                return [flat(sa, 0), flat(sa, 1), flat(sbr, 0), flat(sbr, 1)]

            _rec_phase(nc, tc, consts, work, stagep, ps_rec, ps_big, h_sb, cst,
                       hc, wih2, whh2, bias2, xg2_dram, DK2, gemm2_src, "p2")

            # ---- AllToAll redistribution to data-parallel layout ----
            sb_view = sendb.ap().rearrange("j p (c b t) -> j p c b t", c=2, b=4)
            for j in range(8):
                for c in range(2):
                    nc.sync.dma_start(out=sb_view[j, :, c],
                                      in_=h_sb[:, c, ds(4 * j, 4), 1:HCOL])
            nc.gpsimd.collective_compute(
                "AllToAll", mybir.AluOpType.bypass,
                replica_groups=[list(range(8))],
                ins=[sendb.ap()], outs=[recvb.ap()],
            )

            # ================= PHASE 3: classifier + CRF =================
            cls1 = consts.tile([128, 8, 4, 128], F16, tag="wbig")
            nc.sync.dma_start(
                out=cls1,
                in_=cls1_ext.ap().rearrange("p (k m c) -> p k m c", k=8, m=4))
            clsb1 = consts.tile([128, 4], F32, tag="clsb1")
            nc.sync.dma_start(out=clsb1, in_=clsb1_ext[:, :])
            cls2 = consts.tile([128, 4, 15], F16, tag="cls2")
            nc.sync.dma_start(
                out=cls2, in_=cls2_ext.ap().rearrange("p (k j) -> p k j", k=4))
            clsb2 = consts.tile([15, 1], F32, tag="clsb2")
            nc.sync.dma_start(out=clsb2, in_=clsb2_ext[:, :])
            mexp = consts.tile([15, 15], F32, tag="mexp")
            nc.sync.dma_start(out=mexp, in_=mexp_ext[:, :])
            transn = consts.tile([15, 15], F16, tag="transn")
            nc.sync.dma_start(out=transn, in_=transn_ext[:, :])
            crfstart = consts.tile([15, 1], F32, tag="crfstart")
            nc.sync.dma_start(out=crfstart, in_=start_ext[:, :])
            crfend = consts.tile([15, 1], F32, tag="crfend")
            nc.sync.dma_start(out=crfend, in_=end_ext[:, :])
            lnalpha = consts.tile([15, 1], F32, tag="lnalpha")
            nc.sync.dma_start(out=lnalpha, in_=lna_ext[:, :])
            tago = consts.tile([15, TB3], F16, tag="tago")
            nc.sync.dma_start(out=tago, in_=tago_ext[:, :])

            logits = seqs.tile([15, TB3], F32, tag="logits")

            NT = 64  # t-steps per classifier n-tile (NT*BL3 = 512 cols)
            SHSZ = 128 * 2 * 4 * T  # elements per recv shard

            def comb_ap(kk, half, ns):
                # [128, 4, NT]: dir kk//2, chunk kk%2, half-shard, n-tile ns;
                # always an ascending t window (reversal done in the rhs view)
                d, c = kk // 2, kk % 2
                rev = d in (1, 3)  # c1b, w1b stored time-reversed
                base = (2 * d + half) * SHSZ + c * (4 * T)
                if not rev:
                    off = base + ns * NT
                else:
                    off = base + (T - NT) - ns * NT
                return bass.AP(tensor=recvb, offset=off,
                               ap=[[2 * 4 * T, 128], [T, 4], [1, NT]])

            for ns in range(8):
                comb = stagep.tile([128, 8, BL3, NT], F16, tag="comb", bufs=2)
                for kk in range(8):
                    for half in range(2):
                        nc.sync.dma_start(out=comb[:, kk, ds(4 * half, 4)],
                                          in_=comb_ap(kk, half, ns))
                hmt = []
                for m in range(4):
                    ps = ps_big.tile([128, NT * BL3], F32, tag="gemmps")
                    for kk in range(8):
                        if kk // 2 in (1, 3):
                            p_step = comb.ap[0][0]
                            off = (comb.offset + kk * BL3 * NT + (NT - 1))
                            rhs = bass.AP(tensor=comb.tensor, offset=off,
                                          ap=[[p_step, 128], [-1, NT],
                                              [NT, BL3]])
                        else:
                            rhs = comb[:, kk].rearrange("p b t -> p t b")
                        nc.tensor.matmul(ps, cls1[:, kk, m], rhs,
                                         start=(kk == 0), stop=(kk == 7))
                    hm = stagep.tile([128, NT * BL3], F16, tag="hm", bufs=4,
                                     name=f"hm{m}")
                    nc.scalar.activation(hm, ps, AF.Relu, bias=clsb1[:, m: m + 1])
                    hmt.append(hm)
                ps2 = ps_small.tile([15, NT * BL3], F32, tag="small")
                for m in range(4):
                    nc.tensor.matmul(ps2, cls2[:, m], hmt[m],
                                     start=(m == 0), stop=(m == 3))
                nc.vector.tensor_scalar_add(
                    logits[:, ds(ns * NT * BL3, NT * BL3)], ps2, clsb2)

            # fold CRF start/end into first/last emission columns
            nc.vector.tensor_scalar_add(logits[:, 0:BL3], logits[:, 0:BL3], crfstart)
            nc.vector.tensor_scalar_add(logits[:, TB3 - BL3: TB3],
                                        logits[:, TB3 - BL3: TB3], crfend)

            # ---- CRF numerator ----
            racc = work.tile([15, 16], F32, tag="racc")
            nc.vector.memset(racc, 0.0)
            for ns in range(8):
                pre = stagep.tile([15, 512], F32, tag="prodns")
                nc.vector.tensor_mul(pre, logits[:, ds(ns * 512, 512)],
                                     tago[:, ds(ns * 512, 512)])
                nc.vector.tensor_reduce(racc[:, 8 + ns: 9 + ns], pre,
                                        axis=mybir.AxisListType.X,
                                        op=mybir.AluOpType.add)
                psv = ps_small.tile([15, 512], F32, tag="small")
                nc.tensor.matmul(psv, transn, tago[:, ds(ns * 512, 512)],
                                 start=True, stop=True)
                w = 512 if ns < 7 else 512 - BL3
                pr = stagep.tile([15, 512], F32, tag="prodns")
                nc.vector.tensor_mul(pr[:, :w], psv[:, :w],
                                     tago[:, ds(ns * 512 + BL3, w)])
                nc.vector.tensor_reduce(racc[:, ns: ns + 1], pr[:, :w],
                                        axis=mybir.AxisListType.X,
                                        op=mybir.AluOpType.add)
            nv = stagep.tile([15, 1], F32, tag="nv")
            nc.vector.tensor_reduce(nv, racc, axis=mybir.AxisListType.X,
                                    op=mybir.AluOpType.add)
            ones15 = consts.tile([15, 1], F32, tag="ones15")
            nc.vector.memset(ones15, 1.0)
            psn = ps_small.tile([1, 1], F32, tag="small")
            nc.tensor.matmul(psn, ones15, nv, start=True, stop=True)
            num11 = work.tile([1, 1], F32, tag="num11")
            nc.vector.tensor_copy(num11, psn)

            # ---- CRF forward scan, probability space ----
            # p0 first, then E = alpha*exp(logits) computed in place over
            # logits (last col already includes e^end)
            p = seqs.tile([15, BL3], F32, tag="pvec")
            nc.scalar.activation(p, logits[:, 0:BL3], AF.Exp)
            E = logits
            nc.scalar.activation(E, logits, AF.Exp, bias=lnalpha)

            def crf_step(t_col_ap):
                z = ps_small.tile([15, BL3], F32, tag="small")
                nc.tensor.matmul(z, mexp, p, start=True, stop=True)
                nc.vector.tensor_mul(p, z, t_col_ap)

            for t in range(1, 16):
                crf_step(E[:, t * BL3: (t + 1) * BL3])
            with tc.For_i(0, 496, UNROLL) as tv:
                for j in range(UNROLL):
                    crf_step(E[:, ds((16 + j) * BL3 + tv * BL3, BL3)])

            # ---- denominator + output ----
            psd = ps_small.tile([1, BL3], F32, tag="small")
            nc.tensor.matmul(psd, ones15, p, start=True, stop=True)
            ln8 = stagep.tile([1, BL3], F32, tag="ln8")
            nc.scalar.activation(ln8, psd, AF.Ln)
            den11 = work.tile([1, 1], F32, tag="den11")
            nc.vector.tensor_reduce(den11, ln8, axis=mybir.AxisListType.X,
                                    op=mybir.AluOpType.add)
            res = work.tile([1, 1], F32, tag="res")
            nc.vector.tensor_sub(res, den11, num11)
            nc.sync.dma_start(out=out_ext[:, :], in_=res)

    nc.finalize()
    _BUILD_CACHE["nc"] = nc
    return nc


# ---- host-side input prep ---------------------------------------------------

# gate perm [i(256), f(256), g(256), o(256)] -> [i, f, o, g~]
_GPERM = np.concatenate([np.arange(0, 512), np.arange(768, 1024), np.arange(512, 768)])

# core c -> (pathway, direction, half): 0..3 char f/f/b/b, 4..7 word
_ROLES = [("c", 0, 0), ("c", 0, 1), ("c", 1, 0), ("c", 1, 1),
          ("w", 0, 0), ("w", 0, 1), ("w", 1, 0), ("w", 1, 1)]


def _wih_prep(W, dk_n):
    Wp = W[_GPERM]
    return np.ascontiguousarray(
        Wp.reshape(8, 128, dk_n, 128).transpose(3, 2, 0, 1).reshape(128, dk_n * 8 * 128)
    ).astype(np.float16)


def _make_in_maps(inputs):
    char_ids = np.asarray(inputs["char_ids"])
    tags = np.asarray(inputs["tags"])
    wemb = np.asarray(inputs["word_embeddings"], np.float32)
    emb = np.asarray(inputs["char_emb_table"], np.float32)
    trans = np.asarray(inputs["crf_trans"], np.float32)

    alpha = 1.0 / (15.0 * float(np.exp(trans).mean()))
    common = {}
    w1 = np.asarray(inputs["cls_w1"], np.float32)
    common["cls1"] = np.ascontiguousarray(
        w1.reshape(4, 128, 8, 128).transpose(3, 2, 0, 1).reshape(128, 8 * 4 * 128)
    ).astype(np.float16)
    common["clsb1"] = np.ascontiguousarray(
        np.asarray(inputs["cls_b1"], np.float32).reshape(4, 128).T).astype(np.float32)
    w2 = np.asarray(inputs["cls_w2"], np.float32)
    common["cls2"] = np.ascontiguousarray(
        w2.reshape(15, 4, 128).transpose(2, 1, 0).reshape(128, 4 * 15)).astype(np.float16)
    common["clsb2"] = np.asarray(inputs["cls_b2"], np.float32).reshape(15, 1).copy()
    common["mexp"] = np.exp(trans).astype(np.float32)
    common["transn"] = trans.astype(np.float16)
    common["crfstart"] = np.asarray(inputs["crf_start"], np.float32).reshape(15, 1).copy()
    common["crfend"] = np.asarray(inputs["crf_end"], np.float32).reshape(15, 1).copy()
    common["lnalpha"] = np.full((15, 1), np.log(alpha), np.float32)

    in_maps = []
    for c in range(NC_N):
        pw, d, hf = _ROLES[c]
        lo, hi = hf * BL2, (hf + 1) * BL2
        m = dict(common)

        # phase-1 weights/input
        if pw == "c":
            Wih1 = np.zeros((1024, 768), np.float32)
            Wih1[:, :128] = np.asarray(inputs["c0_Wih"], np.float32)[d]
            Whh1 = np.asarray(inputs["c0_Whh"], np.float32)[d]
            b1 = (np.asarray(inputs["c0_bih"], np.float32)[d]
                  + np.asarray(inputs["c0_bhh"], np.float32)[d])
            ce = emb[char_ids[lo:hi]]  # (32, 512, 128)
            X = np.zeros((128, DK1, T, BL2), np.float32)
            X[:, 0] = ce.transpose(2, 1, 0)
            Wl1 = np.asarray(inputs["c1_Wih"], np.float32)[d]
            Whh2 = np.asarray(inputs["c1_Whh"], np.float32)[d]
            b2 = (np.asarray(inputs["c1_bih"], np.float32)[d]
                  + np.asarray(inputs["c1_bhh"], np.float32)[d])
        else:
            Wih1 = np.asarray(inputs["w0_Wih"], np.float32)[d]
            Whh1 = np.asarray(inputs["w0_Whh"], np.float32)[d]
            b1 = (np.asarray(inputs["w0_bih"], np.float32)[d]
                  + np.asarray(inputs["w0_bhh"], np.float32)[d])
            X = wemb[lo:hi].reshape(BL2, T, DK1, 128).transpose(3, 2, 1, 0)
            Wl1 = np.asarray(inputs["w1_Wih"], np.float32)[d]
            Whh2 = np.asarray(inputs["w1_Whh"], np.float32)[d]
            b2 = (np.asarray(inputs["w1_bih"], np.float32)[d]
                  + np.asarray(inputs["w1_bhh"], np.float32)[d])
        if d == 1:  # backward: reverse local time
            X = X[:, :, ::-1]
        # blockify to b-major within 16-step blocks: col = blk*512 + b*16 + t
        Xb = X.reshape(128, DK1, T // UNROLL, UNROLL, BL2).transpose(0, 1, 2, 4, 3)
        m["x1"] = np.ascontiguousarray(Xb.reshape(128, DK1, T * BL2)).astype(np.float16)
        m["wih1"] = _wih_prep(Wih1, DK1)
        m["whh1"] = _wih_prep(Whh1, 2)
        m["bias1"] = np.ascontiguousarray(b1[_GPERM].reshape(8, 128).T).astype(np.float32)

        # phase-2 weights: columns [own(256) | peer(256)]
        if d == 1:
            Wl1 = Wl1[:, np.r_[256:512, 0:256]]
        m["wih2"] = _wih_prep(Wl1, DK2)
        m["whh2"] = _wih_prep(Whh2, 2)
        m["bias2"] = np.ascontiguousarray(b2[_GPERM].reshape(8, 128).T).astype(np.float32)
        # blend: f-core (d=0) picks AG region 1 (the b-core), b-core picks 0
        msk = np.zeros((128, 2), np.float32)
        msk[:, 1 - d] = 1.0
        m["masks"] = msk

        # phase-3 tags for this core's 8 sequences
        seqs3 = np.r_[4 * c: 4 * c + 4, 32 + 4 * c: 32 + 4 * c + 4]
        oh = (np.arange(K)[:, None, None] == tags[seqs3][None]).astype(np.float32)
        # (15, 8seq, 512t) -> (15, t, b)
        m["tagoneT"] = np.ascontiguousarray(
            oh.transpose(0, 2, 1).reshape(K, TB3)).astype(np.float16)
        in_maps.append(m)
    return in_maps, alpha


def kernel(**inputs):
    nc = _build_nc()
    in_maps, alpha = _make_in_maps(inputs)
    res = run_bass_kernel_spmd(nc, in_maps, core_ids=list(range(NC_N)))
    total = sum(float(res.results[c]["out"][0, 0]) for c in range(NC_N))
    total -= B * (T - 1) * np.log(alpha)
    return np.float32(total / B)


# revision 12
# speedup vs baseline: 3.5305x; 1.0357x over previous
"""BiLSTM dual-pathway + CRF NLL kernel for 8 Trainium2 NeuronCores.

Sharding: direction-parallel for the LSTM recurrences, data-parallel for the
classifier/CRF. Phase 1 runs the four layer-0 directions (char fwd/bwd, word
fwd/bwd) on 8 cores as (direction x batch-half), batch 32 per core, so the
recurrent matmuls run at N=32 instead of N=8 and the sequential chain count
drops from 4096 to 1024 steps. A pairwise AllGather exchanges the L0 hidden
states between fwd/bwd cores, phase 2 runs the four layer-1 directions the
same way, then an 8-rank AllToAll redistributes hidden states to a
data-parallel layout (8 sequences per core) for the classifier and CRF.

SPMD uniformity: every core runs the identical program. Backward directions
receive host-time-reversed inputs; reversed reads of peer hidden states are
fixed negative-stride APs, with host-permuted weight columns absorbing the
f/b role differences. Peer-region selection after the AllGather uses per-core
0/1 blend masks delivered as input data.

The CRF forward scan runs in probability space with a constant per-step
prescale alpha folded into the emission exponentials (corrected analytically
on the host), so each step is one resident-weight 15x15 matmul plus one
vector multiply -- no per-step exp/ln activation-table swaps.
"""

import sys

sys.path.insert(0, "/opt/trn_rl_repo")

import numpy as np

import concourse.bass as bass
import concourse.mybir as mybir
from concourse import bacc
from concourse.bass import ds
from concourse.tile import TileContext
from concourse.bass_utils import run_bass_kernel_spmd

F16 = mybir.dt.float16
F32 = mybir.dt.float32
AF = mybir.ActivationFunctionType

B, T, V, K = 64, 512, 40, 15
NC_N = 8
BL2 = 32            # batch per core in phases 1-2
BL3 = 8             # sequences per core in phase 3
TB3 = T * BL3       # 4096 classifier/CRF columns per core
DK1, DK2 = 6, 4     # input chunks for L0 (word=768, char padded) and L1 (512)
UNROLL = 16
HCOL = T + 1        # h buffer columns per sequence (col 0 = zero init)

_BUILD_CACHE = {}


def _rec_phase(nc, tc, consts, work, stagep, ps_rec, ps_big, h_sb, cst, hc,
               wih, whh, bias, xg_dram, dkn, gemm_src_ap, phase_tag):
    """Emit one GEMM pass (xg to DRAM) + one 512-step recurrence.

    Column order everywhere in the GEMM is b-major within a 16-step block
    (col = b*16 + t_local), so GEMM rhs and psum stay contiguous. The
    recurrence keeps its working h state in `hc` [128, 2, 17, 32] (t-major
    block ring, col 0 = carry-in) so the per-step matmul rhs is contiguous;
    a gpsimd copy per block mirrors h into the seq-major `h_sb`.

    gemm_src_ap(tv) -> list of dkn [128, 512]-column sources for block tv.
    """
    CB = UNROLL * BL2  # columns per time block

    # ---- GEMM: xg[:, m, block] = sum_k wih[k, m]^T X[k, block] + bias ----
    # two blocks per iteration with up-front DMAs so the PE never waits
    with tc.For_i(0, T, 2 * UNROLL) as tv:
        srcsA = gemm_src_ap(tv)
        srcsB = gemm_src_ap(tv + UNROLL)
        for half, srcs in ((0, srcsA), (1, srcsB)):
            for m in range(8):
                ps = ps_big.tile([128, CB], F32, tag="gemmps")
                for k in range(dkn):
                    nc.tensor.matmul(ps, wih[:, k, m], srcs[k],
                                     start=(k == 0), stop=(k == dkn - 1))
                st = stagep.tile([128, CB], F16, tag="xgstage")
                nc.scalar.activation(st, ps, AF.Identity, bias=bias[:, m: m + 1])
                nc.sync.dma_start(
                    out=xg_dram[:, m, ds((tv + half * UNROLL) * BL2, CB)], in_=st)

    # ---- recurrence ----
    nc.vector.memset(cst, 0.0)
    nc.vector.memset(hc[:, :, 0], 0.0)

    with tc.For_i(0, T, UNROLL) as tv:
        xgs = stagep.tile([128, 8, CB], F16, tag="xgs")
        nc.sync.dma_start(out=xgs, in_=xg_dram[:, :, ds(tv * BL2, CB)])
        for j in range(UNROLL):
            ps = ps_rec.tile([128, 8, BL2], F32, tag="recps")
            # m-order: i,f (0-3) first, then g~ (6,7), then o (4,5)
            for m in (0, 1, 2, 3, 6, 7, 4, 5):
                for k in range(2):
                    nc.tensor.matmul(ps[:, m], whh[:, k, m],
                                     hc[:, k, j], start=(k == 0), stop=(k == 1))
            g = stagep.tile([128, 8, BL2], F32, tag="g")
            xsl = xgs[:, :, ds(j * BL2, BL2)]
            nc.vector.tensor_add(g[:, 0:4], ps[:, 0:4], xsl[:, 0:4])
            sig = stagep.tile([128, 6, BL2], F32, tag="sig")
            nc.scalar.activation(sig[:, 0:4], g[:, 0:4], AF.Sigmoid)
            nc.vector.tensor_mul(cst, cst, sig[:, 2:4])
            nc.vector.tensor_add(g[:, 4:8], ps[:, 4:8], xsl[:, 4:8])
            tgg = stagep.tile([128, 2, BL2], F32, tag="tgg")
            nc.scalar.activation(tgg, g[:, 6:8], AF.Tanh)
            tmp = stagep.tile([128, 2, BL2], F32, tag="tmpig")
            nc.vector.tensor_mul(tmp, sig[:, 0:2], tgg)
            nc.scalar.activation(sig[:, 4:6], g[:, 4:6], AF.Sigmoid)
            nc.vector.tensor_add(cst, cst, tmp)
            tch = stagep.tile([128, 2, BL2], F32, tag="tch")
            nc.scalar.activation(tch, cst, AF.Tanh)
            nc.vector.tensor_mul(hc[:, :, j + 1], sig[:, 4:6], tch)
        # mirror the block into the seq-major buffer; carry h into col 0
        nc.gpsimd.tensor_copy(
            h_sb[:, :, :, ds(tv + 1, UNROLL)],
            hc[:, :, 1: UNROLL + 1].rearrange("p c t b -> p c b t"))
        nc.gpsimd.tensor_copy(hc[:, :, 0], hc[:, :, UNROLL])


def _build_nc():
    if "nc" in _BUILD_CACHE:
        return _BUILD_CACHE["nc"]
    nc = bacc.Bacc(target_bir_lowering=False, num_devices=NC_N)

    # ---- external parameters -------------------------------------------------
    x1_ext = nc.declare_dram_parameter("x1", [128, DK1, T * BL2], F16, isOutput=False)
    wih1_ext = nc.declare_dram_parameter("wih1", [128, DK1 * 8 * 128], F16, isOutput=False)
    whh1_ext = nc.declare_dram_parameter("whh1", [128, 2 * 8 * 128], F16, isOutput=False)
    bias1_ext = nc.declare_dram_parameter("bias1", [128, 8], F32, isOutput=False)
    wih2_ext = nc.declare_dram_parameter("wih2", [128, DK2 * 8 * 128], F16, isOutput=False)
    whh2_ext = nc.declare_dram_parameter("whh2", [128, 2 * 8 * 128], F16, isOutput=False)
    bias2_ext = nc.declare_dram_parameter("bias2", [128, 8], F32, isOutput=False)
    masks_ext = nc.declare_dram_parameter("masks", [128, 2], F32, isOutput=False)
    cls1_ext = nc.declare_dram_parameter("cls1", [128, 8 * 4 * 128], F16, isOutput=False)
    clsb1_ext = nc.declare_dram_parameter("clsb1", [128, 4], F32, isOutput=False)
    cls2_ext = nc.declare_dram_parameter("cls2", [128, 4 * 15], F16, isOutput=False)
    clsb2_ext = nc.declare_dram_parameter("clsb2", [15, 1], F32, isOutput=False)
    mexp_ext = nc.declare_dram_parameter("mexp", [15, 15], F32, isOutput=False)
    transn_ext = nc.declare_dram_parameter("transn", [15, 15], F16, isOutput=False)
    start_ext = nc.declare_dram_parameter("crfstart", [15, 1], F32, isOutput=False)
    end_ext = nc.declare_dram_parameter("crfend", [15, 1], F32, isOutput=False)
    lna_ext = nc.declare_dram_parameter("lnalpha", [15, 1], F32, isOutput=False)
    tago_ext = nc.declare_dram_parameter("tagoneT", [15, TB3], F16, isOutput=False)
    out_ext = nc.declare_dram_parameter("out", [1, 1], F32, isOutput=True)

    # ---- internal DRAM -------------------------------------------------------
    HSZ = 2 * BL2 * T  # 32768 cols/partition of h (f16)
    xg1_dram = nc.dram_tensor("xg1", [128, 8, T * BL2], F16)
    xg2_dram = nc.dram_tensor("xg2", [128, 8, T * BL2], F16)
    hmine = nc.dram_tensor("hmine", [128, HSZ], F16)
    agout = nc.dram_tensor("agout", [2, 128, HSZ], F16)
    sendb = nc.dram_tensor("sendb", [8, 128, 2 * 4 * T], F16)
    recvb = nc.dram_tensor("recvb", [8, 128, 2 * 4 * T], F16)

    with TileContext(nc) as tc:
        with (
            tc.tile_pool(name="consts", bufs=1) as consts,
            tc.tile_pool(name="seqs", bufs=1) as seqs,
            tc.tile_pool(name="work", bufs=2) as work,
            tc.tile_pool(name="stage", bufs=2) as stagep,
            tc.tile_pool(name="ps_big", bufs=2, space="PSUM") as ps_big,
            tc.tile_pool(name="ps_rec", bufs=2, space="PSUM") as ps_rec,
            tc.tile_pool(name="ps_small", bufs=3, space="PSUM") as ps_small,
        ):
            # h buffer, seq-major: [128, chunk2, b32, T+1], reused by phases 1+2
            h_sb = seqs.tile([128, 2, BL2, HCOL], F16, tag="h_sb")
            hc = seqs.tile([128, 2, UNROLL + 1, BL2], F16, tag="hcomp")
            cst = seqs.tile([128, 2, BL2], F32, tag="cstate")
            masks = consts.tile([128, 2], F32, tag="masks")
            nc.sync.dma_start(out=masks, in_=masks_ext[:, :])

            # ================= PHASE 1 (L0) =================
            wih1 = consts.tile([128, DK1, 8, 128], F16, tag="wbig")
            nc.sync.dma_start(
                out=wih1,
                in_=wih1_ext.ap().rearrange("p (k m c) -> p k m c", k=DK1, m=8))
            whh1 = consts.tile([128, 2, 8, 128], F16, tag="whh")
            nc.sync.dma_start(
                out=whh1,
                in_=whh1_ext.ap().rearrange("p (k m c) -> p k m c", k=2, m=8))
            bias1 = consts.tile([128, 8], F32, tag="bias1")
            nc.sync.dma_start(out=bias1, in_=bias1_ext[:, :])

            CB = UNROLL * BL2

            def gemm1_src(tv):
                xb = stagep.tile([128, DK1, CB], F16, tag="xb1")
                nc.sync.dma_start(out=xb, in_=x1_ext[:, :, ds(tv * BL2, CB)])
                return [xb[:, k] for k in range(DK1)]  # cols already b-major

            _rec_phase(nc, tc, consts, work, stagep, ps_rec, ps_big, h_sb, cst,
                       hc, wih1, whh1, bias1, xg1_dram, DK1, gemm1_src, "p1")

            # h (cols 1..T+1) -> local DRAM, then pairwise AllGather
            hm_view = hmine.ap().rearrange("p (c b t) -> p c b t", c=2, b=BL2)
            for c in range(2):
                nc.sync.dma_start(out=hm_view[:, c], in_=h_sb[:, c, :, 1:HCOL])
            nc.gpsimd.collective_compute(
                "AllGather", mybir.AluOpType.bypass,
                replica_groups=[[0, 2], [1, 3], [4, 6], [5, 7]],
                ins=[hmine.ap()], outs=[agout.ap()],
            )

            # ================= PHASE 2 (L1) =================
            wih2 = consts.tile([128, DK2, 8, 128], F16, tag="wbig")
            nc.sync.dma_start(
                out=wih2,
                in_=wih2_ext.ap().rearrange("p (k m c) -> p k m c", k=DK2, m=8))
            whh2 = consts.tile([128, 2, 8, 128], F16, tag="whh")
            nc.sync.dma_start(
                out=whh2,
                in_=whh2_ext.ap().rearrange("p (k m c) -> p k m c", k=2, m=8))
            bias2 = consts.tile([128, 8], F32, tag="bias2")
            nc.sync.dma_start(out=bias2, in_=bias2_ext[:, :])

            def slot_ap(tensor_offset, tensor, c, tv, reverse):
                # [128, 32, UNROLL] chunk c of an h-layout DRAM region
                # (p, c2, b32, T), always read as an ascending t window;
                # reversed slots flip t in the SBUF-side rhs view instead.
                if not reverse:
                    off = tensor_offset + c * (BL2 * T) + tv
                else:
                    off = tensor_offset + c * (BL2 * T) + (T - UNROLL) - tv
                return bass.AP(tensor=tensor, offset=off,
                               ap=[[HSZ, 128], [T, BL2], [1, UNROLL]])

            def gemm2_src(tv):
                # DMA'd b-major [128, c2, b32, t16]; the blend/copy step
                # rewrites into t-major tiles so the rhs is (t, b) contiguous
                sad = stagep.tile([128, 2, BL2, UNROLL], F16, tag="slotAd")
                for c in range(2):
                    nc.sync.dma_start(out=sad[:, c], in_=slot_ap(0, hmine, c, tv, False))
                sa = stagep.tile([128, 2, UNROLL, BL2], F16, tag="slotA")
                nc.vector.tensor_copy(sa.rearrange("p c t b -> p c b t"), sad)
                # slotB: blend of the two AG regions, read time-reversed
                r0 = stagep.tile([128, 2, BL2, UNROLL], F16, tag="slotR0")
                r1 = stagep.tile([128, 2, BL2, UNROLL], F16, tag="slotR1")
                for c in range(2):
                    nc.sync.dma_start(out=r0[:, c], in_=slot_ap(0, agout, c, tv, True))
                    nc.sync.dma_start(out=r1[:, c], in_=slot_ap(128 * HSZ, agout, c, tv, True))
                sb = stagep.tile([128, 2, BL2, UNROLL], F16, tag="slotB")
                sbr = stagep.tile([128, 2, UNROLL, BL2], F16, tag="slotBr")
                nc.vector.tensor_scalar_mul(sb, r0, masks[:, 0:1])
                nc.vector.tensor_scalar_mul(r1, r1, masks[:, 1:2])
                # add, writing t-reversed AND transposed to t-major
                p_step = sbr.ap[0][0]
                rev = bass.AP(tensor=sbr.tensor,
                              offset=sbr.offset + (UNROLL - 1) * BL2,
                              ap=[[p_step, 128], [UNROLL * BL2, 2],
                                  [1, BL2], [-BL2, UNROLL]])
                nc.vector.tensor_add(rev, sb, r1)

                def flat(tile, c):
                    return tile[:, c].rearrange("p t b -> p (t b)")

                return [flat(sa, 0), flat(sa, 1), flat(sbr, 0), flat(sbr, 1)]

            _rec_phase(nc, tc, consts, work, stagep, ps_rec, ps_big, h_sb, cst,
                       hc, wih2, whh2, bias2, xg2_dram, DK2, gemm2_src, "p2")

            # ---- AllToAll redistribution to data-parallel layout ----
            sb_view = sendb.ap().rearrange("j p (c b t) -> j p c b t", c=2, b=4)
            for j in range(8):
                for c in range(2):
                    nc.sync.dma_start(out=sb_view[j, :, c],
                                      in_=h_sb[:, c, ds(4 * j, 4), 1:HCOL])
            nc.gpsimd.collective_compute(
                "AllToAll", mybir.AluOpType.bypass,
                replica_groups=[list(range(8))],
                ins=[sendb.ap()], outs=[recvb.ap()],
            )

            # ================= PHASE 3: classifier + CRF =================
            cls1 = consts.tile([128, 8, 4, 128], F16, tag="wbig")
            nc.sync.dma_start(
                out=cls1,
                in_=cls1_ext.ap().rearrange("p (k m c) -> p k m c", k=8, m=4))
            clsb1 = consts.tile([128, 4], F32, tag="clsb1")
            nc.sync.dma_start(out=clsb1, in_=clsb1_ext[:, :])
            cls2 = consts.tile([128, 4, 15], F16, tag="cls2")
            nc.sync.dma_start(
                out=cls2, in_=cls2_ext.ap().rearrange("p (k j) -> p k j", k=4))
            clsb2 = consts.tile([15, 1], F32, tag="clsb2")
            nc.sync.dma_start(out=clsb2, in_=clsb2_ext[:, :])
            mexp = consts.tile([15, 15], F32, tag="mexp")
            nc.sync.dma_start(out=mexp, in_=mexp_ext[:, :])
            transn = consts.tile([15, 15], F16, tag="transn")
            nc.sync.dma_start(out=transn, in_=transn_ext[:, :])
            crfstart = consts.tile([15, 1], F32, tag="crfstart")
            nc.sync.dma_start(out=crfstart, in_=start_ext[:, :])
            crfend = consts.tile([15, 1], F32, tag="crfend")
            nc.sync.dma_start(out=crfend, in_=end_ext[:, :])
            lnalpha = consts.tile([15, 1], F32, tag="lnalpha")
            nc.sync.dma_start(out=lnalpha, in_=lna_ext[:, :])
            tago = consts.tile([15, TB3], F16, tag="tago")
            nc.sync.dma_start(out=tago, in_=tago_ext[:, :])

            logits = seqs.tile([15, TB3], F32, tag="logits")

            NT = 64  # t-steps per classifier n-tile (NT*BL3 = 512 cols)
            SHSZ = 128 * 2 * 4 * T  # elements per recv shard

            def comb_ap(kk, half, ns):
                # [128, 4, NT]: dir kk//2, chunk kk%2, half-shard, n-tile ns;
                # always an ascending t window (reversal done in the rhs view)
                d, c = kk // 2, kk % 2
                rev = d in (1, 3)  # c1b, w1b stored time-reversed
                base = (2 * d + half) * SHSZ + c * (4 * T)
                if not rev:
                    off = base + ns * NT
                else:
                    off = base + (T - NT) - ns * NT
                return bass.AP(tensor=recvb, offset=off,
                               ap=[[2 * 4 * T, 128], [T, 4], [1, NT]])

            for ns in range(8):
                comb = stagep.tile([128, 8, BL3, NT], F16, tag="comb", bufs=2)
                for kk in range(8):
                    for half in range(2):
                        nc.sync.dma_start(out=comb[:, kk, ds(4 * half, 4)],
                                          in_=comb_ap(kk, half, ns))
                hmt = []
                for m in range(4):
                    ps = ps_big.tile([128, NT * BL3], F32, tag="gemmps")
                    for kk in range(8):
                        if kk // 2 in (1, 3):
                            p_step = comb.ap[0][0]
                            off = (comb.offset + kk * BL3 * NT + (NT - 1))
                            rhs = bass.AP(tensor=comb.tensor, offset=off,
                                          ap=[[p_step, 128], [-1, NT],
                                              [NT, BL3]])
                        else:
                            rhs = comb[:, kk].rearrange("p b t -> p t b")
                        nc.tensor.matmul(ps, cls1[:, kk, m], rhs,
                                         start=(kk == 0), stop=(kk == 7))
                    hm = stagep.tile([128, NT * BL3], F16, tag="hm", bufs=4,
                                     name=f"hm{m}")
                    nc.scalar.activation(hm, ps, AF.Relu, bias=clsb1[:, m: m + 1])
                    hmt.append(hm)
                ps2 = ps_small.tile([15, NT * BL3], F32, tag="small")
                for m in range(4):
                    nc.tensor.matmul(ps2, cls2[:, m], hmt[m],
                                     start=(m == 0), stop=(m == 3))
                nc.vector.tensor_scalar_add(
                    logits[:, ds(ns * NT * BL3, NT * BL3)], ps2, clsb2)

            # fold CRF start/end into first/last emission columns
            nc.vector.tensor_scalar_add(logits[:, 0:BL3], logits[:, 0:BL3], crfstart)
            nc.vector.tensor_scalar_add(logits[:, TB3 - BL3: TB3],
                                        logits[:, TB3 - BL3: TB3], crfend)

            # ---- CRF numerator ----
            racc = work.tile([15, 16], F32, tag="racc")
            nc.vector.memset(racc, 0.0)
            for ns in range(8):
                pre = stagep.tile([15, 512], F32, tag="prodns")
                nc.vector.tensor_mul(pre, logits[:, ds(ns * 512, 512)],
                                     tago[:, ds(ns * 512, 512)])
                nc.vector.tensor_reduce(racc[:, 8 + ns: 9 + ns], pre,
                                        axis=mybir.AxisListType.X,
                                        op=mybir.AluOpType.add)
                psv = ps_small.tile([15, 512], F32, tag="small")
                nc.tensor.matmul(psv, transn, tago[:, ds(ns * 512, 512)],
                                 start=True, stop=True)
                w = 512 if ns < 7 else 512 - BL3
                pr = stagep.tile([15, 512], F32, tag="prodns")
                nc.vector.tensor_mul(pr[:, :w], psv[:, :w],
                                     tago[:, ds(ns * 512 + BL3, w)])
                nc.vector.tensor_reduce(racc[:, ns: ns + 1], pr[:, :w],
                                        axis=mybir.AxisListType.X,
                                        op=mybir.AluOpType.add)
            nv = stagep.tile([15, 1], F32, tag="nv")
            nc.vector.tensor_reduce(nv, racc, axis=mybir.AxisListType.X,
                                    op=mybir.AluOpType.add)
            ones15 = consts.tile([15, 1], F32, tag="ones15")
            nc.vector.memset(ones15, 1.0)
            psn = ps_small.tile([1, 1], F32, tag="small")
            nc.tensor.matmul(psn, ones15, nv, start=True, stop=True)
            num11 = work.tile([1, 1], F32, tag="num11")
            nc.vector.tensor_copy(num11, psn)

            # ---- CRF forward scan, probability space ----
            # p0 first, then E = alpha*exp(logits) computed in place over
            # logits (last col already includes e^end)
            p = seqs.tile([15, BL3], F32, tag="pvec")
            nc.scalar.activation(p, logits[:, 0:BL3], AF.Exp)
            E = logits
            nc.scalar.activation(E, logits, AF.Exp, bias=lnalpha)

            def crf_step(t_col_ap):
                z = ps_small.tile([15, BL3], F32, tag="small")
                nc.tensor.matmul(z, mexp, p, start=True, stop=True)
                nc.vector.tensor_mul(p, z, t_col_ap)

            for t in range(1, 16):
                crf_step(E[:, t * BL3: (t + 1) * BL3])
            with tc.For_i(0, 496, UNROLL) as tv:
                for j in range(UNROLL):
                    crf_step(E[:, ds((16 + j) * BL3 + tv * BL3, BL3)])

            # ---- denominator + output ----
            psd = ps_small.tile([1, BL3], F32, tag="small")
            nc.tensor.matmul(psd, ones15, p, start=True, stop=True)
            ln8 = stagep.tile([1, BL3], F32, tag="ln8")
            nc.scalar.activation(ln8, psd, AF.Ln)
            den11 = work.tile([1, 1], F32, tag="den11")
            nc.vector.tensor_reduce(den11, ln8, axis=mybir.AxisListType.X,
                                    op=mybir.AluOpType.add)
            res = work.tile([1, 1], F32, tag="res")
            nc.vector.tensor_sub(res, den11, num11)
            nc.sync.dma_start(out=out_ext[:, :], in_=res)

    nc.finalize()
    _BUILD_CACHE["nc"] = nc
    return nc


# ---- host-side input prep ---------------------------------------------------

# gate perm [i(256), f(256), g(256), o(256)] -> [i, f, o, g~]
_GPERM = np.concatenate([np.arange(0, 512), np.arange(768, 1024), np.arange(512, 768)])

# core c -> (pathway, direction, half): 0..3 char f/f/b/b, 4..7 word
_ROLES = [("c", 0, 0), ("c", 0, 1), ("c", 1, 0), ("c", 1, 1),
          ("w", 0, 0), ("w", 0, 1), ("w", 1, 0), ("w", 1, 1)]


def _wih_prep(W, dk_n):
    Wp = W[_GPERM]
    return np.ascontiguousarray(
        Wp.reshape(8, 128, dk_n, 128).transpose(3, 2, 0, 1).reshape(128, dk_n * 8 * 128)
    ).astype(np.float16)


def _make_in_maps(inputs):
    char_ids = np.asarray(inputs["char_ids"])
    tags = np.asarray(inputs["tags"])
    wemb = np.asarray(inputs["word_embeddings"], np.float32)
    emb = np.asarray(inputs["char_emb_table"], np.float32)
    trans = np.asarray(inputs["crf_trans"], np.float32)

    alpha = 1.0 / (15.0 * float(np.exp(trans).mean()))
    common = {}
    w1 = np.asarray(inputs["cls_w1"], np.float32)
    common["cls1"] = np.ascontiguousarray(
        w1.reshape(4, 128, 8, 128).transpose(3, 2, 0, 1).reshape(128, 8 * 4 * 128)
    ).astype(np.float16)
    common["clsb1"] = np.ascontiguousarray(
        np.asarray(inputs["cls_b1"], np.float32).reshape(4, 128).T).astype(np.float32)
    w2 = np.asarray(inputs["cls_w2"], np.float32)
    common["cls2"] = np.ascontiguousarray(
        w2.reshape(15, 4, 128).transpose(2, 1, 0).reshape(128, 4 * 15)).astype(np.float16)
    common["clsb2"] = np.asarray(inputs["cls_b2"], np.float32).reshape(15, 1).copy()
    common["mexp"] = np.exp(trans).astype(np.float32)
    common["transn"] = trans.astype(np.float16)
    common["crfstart"] = np.asarray(inputs["crf_start"], np.float32).reshape(15, 1).copy()
    common["crfend"] = np.asarray(inputs["crf_end"], np.float32).reshape(15, 1).copy()
    common["lnalpha"] = np.full((15, 1), np.log(alpha), np.float32)

    in_maps = []
    for c in range(NC_N):
        pw, d, hf = _ROLES[c]
        lo, hi = hf * BL2, (hf + 1) * BL2
        m = dict(common)

        # phase-1 weights/input
        if pw == "c":
            Wih1 = np.zeros((1024, 768), np.float32)
            Wih1[:, :128] = np.asarray(inputs["c0_Wih"], np.float32)[d]
            Whh1 = np.asarray(inputs["c0_Whh"], np.float32)[d]
            b1 = (np.asarray(inputs["c0_bih"], np.float32)[d]
                  + np.asarray(inputs["c0_bhh"], np.float32)[d])
            ce = emb[char_ids[lo:hi]]  # (32, 512, 128)
            X = np.zeros((128, DK1, T, BL2), np.float32)
            X[:, 0] = ce.transpose(2, 1, 0)
            Wl1 = np.asarray(inputs["c1_Wih"], np.float32)[d]
            Whh2 = np.asarray(inputs["c1_Whh"], np.float32)[d]
            b2 = (np.asarray(inputs["c1_bih"], np.float32)[d]
                  + np.asarray(inputs["c1_bhh"], np.float32)[d])
        else:
            Wih1 = np.asarray(inputs["w0_Wih"], np.float32)[d]
            Whh1 = np.asarray(inputs["w0_Whh"], np.float32)[d]
            b1 = (np.asarray(inputs["w0_bih"], np.float32)[d]
                  + np.asarray(inputs["w0_bhh"], np.float32)[d])
            X = wemb[lo:hi].reshape(BL2, T, DK1, 128).transpose(3, 2, 1, 0)
            Wl1 = np.asarray(inputs["w1_Wih"], np.float32)[d]
            Whh2 = np.asarray(inputs["w1_Whh"], np.float32)[d]
            b2 = (np.asarray(inputs["w1_bih"], np.float32)[d]
                  + np.asarray(inputs["w1_bhh"], np.float32)[d])
        if d == 1:  # backward: reverse local time
            X = X[:, :, ::-1]
        m["x1"] = np.ascontiguousarray(X.reshape(128, DK1, T * BL2)).astype(np.float16)
        m["wih1"] = _wih_prep(Wih1, DK1)
        m["whh1"] = _wih_prep(Whh1, 2)
        m["bias1"] = np.ascontiguousarray(b1[_GPERM].reshape(8, 128).T).astype(np.float32)

        # phase-2 weights: columns [own(256) | peer(256)]
        if d == 1:
            Wl1 = Wl1[:, np.r_[256:512, 0:256]]
        m["wih2"] = _wih_prep(Wl1, DK2)
        m["whh2"] = _wih_prep(Whh2, 2)
        m["bias2"] = np.ascontiguousarray(b2[_GPERM].reshape(8, 128).T).astype(np.float32)
        # blend: f-core (d=0) picks AG region 1 (the b-core), b-core picks 0
        msk = np.zeros((128, 2), np.float32)
        msk[:, 1 - d] = 1.0
        m["masks"] = msk

        # phase-3 tags for this core's 8 sequences
        seqs3 = np.r_[4 * c: 4 * c + 4, 32 + 4 * c: 32 + 4 * c + 4]
        oh = (np.arange(K)[:, None, None] == tags[seqs3][None]).astype(np.float32)
        # (15, 8seq, 512t) -> (15, t, b)
        m["tagoneT"] = np.ascontiguousarray(
            oh.transpose(0, 2, 1).reshape(K, TB3)).astype(np.float16)
        in_maps.append(m)
    return in_maps, alpha


def kernel(**inputs):
    nc = _build_nc()
    in_maps, alpha = _make_in_maps(inputs)
    res = run_bass_kernel_spmd(nc, in_maps, core_ids=list(range(NC_N)))
    total = sum(float(res.results[c]["out"][0, 0]) for c in range(NC_N))
    total -= B * (T - 1) * np.log(alpha)
    return np.float32(total / B)


# revision 13
# speedup vs baseline: 3.9222x; 1.1109x over previous
"""BiLSTM dual-pathway + CRF NLL kernel for 8 Trainium2 NeuronCores.

Sharding: direction-parallel for the LSTM recurrences, data-parallel for the
classifier/CRF. Phase 1 runs the four layer-0 directions (char fwd/bwd, word
fwd/bwd) on 8 cores as (direction x batch-half), batch 32 per core, so the
recurrent matmuls run at N=32 instead of N=8 and the sequential chain count
drops from 4096 to 1024 steps. A pairwise AllGather exchanges the L0 hidden
states between fwd/bwd cores, phase 2 runs the four layer-1 directions the
same way, then an 8-rank AllToAll redistributes hidden states to a
data-parallel layout (8 sequences per core) for the classifier and CRF.

SPMD uniformity: every core runs the identical program. Backward directions
receive host-time-reversed inputs; reversed reads of peer hidden states are
fixed negative-stride APs, with host-permuted weight columns absorbing the
f/b role differences. Peer-region selection after the AllGather uses per-core
0/1 blend masks delivered as input data.

The CRF forward scan runs in probability space with a constant per-step
prescale alpha folded into the emission exponentials (corrected analytically
on the host), so each step is one resident-weight 15x15 matmul plus one
vector multiply -- no per-step exp/ln activation-table swaps.
"""

import sys

sys.path.insert(0, "/opt/trn_rl_repo")

import numpy as np

import concourse.bass as bass
import concourse.mybir as mybir
from concourse import bacc
from concourse.bass import ds
from concourse.tile import TileContext
from concourse.bass_utils import run_bass_kernel_spmd

F16 = mybir.dt.float16
F32 = mybir.dt.float32
AF = mybir.ActivationFunctionType

B, T, V, K = 64, 512, 40, 15
NC_N = 8
BL2 = 32            # batch per core in phases 1-2
BL3 = 8             # sequences per core in phase 3
TB3 = T * BL3       # 4096 classifier/CRF columns per core
DK1, DK2 = 6, 4     # input chunks for L0 (word=768, char padded) and L1 (512)
UNROLL = 16
HCOL = T + 1        # h buffer columns per sequence (col 0 = zero init)

_BUILD_CACHE = {}


def _rec_phase(nc, tc, consts, work, stagep, ps_rec, ps_big, h_sb, cst, hc,
               wih, whh, bias, xg_dram, dkn, gemm_src_ap, phase_tag, hmv=None):
    """Emit one GEMM pass (xg to DRAM) + one 512-step recurrence.

    Column order everywhere in the GEMM is b-major within a 16-step block
    (col = b*16 + t_local), so GEMM rhs and psum stay contiguous. The
    recurrence keeps its working h state in `hc` [128, 2, 17, 32] (t-major
    block ring, col 0 = carry-in) so the per-step matmul rhs is contiguous;
    a gpsimd copy per block mirrors h into the seq-major `h_sb`.

    gemm_src_ap(tv) -> list of dkn [128, 512]-column sources for block tv.
    """
    CB = UNROLL * BL2  # columns per time block

    # ---- GEMM: xg[:, m, block] = sum_k wih[k, m]^T X[k, block] + bias ----
    # two blocks per iteration with up-front DMAs so the PE never waits
    with tc.For_i(0, T, 2 * UNROLL) as tv:
        srcsA = gemm_src_ap(tv)
        srcsB = gemm_src_ap(tv + UNROLL)
        for half, srcs in ((0, srcsA), (1, srcsB)):
            for m in range(8):
                ps = ps_big.tile([128, CB], F32, tag="gemmps")
                for k in range(dkn):
                    nc.tensor.matmul(ps, wih[:, k, m], srcs[k],
                                     start=(k == 0), stop=(k == dkn - 1))
                st = stagep.tile([128, CB], F16, tag="xgstage")
                nc.scalar.activation(st, ps, AF.Identity, bias=bias[:, m: m + 1])
                nc.sync.dma_start(
                    out=xg_dram[:, m, ds((tv + half * UNROLL) * BL2, CB)], in_=st)

    # ---- recurrence ----
    nc.vector.memset(cst, 0.0)
    nc.vector.memset(hc[:, :, 0], 0.0)

    with tc.For_i(0, T, UNROLL) as tv:
        xgs = stagep.tile([128, 8, CB], F16, tag="xgs")
        nc.sync.dma_start(out=xgs, in_=xg_dram[:, :, ds(tv * BL2, CB)])
        for j in range(UNROLL):
            ps = ps_rec.tile([128, 8, BL2], F32, tag="recps")
            # m-order: i,f (0-3) first, then g~ (6,7), then o (4,5)
            for m in (0, 1, 2, 3, 6, 7, 4, 5):
                for k in range(2):
                    nc.tensor.matmul(ps[:, m], whh[:, k, m],
                                     hc[:, k, j], start=(k == 0), stop=(k == 1))
            g = stagep.tile([128, 8, BL2], F32, tag="g")
            xsl = xgs[:, :, ds(j * BL2, BL2)]
            nc.vector.tensor_add(g[:, 0:4], ps[:, 0:4], xsl[:, 0:4])
            sig = stagep.tile([128, 6, BL2], F32, tag="sig")
            nc.scalar.activation(sig[:, 0:4], g[:, 0:4], AF.Sigmoid)
            nc.vector.tensor_mul(cst, cst, sig[:, 2:4])
            nc.vector.tensor_add(g[:, 4:8], ps[:, 4:8], xsl[:, 4:8])
            tgg = stagep.tile([128, 2, BL2], F32, tag="tgg")
            nc.scalar.activation(tgg, g[:, 6:8], AF.Tanh)
            tmp = stagep.tile([128, 2, BL2], F32, tag="tmpig")
            nc.vector.tensor_mul(tmp, sig[:, 0:2], tgg)
            nc.scalar.activation(sig[:, 4:6], g[:, 4:6], AF.Sigmoid)
            nc.vector.tensor_add(cst, cst, tmp)
            tch = stagep.tile([128, 2, BL2], F32, tag="tch")
            nc.scalar.activation(tch, cst, AF.Tanh)
            nc.vector.tensor_mul(hc[:, :, j + 1], sig[:, 4:6], tch)
        if phase_tag == "p1":
            # t-major h goes straight to DRAM (contiguous both sides)
            nc.sync.dma_start(out=hmv[:, :, ds(tv, UNROLL)],
                              in_=hc[:, :, 1: UNROLL + 1])
        else:
            # phase 2: seq-major SBUF mirror (A2A shard build needs it)
            nc.gpsimd.tensor_copy(
                h_sb[:, :, :, ds(tv + 1, UNROLL)],
                hc[:, :, 1: UNROLL + 1].rearrange("p c t b -> p c b t"))
        nc.gpsimd.tensor_copy(hc[:, :, 0], hc[:, :, UNROLL])


def _build_nc():
    if "nc" in _BUILD_CACHE:
        return _BUILD_CACHE["nc"]
    nc = bacc.Bacc(target_bir_lowering=False, num_devices=NC_N)

    # ---- external parameters -------------------------------------------------
    x1_ext = nc.declare_dram_parameter("x1", [128, DK1, T * BL2], F16, isOutput=False)
    wih1_ext = nc.declare_dram_parameter("wih1", [128, DK1 * 8 * 128], F16, isOutput=False)
    whh1_ext = nc.declare_dram_parameter("whh1", [128, 2 * 8 * 128], F16, isOutput=False)
    bias1_ext = nc.declare_dram_parameter("bias1", [128, 8], F32, isOutput=False)
    wih2_ext = nc.declare_dram_parameter("wih2", [128, DK2 * 8 * 128], F16, isOutput=False)
    whh2_ext = nc.declare_dram_parameter("whh2", [128, 2 * 8 * 128], F16, isOutput=False)
    bias2_ext = nc.declare_dram_parameter("bias2", [128, 8], F32, isOutput=False)
    masks_ext = nc.declare_dram_parameter("masks", [128, 2], F32, isOutput=False)
    cls1_ext = nc.declare_dram_parameter("cls1", [128, 8 * 4 * 128], F16, isOutput=False)
    clsb1_ext = nc.declare_dram_parameter("clsb1", [128, 4], F32, isOutput=False)
    cls2_ext = nc.declare_dram_parameter("cls2", [128, 4 * 15], F16, isOutput=False)
    clsb2_ext = nc.declare_dram_parameter("clsb2", [15, 1], F32, isOutput=False)
    mexp_ext = nc.declare_dram_parameter("mexp", [15, 15], F32, isOutput=False)
    transn_ext = nc.declare_dram_parameter("transn", [15, 15], F16, isOutput=False)
    start_ext = nc.declare_dram_parameter("crfstart", [15, 1], F32, isOutput=False)
    end_ext = nc.declare_dram_parameter("crfend", [15, 1], F32, isOutput=False)
    lna_ext = nc.declare_dram_parameter("lnalpha", [15, 1], F32, isOutput=False)
    tago_ext = nc.declare_dram_parameter("tagoneT", [15, TB3], F16, isOutput=False)
    out_ext = nc.declare_dram_parameter("out", [1, 1], F32, isOutput=True)

    # ---- internal DRAM -------------------------------------------------------
    HSZ = 2 * BL2 * T  # 32768 cols/partition of h (f16)
    xg1_dram = nc.dram_tensor("xg1", [128, 8, T * BL2], F16)
    xg2_dram = nc.dram_tensor("xg2", [128, 8, T * BL2], F16)
    hmine = nc.dram_tensor("hmine", [128, HSZ], F16)
    agout = nc.dram_tensor("agout", [2, 128, HSZ], F16)
    sendb = nc.dram_tensor("sendb", [8, 128, 2 * 4 * T], F16)
    recvb = nc.dram_tensor("recvb", [8, 128, 2 * 4 * T], F16)

    with TileContext(nc) as tc:
        with (
            tc.tile_pool(name="consts", bufs=1) as consts,
            tc.tile_pool(name="seqs", bufs=1) as seqs,
            tc.tile_pool(name="work", bufs=2) as work,
            tc.tile_pool(name="stage", bufs=2) as stagep,
            tc.tile_pool(name="ps_big", bufs=2, space="PSUM") as ps_big,
            tc.tile_pool(name="ps_rec", bufs=2, space="PSUM") as ps_rec,
            tc.tile_pool(name="ps_small", bufs=3, space="PSUM") as ps_small,
        ):
            # h buffer, seq-major: [128, chunk2, b32, T+1], reused by phases 1+2
            h_sb = seqs.tile([128, 2, BL2, HCOL], F16, tag="h_sb")
            hc = seqs.tile([128, 2, UNROLL + 1, BL2], F16, tag="hcomp")
            cst = seqs.tile([128, 2, BL2], F32, tag="cstate")
            masks = consts.tile([128, 2], F32, tag="masks")
            nc.sync.dma_start(out=masks, in_=masks_ext[:, :])

            # ================= PHASE 1 (L0) =================
            wih1 = consts.tile([128, DK1, 8, 128], F16, tag="wbig")
            nc.sync.dma_start(
                out=wih1,
                in_=wih1_ext.ap().rearrange("p (k m c) -> p k m c", k=DK1, m=8))
            whh1 = consts.tile([128, 2, 8, 128], F16, tag="whh")
            nc.sync.dma_start(
                out=whh1,
                in_=whh1_ext.ap().rearrange("p (k m c) -> p k m c", k=2, m=8))
            bias1 = consts.tile([128, 8], F32, tag="bias1")
            nc.sync.dma_start(out=bias1, in_=bias1_ext[:, :])

            CB = UNROLL * BL2

            def gemm1_src(tv):
                xb = stagep.tile([128, DK1, CB], F16, tag="xb1")
                nc.sync.dma_start(out=xb, in_=x1_ext[:, :, ds(tv * BL2, CB)])
                return [xb[:, k] for k in range(DK1)]  # cols already b-major

            # phase-1 h layout in DRAM is t-major: (p, c, t, b)
            hm_view = hmine.ap().rearrange("p (c t b) -> p c t b", c=2, t=T)
            _rec_phase(nc, tc, consts, work, stagep, ps_rec, ps_big, h_sb, cst,
                       hc, wih1, whh1, bias1, xg1_dram, DK1, gemm1_src, "p1",
                       hmv=hm_view)
            nc.gpsimd.collective_compute(
                "AllGather", mybir.AluOpType.bypass,
                replica_groups=[[0, 2], [1, 3], [4, 6], [5, 7]],
                ins=[hmine.ap()], outs=[agout.ap()],
            )

            # ================= PHASE 2 (L1) =================
            wih2 = consts.tile([128, DK2, 8, 128], F16, tag="wbig")
            nc.sync.dma_start(
                out=wih2,
                in_=wih2_ext.ap().rearrange("p (k m c) -> p k m c", k=DK2, m=8))
            whh2 = consts.tile([128, 2, 8, 128], F16, tag="whh")
            nc.sync.dma_start(
                out=whh2,
                in_=whh2_ext.ap().rearrange("p (k m c) -> p k m c", k=2, m=8))
            bias2 = consts.tile([128, 8], F32, tag="bias2")
            nc.sync.dma_start(out=bias2, in_=bias2_ext[:, :])

            def slot_ap(tensor_offset, tensor, c, tv, reverse):
                # [128, UNROLL*32] chunk c of a t-major h DRAM region
                # (p, c2, T, b32): a contiguous ascending t window; reversed
                # slots flip t on the SBUF side.
                if not reverse:
                    w = tv
                else:
                    w = (T - UNROLL) - tv
                off = tensor_offset + c * (T * BL2) + w * BL2
                return bass.AP(tensor=tensor, offset=off,
                               ap=[[HSZ, 128], [1, UNROLL * BL2]])

            def gemm2_src(tv):
                # everything t-major: slot DMAs are contiguous 1KB-run reads
                sa = stagep.tile([128, 2, UNROLL, BL2], F16, tag="slotA")
                r0 = stagep.tile([128, 2, UNROLL, BL2], F16, tag="slotR0")
                r1 = stagep.tile([128, 2, UNROLL, BL2], F16, tag="slotR1")
                for c in range(2):
                    nc.sync.dma_start(out=sa[:, c], in_=slot_ap(0, hmine, c, tv, False))
                    nc.sync.dma_start(out=r0[:, c], in_=slot_ap(0, agout, c, tv, True))
                    nc.sync.dma_start(out=r1[:, c], in_=slot_ap(128 * HSZ, agout, c, tv, True))
                sb = stagep.tile([128, 2, UNROLL, BL2], F16, tag="slotB")
                sbr = stagep.tile([128, 2, UNROLL, BL2], F16, tag="slotBr")
                nc.vector.tensor_scalar_mul(sb, r0, masks[:, 0:1])
                nc.vector.tensor_scalar_mul(r1, r1, masks[:, 1:2])
                # add, writing with the t window reversed
                p_step = sbr.ap[0][0]
                rev = bass.AP(tensor=sbr.tensor,
                              offset=sbr.offset + (UNROLL - 1) * BL2,
                              ap=[[p_step, 128], [UNROLL * BL2, 2],
                                  [-BL2, UNROLL], [1, BL2]])
                nc.vector.tensor_add(rev, sb, r1)

                def flat(tile, c):
                    return tile[:, c].rearrange("p t b -> p (t b)")

                return [flat(sa, 0), flat(sa, 1), flat(sbr, 0), flat(sbr, 1)]

            _rec_phase(nc, tc, consts, work, stagep, ps_rec, ps_big, h_sb, cst,
                       hc, wih2, whh2, bias2, xg2_dram, DK2, gemm2_src, "p2")

            # ---- AllToAll redistribution to data-parallel layout ----
            sb_view = sendb.ap().rearrange("j p (c b t) -> j p c b t", c=2, b=4)
            for j in range(8):
                for c in range(2):
                    nc.sync.dma_start(out=sb_view[j, :, c],
                                      in_=h_sb[:, c, ds(4 * j, 4), 1:HCOL])
            nc.gpsimd.collective_compute(
                "AllToAll", mybir.AluOpType.bypass,
                replica_groups=[list(range(8))],
                ins=[sendb.ap()], outs=[recvb.ap()],
            )

            # ================= PHASE 3: classifier + CRF =================
            cls1 = consts.tile([128, 8, 4, 128], F16, tag="wbig")
            nc.sync.dma_start(
                out=cls1,
                in_=cls1_ext.ap().rearrange("p (k m c) -> p k m c", k=8, m=4))
            clsb1 = consts.tile([128, 4], F32, tag="clsb1")
            nc.sync.dma_start(out=clsb1, in_=clsb1_ext[:, :])
            cls2 = consts.tile([128, 4, 15], F16, tag="cls2")
            nc.sync.dma_start(
                out=cls2, in_=cls2_ext.ap().rearrange("p (k j) -> p k j", k=4))
            clsb2 = consts.tile([15, 1], F32, tag="clsb2")
            nc.sync.dma_start(out=clsb2, in_=clsb2_ext[:, :])
            mexp = consts.tile([15, 15], F32, tag="mexp")
            nc.sync.dma_start(out=mexp, in_=mexp_ext[:, :])
            transn = consts.tile([15, 15], F16, tag="transn")
            nc.sync.dma_start(out=transn, in_=transn_ext[:, :])
            crfstart = consts.tile([15, 1], F32, tag="crfstart")
            nc.sync.dma_start(out=crfstart, in_=start_ext[:, :])
            crfend = consts.tile([15, 1], F32, tag="crfend")
            nc.sync.dma_start(out=crfend, in_=end_ext[:, :])
            lnalpha = consts.tile([15, 1], F32, tag="lnalpha")
            nc.sync.dma_start(out=lnalpha, in_=lna_ext[:, :])
            tago = consts.tile([15, TB3], F16, tag="tago")
            nc.sync.dma_start(out=tago, in_=tago_ext[:, :])

            logits = seqs.tile([15, TB3], F32, tag="logits")

            NT = 64  # t-steps per classifier n-tile (NT*BL3 = 512 cols)
            SHSZ = 128 * 2 * 4 * T  # elements per recv shard

            def comb_ap(kk, half, ns):
                # [128, 4, NT]: dir kk//2, chunk kk%2, half-shard, n-tile ns;
                # always an ascending t window (reversal done in the rhs view)
                d, c = kk // 2, kk % 2
                rev = d in (1, 3)  # c1b, w1b stored time-reversed
                base = (2 * d + half) * SHSZ + c * (4 * T)
                if not rev:
                    off = base + ns * NT
                else:
                    off = base + (T - NT) - ns * NT
                return bass.AP(tensor=recvb, offset=off,
                               ap=[[2 * 4 * T, 128], [T, 4], [1, NT]])

            for ns in range(8):
                comb = stagep.tile([128, 8, BL3, NT], F16, tag="comb", bufs=2)
                for kk in range(8):
                    for half in range(2):
                        nc.sync.dma_start(out=comb[:, kk, ds(4 * half, 4)],
                                          in_=comb_ap(kk, half, ns))
                hmt = []
                for m in range(4):
                    ps = ps_big.tile([128, NT * BL3], F32, tag="gemmps")
                    for kk in range(8):
                        if kk // 2 in (1, 3):
                            p_step = comb.ap[0][0]
                            off = (comb.offset + kk * BL3 * NT + (NT - 1))
                            rhs = bass.AP(tensor=comb.tensor, offset=off,
                                          ap=[[p_step, 128], [-1, NT],
                                              [NT, BL3]])
                        else:
                            rhs = comb[:, kk].rearrange("p b t -> p t b")
                        nc.tensor.matmul(ps, cls1[:, kk, m], rhs,
                                         start=(kk == 0), stop=(kk == 7))
                    hm = stagep.tile([128, NT * BL3], F16, tag="hm", bufs=4,
                                     name=f"hm{m}")
                    nc.scalar.activation(hm, ps, AF.Relu, bias=clsb1[:, m: m + 1])
                    hmt.append(hm)
                ps2 = ps_small.tile([15, NT * BL3], F32, tag="small")
                for m in range(4):
                    nc.tensor.matmul(ps2, cls2[:, m], hmt[m],
                                     start=(m == 0), stop=(m == 3))
                nc.vector.tensor_scalar_add(
                    logits[:, ds(ns * NT * BL3, NT * BL3)], ps2, clsb2)

            # fold CRF start/end into first/last emission columns
            nc.vector.tensor_scalar_add(logits[:, 0:BL3], logits[:, 0:BL3], crfstart)
            nc.vector.tensor_scalar_add(logits[:, TB3 - BL3: TB3],
                                        logits[:, TB3 - BL3: TB3], crfend)

            # ---- CRF numerator ----
            racc = work.tile([15, 16], F32, tag="racc")
            nc.vector.memset(racc, 0.0)
            for ns in range(8):
                pre = stagep.tile([15, 512], F32, tag="prodns")
                nc.vector.tensor_mul(pre, logits[:, ds(ns * 512, 512)],
                                     tago[:, ds(ns * 512, 512)])
                nc.vector.tensor_reduce(racc[:, 8 + ns: 9 + ns], pre,
                                        axis=mybir.AxisListType.X,
                                        op=mybir.AluOpType.add)
                psv = ps_small.tile([15, 512], F32, tag="small")
                nc.tensor.matmul(psv, transn, tago[:, ds(ns * 512, 512)],
                                 start=True, stop=True)
                w = 512 if ns < 7 else 512 - BL3
                pr = stagep.tile([15, 512], F32, tag="prodns")
                nc.vector.tensor_mul(pr[:, :w], psv[:, :w],
                                     tago[:, ds(ns * 512 + BL3, w)])
                nc.vector.tensor_reduce(racc[:, ns: ns + 1], pr[:, :w],
                                        axis=mybir.AxisListType.X,
                                        op=mybir.AluOpType.add)
            nv = stagep.tile([15, 1], F32, tag="nv")
            nc.vector.tensor_reduce(nv, racc, axis=mybir.AxisListType.X,
                                    op=mybir.AluOpType.add)
            ones15 = consts.tile([15, 1], F32, tag="ones15")
            nc.vector.memset(ones15, 1.0)
            psn = ps_small.tile([1, 1], F32, tag="small")
            nc.tensor.matmul(psn, ones15, nv, start=True, stop=True)
            num11 = work.tile([1, 1], F32, tag="num11")
            nc.vector.tensor_copy(num11, psn)

            # ---- CRF forward scan, probability space ----
            # p0 first, then E = alpha*exp(logits) computed in place over
            # logits (last col already includes e^end)
            p = seqs.tile([15, BL3], F32, tag="pvec")
            nc.scalar.activation(p, logits[:, 0:BL3], AF.Exp)
            E = logits
            nc.scalar.activation(E, logits, AF.Exp, bias=lnalpha)

            def crf_step(t_col_ap):
                z = ps_small.tile([15, BL3], F32, tag="small")
                nc.tensor.matmul(z, mexp, p, start=True, stop=True)
                nc.vector.tensor_mul(p, z, t_col_ap)

            for t in range(1, 16):
                crf_step(E[:, t * BL3: (t + 1) * BL3])
            with tc.For_i(0, 496, UNROLL) as tv:
                for j in range(UNROLL):
                    crf_step(E[:, ds((16 + j) * BL3 + tv * BL3, BL3)])

            # ---- denominator + output ----
            psd = ps_small.tile([1, BL3], F32, tag="small")
            nc.tensor.matmul(psd, ones15, p, start=True, stop=True)
            ln8 = stagep.tile([1, BL3], F32, tag="ln8")
            nc.scalar.activation(ln8, psd, AF.Ln)
            den11 = work.tile([1, 1], F32, tag="den11")
            nc.vector.tensor_reduce(den11, ln8, axis=mybir.AxisListType.X,
                                    op=mybir.AluOpType.add)
            res = work.tile([1, 1], F32, tag="res")
            nc.vector.tensor_sub(res, den11, num11)
            nc.sync.dma_start(out=out_ext[:, :], in_=res)

    nc.finalize()
    _BUILD_CACHE["nc"] = nc
    return nc


# ---- host-side input prep ---------------------------------------------------

# gate perm [i(256), f(256), g(256), o(256)] -> [i, f, o, g~]
_GPERM = np.concatenate([np.arange(0, 512), np.arange(768, 1024), np.arange(512, 768)])

# core c -> (pathway, direction, half): 0..3 char f/f/b/b, 4..7 word
_ROLES = [("c", 0, 0), ("c", 0, 1), ("c", 1, 0), ("c", 1, 1),
          ("w", 0, 0), ("w", 0, 1), ("w", 1, 0), ("w", 1, 1)]


def _wih_prep(W, dk_n):
    Wp = W[_GPERM]
    return np.ascontiguousarray(
        Wp.reshape(8, 128, dk_n, 128).transpose(3, 2, 0, 1).reshape(128, dk_n * 8 * 128)
    ).astype(np.float16)


def _make_in_maps(inputs):
    char_ids = np.asarray(inputs["char_ids"])
    tags = np.asarray(inputs["tags"])
    wemb = np.asarray(inputs["word_embeddings"], np.float32)
    emb = np.asarray(inputs["char_emb_table"], np.float32)
    trans = np.asarray(inputs["crf_trans"], np.float32)

    alpha = 1.0 / (15.0 * float(np.exp(trans).mean()))
    common = {}
    w1 = np.asarray(inputs["cls_w1"], np.float32)
    common["cls1"] = np.ascontiguousarray(
        w1.reshape(4, 128, 8, 128).transpose(3, 2, 0, 1).reshape(128, 8 * 4 * 128)
    ).astype(np.float16)
    common["clsb1"] = np.ascontiguousarray(
        np.asarray(inputs["cls_b1"], np.float32).reshape(4, 128).T).astype(np.float32)
    w2 = np.asarray(inputs["cls_w2"], np.float32)
    common["cls2"] = np.ascontiguousarray(
        w2.reshape(15, 4, 128).transpose(2, 1, 0).reshape(128, 4 * 15)).astype(np.float16)
    common["clsb2"] = np.asarray(inputs["cls_b2"], np.float32).reshape(15, 1).copy()
    common["mexp"] = np.exp(trans).astype(np.float32)
    common["transn"] = trans.astype(np.float16)
    common["crfstart"] = np.asarray(inputs["crf_start"], np.float32).reshape(15, 1).copy()
    common["crfend"] = np.asarray(inputs["crf_end"], np.float32).reshape(15, 1).copy()
    common["lnalpha"] = np.full((15, 1), np.log(alpha), np.float32)

    in_maps = []
    for c in range(NC_N):
        pw, d, hf = _ROLES[c]
        lo, hi = hf * BL2, (hf + 1) * BL2
        m = dict(common)

        # phase-1 weights/input
        if pw == "c":
            Wih1 = np.zeros((1024, 768), np.float32)
            Wih1[:, :128] = np.asarray(inputs["c0_Wih"], np.float32)[d]
            Whh1 = np.asarray(inputs["c0_Whh"], np.float32)[d]
            b1 = (np.asarray(inputs["c0_bih"], np.float32)[d]
                  + np.asarray(inputs["c0_bhh"], np.float32)[d])
            ce = emb[char_ids[lo:hi]]  # (32, 512, 128)
            X = np.zeros((128, DK1, T, BL2), np.float32)
            X[:, 0] = ce.transpose(2, 1, 0)
            Wl1 = np.asarray(inputs["c1_Wih"], np.float32)[d]
            Whh2 = np.asarray(inputs["c1_Whh"], np.float32)[d]
            b2 = (np.asarray(inputs["c1_bih"], np.float32)[d]
                  + np.asarray(inputs["c1_bhh"], np.float32)[d])
        else:
            Wih1 = np.asarray(inputs["w0_Wih"], np.float32)[d]
            Whh1 = np.asarray(inputs["w0_Whh"], np.float32)[d]
            b1 = (np.asarray(inputs["w0_bih"], np.float32)[d]
                  + np.asarray(inputs["w0_bhh"], np.float32)[d])
            X = wemb[lo:hi].reshape(BL2, T, DK1, 128).transpose(3, 2, 1, 0)
            Wl1 = np.asarray(inputs["w1_Wih"], np.float32)[d]
            Whh2 = np.asarray(inputs["w1_Whh"], np.float32)[d]
            b2 = (np.asarray(inputs["w1_bih"], np.float32)[d]
                  + np.asarray(inputs["w1_bhh"], np.float32)[d])
        if d == 1:  # backward: reverse local time
            X = X[:, :, ::-1]
        m["x1"] = np.ascontiguousarray(X.reshape(128, DK1, T * BL2)).astype(np.float16)
        m["wih1"] = _wih_prep(Wih1, DK1)
        m["whh1"] = _wih_prep(Whh1, 2)
        m["bias1"] = np.ascontiguousarray(b1[_GPERM].reshape(8, 128).T).astype(np.float32)

        # phase-2 weights: columns [own(256) | peer(256)]
        if d == 1:
            Wl1 = Wl1[:, np.r_[256:512, 0:256]]
        m["wih2"] = _wih_prep(Wl1, DK2)
        m["whh2"] = _wih_prep(Whh2, 2)
        m["bias2"] = np.ascontiguousarray(b2[_GPERM].reshape(8, 128).T).astype(np.float32)
        # blend: f-core (d=0) picks AG region 1 (the b-core), b-core picks 0
        msk = np.zeros((128, 2), np.float32)
        msk[:, 1 - d] = 1.0
        m["masks"] = msk

        # phase-3 tags for this core's 8 sequences
        seqs3 = np.r_[4 * c: 4 * c + 4, 32 + 4 * c: 32 + 4 * c + 4]
        oh = (np.arange(K)[:, None, None] == tags[seqs3][None]).astype(np.float32)
        # (15, 8seq, 512t) -> (15, t, b)
        m["tagoneT"] = np.ascontiguousarray(
            oh.transpose(0, 2, 1).reshape(K, TB3)).astype(np.float16)
        in_maps.append(m)
    return in_maps, alpha


def kernel(**inputs):
    nc = _build_nc()
    in_maps, alpha = _make_in_maps(inputs)
    res = run_bass_kernel_spmd(nc, in_maps, core_ids=list(range(NC_N)))
    total = sum(float(res.results[c]["out"][0, 0]) for c in range(NC_N))
    total -= B * (T - 1) * np.log(alpha)
    return np.float32(total / B)


# revision 15
# speedup vs baseline: 4.3139x; 1.0999x over previous
"""BiLSTM dual-pathway + CRF NLL kernel for 8 Trainium2 NeuronCores.

Sharding: direction-parallel for the LSTM recurrences, data-parallel for the
classifier/CRF. Phase 1 runs the four layer-0 directions (char fwd/bwd, word
fwd/bwd) on 8 cores as (direction x batch-half), batch 32 per core, so the
recurrent matmuls run at N=32 instead of N=8 and the sequential chain count
drops from 4096 to 1024 steps. A pairwise AllGather exchanges the L0 hidden
states between fwd/bwd cores, phase 2 runs the four layer-1 directions the
same way, then an 8-rank AllToAll redistributes hidden states to a
data-parallel layout (8 sequences per core) for the classifier and CRF.

SPMD uniformity: every core runs the identical program. Backward directions
receive host-time-reversed inputs; reversed reads of peer hidden states are
fixed negative-stride APs, with host-permuted weight columns absorbing the
f/b role differences. Peer-region selection after the AllGather uses per-core
0/1 blend masks delivered as input data.

The CRF forward scan runs in probability space with a constant per-step
prescale alpha folded into the emission exponentials (corrected analytically
on the host), so each step is one resident-weight 15x15 matmul plus one
vector multiply -- no per-step exp/ln activation-table swaps.
"""

import sys

sys.path.insert(0, "/opt/trn_rl_repo")

import numpy as np

import concourse.bass as bass
import concourse.mybir as mybir
from concourse import bacc
from concourse.bass import ds
from concourse.tile import TileContext
from concourse.bass_utils import run_bass_kernel_spmd

F16 = mybir.dt.float16
F32 = mybir.dt.float32
AF = mybir.ActivationFunctionType

B, T, V, K = 64, 512, 40, 15
NC_N = 8
BL2 = 32            # batch per core in phases 1-2
BL3 = 8             # sequences per core in phase 3
TB3 = T * BL3       # 4096 classifier/CRF columns per core
DK1, DK2 = 6, 4     # input chunks for L0 (word=768, char padded) and L1 (512)
UNROLL = 16
HCOL = T + 1        # h buffer columns per sequence (col 0 = zero init)

_BUILD_CACHE = {}


def _emit_rec_block(nc, stagep, ps_rec, hc, cst, whh, xg_dram, tv, fodder):
    """Emit 16 recurrence steps for time block tv.

    fodder: list of (items, per_step) - closures emitting one PE-side quantum
    each (GEMM matmuls for other pipeline stages), dispensed between the
    step's recurrent matmuls and its vector/scalar chain so they execute in
    the PE's dependency-stall gaps.
    """
    CB = UNROLL * BL2
    xgs = stagep.tile([128, 8, CB], F16, tag="xgs")
    nc.sync.dma_start(out=xgs, in_=xg_dram[:, :, ds(tv * BL2, CB)])
    cursors = [[items, 0, per_step] for items, per_step in fodder]
    for j in range(UNROLL):
        ps = ps_rec.tile([128, 8, BL2], F32, tag="recps")
        # m-order: i,f (0-3) first, then g~ (6,7), then o (4,5)
        for m in (0, 1, 2, 3, 6, 7, 4, 5):
            for k in range(2):
                nc.tensor.matmul(ps[:, m], whh[:, k, m],
                                 hc[:, k, j], start=(k == 0), stop=(k == 1))
        for cur in cursors:
            for _ in range(cur[2]):
                if cur[1] < len(cur[0]):
                    cur[0][cur[1]]()
                    cur[1] += 1
        g = stagep.tile([128, 8, BL2], F32, tag="g")
        xsl = xgs[:, :, ds(j * BL2, BL2)]
        nc.vector.tensor_add(g[:, 0:4], ps[:, 0:4], xsl[:, 0:4])
        sig = stagep.tile([128, 6, BL2], F32, tag="sig")
        nc.scalar.activation(sig[:, 0:4], g[:, 0:4], AF.Sigmoid)
        nc.vector.tensor_mul(cst, cst, sig[:, 2:4])
        nc.vector.tensor_add(g[:, 4:8], ps[:, 4:8], xsl[:, 4:8])
        tgg = stagep.tile([128, 2, BL2], F32, tag="tgg")
        nc.scalar.activation(tgg, g[:, 6:8], AF.Tanh)
        tmp = stagep.tile([128, 2, BL2], F32, tag="tmpig")
        nc.vector.tensor_mul(tmp, sig[:, 0:2], tgg)
        nc.scalar.activation(sig[:, 4:6], g[:, 4:6], AF.Sigmoid)
        nc.vector.tensor_add(cst, cst, tmp)
        tch = stagep.tile([128, 2, BL2], F32, tag="tch")
        nc.scalar.activation(tch, cst, AF.Tanh)
        nc.vector.tensor_mul(hc[:, :, j + 1], sig[:, 4:6], tch)
    for cur in cursors:
        while cur[1] < len(cur[0]):
            cur[0][cur[1]]()
            cur[1] += 1


def _build_nc():
    if "nc" in _BUILD_CACHE:
        return _BUILD_CACHE["nc"]
    nc = bacc.Bacc(target_bir_lowering=False, num_devices=NC_N)

    # ---- external parameters -------------------------------------------------
    x1_ext = nc.declare_dram_parameter("x1", [128, DK1, T * BL2], F16, isOutput=False)
    wih1_ext = nc.declare_dram_parameter("wih1", [128, DK1 * 8 * 128], F16, isOutput=False)
    whh1_ext = nc.declare_dram_parameter("whh1", [128, 2 * 8 * 128], F16, isOutput=False)
    bias1_ext = nc.declare_dram_parameter("bias1", [128, 8], F32, isOutput=False)
    wih2_ext = nc.declare_dram_parameter("wih2", [128, DK2 * 8 * 128], F16, isOutput=False)
    whh2_ext = nc.declare_dram_parameter("whh2", [128, 2 * 8 * 128], F16, isOutput=False)
    bias2_ext = nc.declare_dram_parameter("bias2", [128, 8], F32, isOutput=False)
    masks_ext = nc.declare_dram_parameter("masks", [128, 2], F32, isOutput=False)
    cls1_ext = nc.declare_dram_parameter("cls1", [128, 8 * 4 * 128], F16, isOutput=False)
    clsb1_ext = nc.declare_dram_parameter("clsb1", [128, 4], F32, isOutput=False)
    cls2_ext = nc.declare_dram_parameter("cls2", [128, 4 * 15], F16, isOutput=False)
    clsb2_ext = nc.declare_dram_parameter("clsb2", [15, 1], F32, isOutput=False)
    mexp_ext = nc.declare_dram_parameter("mexp", [15, 15], F32, isOutput=False)
    transn_ext = nc.declare_dram_parameter("transn", [15, 15], F16, isOutput=False)
    start_ext = nc.declare_dram_parameter("crfstart", [15, 1], F32, isOutput=False)
    end_ext = nc.declare_dram_parameter("crfend", [15, 1], F32, isOutput=False)
    lna_ext = nc.declare_dram_parameter("lnalpha", [15, 1], F32, isOutput=False)
    tago_ext = nc.declare_dram_parameter("tagoneT", [15, TB3], F16, isOutput=False)
    out_ext = nc.declare_dram_parameter("out", [1, 1], F32, isOutput=True)

    # ---- internal DRAM -------------------------------------------------------
    HSZ = 2 * BL2 * T  # 32768 cols/partition of h (f16)
    xg1_dram = nc.dram_tensor("xg1", [128, 8, T * BL2], F16)
    xg2_dram = nc.dram_tensor("xg2", [128, 8, T * BL2], F16)
    xg2a_dram = nc.dram_tensor("xg2a", [128, 8, T * BL2], F16)
    hmine = nc.dram_tensor("hmine", [128, HSZ], F16)
    agout = nc.dram_tensor("agout", [2, 128, HSZ], F16)
    sendb = nc.dram_tensor("sendb", [8, 128, 2 * 4 * T], F16)
    recvb = nc.dram_tensor("recvb", [8, 128, 2 * 4 * T], F16)

    with TileContext(nc) as tc:
        with (
            tc.tile_pool(name="consts", bufs=1) as consts,
            tc.tile_pool(name="seqs", bufs=1) as seqs,
            tc.tile_pool(name="work", bufs=2) as work,
            tc.tile_pool(name="stage", bufs=2) as stagep,
            tc.tile_pool(name="ps_big", bufs=3, space="PSUM") as ps_big,
            tc.tile_pool(name="ps_rec", bufs=2, space="PSUM") as ps_rec,
            tc.tile_pool(name="ps_small", bufs=3, space="PSUM") as ps_small,
        ):
            # h buffer, seq-major: [128, chunk2, b32, T+1], reused by phases 1+2
            h_sb = seqs.tile([128, 2, BL2, HCOL], F16, tag="h_sb")
            hc = seqs.tile([128, 2, UNROLL + 1, BL2], F16, tag="hcomp")
            cst = seqs.tile([128, 2, BL2], F32, tag="cstate")
            masks = consts.tile([128, 2], F32, tag="masks")
            nc.sync.dma_start(out=masks, in_=masks_ext[:, :])

            # ================= PHASE 1 (L0) =================
            wih1 = consts.tile([128, DK1, 8, 128], F16, tag="wbig")
            nc.sync.dma_start(
                out=wih1,
                in_=wih1_ext.ap().rearrange("p (k m c) -> p k m c", k=DK1, m=8))
            whh1 = consts.tile([128, 2, 8, 128], F16, tag="whh")
            nc.sync.dma_start(
                out=whh1,
                in_=whh1_ext.ap().rearrange("p (k m c) -> p k m c", k=2, m=8))
            bias1 = consts.tile([128, 8], F32, tag="bias1")
            nc.sync.dma_start(out=bias1, in_=bias1_ext[:, :])
            # wih2/bias2 load now: the L1 input GEMM's slotA half runs
            # interleaved inside the phase-1 recurrence
            wih2 = consts.tile([128, DK2, 8, 128], F16, tag="wih2")
            nc.sync.dma_start(
                out=wih2,
                in_=wih2_ext.ap().rearrange("p (k m c) -> p k m c", k=DK2, m=8))
            bias2 = consts.tile([128, 8], F32, tag="bias2")
            nc.sync.dma_start(out=bias2, in_=bias2_ext[:, :])

            CB = UNROLL * BL2
            xh = seqs.tile([128, 2, UNROLL, BL2], F16, tag="xh")
            # phase-1 h layout in DRAM is t-major: (p, c, t, b)
            hm_view = hmine.ap().rearrange("p (c t b) -> p c t b", c=2, t=T)

            def g1_items(tv_lead):
                # xg1 block tv_lead: input DMA now, 48 matmul quanta
                xb = stagep.tile([128, DK1, CB], F16, tag="xb1")
                nc.sync.dma_start(out=xb, in_=x1_ext[:, :, ds(tv_lead * BL2, CB)])
                items, pss = [], {}
                for m in range(8):
                    for k in range(DK1):
                        def it(m=m, k=k):
                            if k == 0:
                                pss[m] = ps_big.tile([128, CB], F32, tag="gemmps",
                                                     name=f"g1ps{m}")
                            nc.tensor.matmul(pss[m], wih1[:, k, m], xb[:, k],
                                             start=(k == 0), stop=(k == DK1 - 1))
                            if k == DK1 - 1:
                                st = stagep.tile([128, CB], F16, tag="xgstage",
                                                 name=f"g1st{m}")
                                nc.scalar.activation(st, pss[m], AF.Identity,
                                                     bias=bias1[:, m: m + 1])
                                nc.sync.dma_start(
                                    out=xg1_dram[:, m, ds(tv_lead * BL2, CB)], in_=st)
                        items.append(it)
                return items

            def g2a_items(tv_lag):
                # slotA half of xg2 (own h, forward) for block tv_lag from xh
                items, pss = [], {}
                for m in range(8):
                    for c in range(2):
                        def it(m=m, c=c):
                            if c == 0:
                                pss[m] = ps_big.tile([128, CB], F32, tag="gemmps",
                                                     name=f"g2aps{m}")
                            nc.tensor.matmul(
                                pss[m], wih2[:, c, m],
                                xh[:, c].rearrange("p t b -> p (t b)"),
                                start=(c == 0), stop=(c == 1))
                            if c == 1:
                                st = stagep.tile([128, CB], F16, tag="xgstage",
                                                 name=f"g2ast{m}")
                                nc.scalar.activation(st, pss[m], AF.Identity,
                                                     bias=bias2[:, m: m + 1])
                                nc.sync.dma_start(
                                    out=xg2a_dram[:, m, ds(tv_lag * BL2, CB)], in_=st)
                        items.append(it)
                return items

            def p1_end(tv):
                nc.sync.dma_start(out=hm_view[:, :, ds(tv, UNROLL)],
                                  in_=hc[:, :, 1: UNROLL + 1])
                nc.gpsimd.tensor_copy(xh, hc[:, :, 1: UNROLL + 1])
                nc.gpsimd.tensor_copy(hc[:, :, 0], hc[:, :, UNROLL])

            def rec1_block(tv, fodder):
                _emit_rec_block(nc, stagep, ps_rec, hc, cst, whh1, xg1_dram,
                                tv, fodder)
                p1_end(tv)

            nc.vector.memset(cst, 0.0)
            nc.vector.memset(hc[:, :, 0], 0.0)
            for blk in range(4):
                for it in g1_items(16 * blk):
                    it()
            rec1_block(0, [])
            rec1_block(16, [(g2a_items(0), 1)])
            with tc.For_i(32, 480, 16) as tv:
                items1 = g1_items(tv + 32)
                items2 = g2a_items(tv - 16)
                _emit_rec_block(nc, stagep, ps_rec, hc, cst, whh1, xg1_dram,
                                tv, [(items1, 3), (items2, 1)])
                p1_end(tv)
            rec1_block(480, [(g2a_items(464), 1)])
            rec1_block(496, [(g2a_items(480), 1)])
            for it in g2a_items(496):
                it()

            nc.gpsimd.collective_compute(
                "AllGather", mybir.AluOpType.bypass,
                replica_groups=[[0, 2], [1, 3], [4, 6], [5, 7]],
                ins=[hmine.ap()], outs=[agout.ap()],
            )

            # ================= PHASE 2 (L1) =================
            whh2 = consts.tile([128, 2, 8, 128], F16, tag="whh")
            nc.sync.dma_start(
                out=whh2,
                in_=whh2_ext.ap().rearrange("p (k m c) -> p k m c", k=2, m=8))

            def slot_ap(tensor_offset, tensor, c, tv, reverse):
                # [128, UNROLL*32] chunk c of a t-major h DRAM region
                # (p, c2, T, b32): a contiguous ascending t window; reversed
                # slots flip t on the SBUF side.
                if not reverse:
                    w = tv
                else:
                    w = (T - UNROLL) - tv
                off = tensor_offset + c * (T * BL2) + w * BL2
                return bass.AP(tensor=tensor, offset=off,
                               ap=[[HSZ, 128], [1, UNROLL * BL2]])

            def g2b_items(tv_lead):
                # slotB half of xg2 (peer h via AG, reversed) + A-half merge
                r0 = stagep.tile([128, 2, UNROLL, BL2], F16, tag="slotR0")
                r1 = stagep.tile([128, 2, UNROLL, BL2], F16, tag="slotR1")
                for c in range(2):
                    nc.sync.dma_start(out=r0[:, c],
                                      in_=slot_ap(0, agout, c, tv_lead, True))
                    nc.sync.dma_start(out=r1[:, c],
                                      in_=slot_ap(128 * HSZ, agout, c, tv_lead, True))
                sbr = stagep.tile([128, 2, UNROLL, BL2], F16, tag="slotBr")
                items, pss, xas = [], {}, {}

                def blend0():
                    nc.vector.tensor_scalar_mul(r0, r0, masks[:, 0:1])

                def blend1():
                    nc.vector.tensor_scalar_mul(r1, r1, masks[:, 1:2])

                def blend2():
                    # add, writing with the t window reversed
                    p_step = sbr.ap[0][0]
                    rev = bass.AP(tensor=sbr.tensor,
                                  offset=sbr.offset + (UNROLL - 1) * BL2,
                                  ap=[[p_step, 128], [UNROLL * BL2, 2],
                                      [-BL2, UNROLL], [1, BL2]])
                    nc.vector.tensor_add(rev, r0, r1)

                items += [blend0, blend1, blend2]
                for m in range(8):
                    for c in range(2):
                        def it(m=m, c=c):
                            if c == 0:
                                pss[m] = ps_big.tile([128, CB], F32, tag="gemmps",
                                                     name=f"g2bps{m}")
                                xas[m] = stagep.tile([128, CB], F16, tag="xg2am",
                                                     name=f"g2bxa{m}")
                                nc.sync.dma_start(
                                    out=xas[m],
                                    in_=xg2a_dram[:, m, ds(tv_lead * BL2, CB)])
                            nc.tensor.matmul(
                                pss[m], wih2[:, 2 + c, m],
                                sbr[:, c].rearrange("p t b -> p (t b)"),
                                start=(c == 0), stop=(c == 1))
                            if c == 1:
                                st = stagep.tile([128, CB], F16, tag="xgstage",
                                                 name=f"g2bst{m}")
                                nc.vector.tensor_add(st, pss[m], xas[m])
                                nc.sync.dma_start(
                                    out=xg2_dram[:, m, ds(tv_lead * BL2, CB)], in_=st)
                        items.append(it)
                return items

            def p2_end(tv):
                nc.gpsimd.tensor_copy(
                    h_sb[:, :, :, ds(tv + 1, UNROLL)],
                    hc[:, :, 1: UNROLL + 1].rearrange("p c t b -> p c b t"))
                nc.gpsimd.tensor_copy(hc[:, :, 0], hc[:, :, UNROLL])

            nc.vector.memset(cst, 0.0)
            nc.vector.memset(hc[:, :, 0], 0.0)
            for blk in (0, 1):
                for it in g2b_items(16 * blk):
                    it()
            with tc.For_i(0, 480, 16) as tv:
                items = g2b_items(tv + 32)
                _emit_rec_block(nc, stagep, ps_rec, hc, cst, whh2, xg2_dram,
                                tv, [(items, 2)])
                p2_end(tv)
            for blk in (30, 31):
                _emit_rec_block(nc, stagep, ps_rec, hc, cst, whh2, xg2_dram,
                                16 * blk, [])
                p2_end(16 * blk)

            # ---- AllToAll redistribution to data-parallel layout ----
            sb_view = sendb.ap().rearrange("j p (c b t) -> j p c b t", c=2, b=4)
            for j in range(8):
                for c in range(2):
                    nc.sync.dma_start(out=sb_view[j, :, c],
                                      in_=h_sb[:, c, ds(4 * j, 4), 1:HCOL])
            nc.gpsimd.collective_compute(
                "AllToAll", mybir.AluOpType.bypass,
                replica_groups=[list(range(8))],
                ins=[sendb.ap()], outs=[recvb.ap()],
            )

            # ================= PHASE 3: classifier + CRF =================
            cls1 = consts.tile([128, 8, 4, 128], F16, tag="wbig")
            nc.sync.dma_start(
                out=cls1,
                in_=cls1_ext.ap().rearrange("p (k m c) -> p k m c", k=8, m=4))
            clsb1 = consts.tile([128, 4], F32, tag="clsb1")
            nc.sync.dma_start(out=clsb1, in_=clsb1_ext[:, :])
            cls2 = consts.tile([128, 4, 15], F16, tag="cls2")
            nc.sync.dma_start(
                out=cls2, in_=cls2_ext.ap().rearrange("p (k j) -> p k j", k=4))
            clsb2 = consts.tile([15, 1], F32, tag="clsb2")
            nc.sync.dma_start(out=clsb2, in_=clsb2_ext[:, :])
            mexp = consts.tile([15, 15], F32, tag="mexp")
            nc.sync.dma_start(out=mexp, in_=mexp_ext[:, :])
            transn = consts.tile([15, 15], F16, tag="transn")
            nc.sync.dma_start(out=transn, in_=transn_ext[:, :])
            crfstart = consts.tile([15, 1], F32, tag="crfstart")
            nc.sync.dma_start(out=crfstart, in_=start_ext[:, :])
            crfend = consts.tile([15, 1], F32, tag="crfend")
            nc.sync.dma_start(out=crfend, in_=end_ext[:, :])
            lnalpha = consts.tile([15, 1], F32, tag="lnalpha")
            nc.sync.dma_start(out=lnalpha, in_=lna_ext[:, :])
            tago = consts.tile([15, TB3], F16, tag="tago")
            nc.sync.dma_start(out=tago, in_=tago_ext[:, :])

            logits = seqs.tile([15, TB3], F32, tag="logits")

            NT = 64  # t-steps per classifier n-tile (NT*BL3 = 512 cols)
            SHSZ = 128 * 2 * 4 * T  # elements per recv shard

            def comb_ap(kk, half, ns):
                # [128, 4, NT]: dir kk//2, chunk kk%2, half-shard, n-tile ns;
                # always an ascending t window (reversal done in the rhs view)
                d, c = kk // 2, kk % 2
                rev = d in (1, 3)  # c1b, w1b stored time-reversed
                base = (2 * d + half) * SHSZ + c * (4 * T)
                if not rev:
                    off = base + ns * NT
                else:
                    off = base + (T - NT) - ns * NT
                return bass.AP(tensor=recvb, offset=off,
                               ap=[[2 * 4 * T, 128], [T, 4], [1, NT]])

            for ns in range(8):
                comb = stagep.tile([128, 8, BL3, NT], F16, tag="comb", bufs=2)
                for kk in range(8):
                    for half in range(2):
                        nc.sync.dma_start(out=comb[:, kk, ds(4 * half, 4)],
                                          in_=comb_ap(kk, half, ns))
                hmt = []
                for m in range(4):
                    ps = ps_big.tile([128, NT * BL3], F32, tag="gemmps")
                    for kk in range(8):
                        if kk // 2 in (1, 3):
                            p_step = comb.ap[0][0]
                            off = (comb.offset + kk * BL3 * NT + (NT - 1))
                            rhs = bass.AP(tensor=comb.tensor, offset=off,
                                          ap=[[p_step, 128], [-1, NT],
                                              [NT, BL3]])
                        else:
                            rhs = comb[:, kk].rearrange("p b t -> p t b")
                        nc.tensor.matmul(ps, cls1[:, kk, m], rhs,
                                         start=(kk == 0), stop=(kk == 7))
                    hm = stagep.tile([128, NT * BL3], F16, tag="hm", bufs=4,
                                     name=f"hm{m}")
                    nc.scalar.activation(hm, ps, AF.Relu, bias=clsb1[:, m: m + 1])
                    hmt.append(hm)
                ps2 = ps_small.tile([15, NT * BL3], F32, tag="small")
                for m in range(4):
                    nc.tensor.matmul(ps2, cls2[:, m], hmt[m],
                                     start=(m == 0), stop=(m == 3))
                nc.vector.tensor_scalar_add(
                    logits[:, ds(ns * NT * BL3, NT * BL3)], ps2, clsb2)

            # fold CRF start/end into first/last emission columns
            nc.vector.tensor_scalar_add(logits[:, 0:BL3], logits[:, 0:BL3], crfstart)
            nc.vector.tensor_scalar_add(logits[:, TB3 - BL3: TB3],
                                        logits[:, TB3 - BL3: TB3], crfend)

            # ---- CRF numerator ----
            racc = work.tile([15, 16], F32, tag="racc")
            nc.vector.memset(racc, 0.0)
            for ns in range(8):
                pre = stagep.tile([15, 512], F32, tag="prodns")
                nc.vector.tensor_mul(pre, logits[:, ds(ns * 512, 512)],
                                     tago[:, ds(ns * 512, 512)])
                nc.vector.tensor_reduce(racc[:, 8 + ns: 9 + ns], pre,
                                        axis=mybir.AxisListType.X,
                                        op=mybir.AluOpType.add)
                psv = ps_small.tile([15, 512], F32, tag="small")
                nc.tensor.matmul(psv, transn, tago[:, ds(ns * 512, 512)],
                                 start=True, stop=True)
                w = 512 if ns < 7 else 512 - BL3
                pr = stagep.tile([15, 512], F32, tag="prodns")
                nc.vector.tensor_mul(pr[:, :w], psv[:, :w],
                                     tago[:, ds(ns * 512 + BL3, w)])
                nc.vector.tensor_reduce(racc[:, ns: ns + 1], pr[:, :w],
                                        axis=mybir.AxisListType.X,
                                        op=mybir.AluOpType.add)
            nv = stagep.tile([15, 1], F32, tag="nv")
            nc.vector.tensor_reduce(nv, racc, axis=mybir.AxisListType.X,
                                    op=mybir.AluOpType.add)
            ones15 = consts.tile([15, 1], F32, tag="ones15")
            nc.vector.memset(ones15, 1.0)
            psn = ps_small.tile([1, 1], F32, tag="small")
            nc.tensor.matmul(psn, ones15, nv, start=True, stop=True)
            num11 = work.tile([1, 1], F32, tag="num11")
            nc.vector.tensor_copy(num11, psn)

            # ---- CRF forward scan, probability space ----
            # p0 first, then E = alpha*exp(logits) computed in place over
            # logits (last col already includes e^end)
            p = seqs.tile([15, BL3], F32, tag="pvec")
            nc.scalar.activation(p, logits[:, 0:BL3], AF.Exp)
            E = logits
            nc.scalar.activation(E, logits, AF.Exp, bias=lnalpha)

            def crf_step(t_col_ap):
                z = ps_small.tile([15, BL3], F32, tag="small")
                nc.tensor.matmul(z, mexp, p, start=True, stop=True)
                nc.vector.tensor_mul(p, z, t_col_ap)

            for t in range(1, 16):
                crf_step(E[:, t * BL3: (t + 1) * BL3])
            with tc.For_i(0, 496, UNROLL) as tv:
                for j in range(UNROLL):
                    crf_step(E[:, ds((16 + j) * BL3 + tv * BL3, BL3)])

            # ---- denominator + output ----
            psd = ps_small.tile([1, BL3], F32, tag="small")
            nc.tensor.matmul(psd, ones15, p, start=True, stop=True)
            ln8 = stagep.tile([1, BL3], F32, tag="ln8")
            nc.scalar.activation(ln8, psd, AF.Ln)
            den11 = work.tile([1, 1], F32, tag="den11")
            nc.vector.tensor_reduce(den11, ln8, axis=mybir.AxisListType.X,
                                    op=mybir.AluOpType.add)
            res = work.tile([1, 1], F32, tag="res")
            nc.vector.tensor_sub(res, den11, num11)
            nc.sync.dma_start(out=out_ext[:, :], in_=res)

    nc.finalize()
    _BUILD_CACHE["nc"] = nc
    return nc


# ---- host-side input prep ---------------------------------------------------

# gate perm [i(256), f(256), g(256), o(256)] -> [i, f, o, g~]
_GPERM = np.concatenate([np.arange(0, 512), np.arange(768, 1024), np.arange(512, 768)])

# core c -> (pathway, direction, half): 0..3 char f/f/b/b, 4..7 word
_ROLES = [("c", 0, 0), ("c", 0, 1), ("c", 1, 0), ("c", 1, 1),
          ("w", 0, 0), ("w", 0, 1), ("w", 1, 0), ("w", 1, 1)]


def _wih_prep(W, dk_n):
    Wp = W[_GPERM]
    return np.ascontiguousarray(
        Wp.reshape(8, 128, dk_n, 128).transpose(3, 2, 0, 1).reshape(128, dk_n * 8 * 128)
    ).astype(np.float16)


def _make_in_maps(inputs):
    char_ids = np.asarray(inputs["char_ids"])
    tags = np.asarray(inputs["tags"])
    wemb = np.asarray(inputs["word_embeddings"], np.float32)
    emb = np.asarray(inputs["char_emb_table"], np.float32)
    trans = np.asarray(inputs["crf_trans"], np.float32)

    alpha = 1.0 / (15.0 * float(np.exp(trans).mean()))
    common = {}
    w1 = np.asarray(inputs["cls_w1"], np.float32)
    common["cls1"] = np.ascontiguousarray(
        w1.reshape(4, 128, 8, 128).transpose(3, 2, 0, 1).reshape(128, 8 * 4 * 128)
    ).astype(np.float16)
    common["clsb1"] = np.ascontiguousarray(
        np.asarray(inputs["cls_b1"], np.float32).reshape(4, 128).T).astype(np.float32)
    w2 = np.asarray(inputs["cls_w2"], np.float32)
    common["cls2"] = np.ascontiguousarray(
        w2.reshape(15, 4, 128).transpose(2, 1, 0).reshape(128, 4 * 15)).astype(np.float16)
    common["clsb2"] = np.asarray(inputs["cls_b2"], np.float32).reshape(15, 1).copy()
    common["mexp"] = np.exp(trans).astype(np.float32)
    common["transn"] = trans.astype(np.float16)
    common["crfstart"] = np.asarray(inputs["crf_start"], np.float32).reshape(15, 1).copy()
    common["crfend"] = np.asarray(inputs["crf_end"], np.float32).reshape(15, 1).copy()
    common["lnalpha"] = np.full((15, 1), np.log(alpha), np.float32)

    in_maps = []
    for c in range(NC_N):
        pw, d, hf = _ROLES[c]
        lo, hi = hf * BL2, (hf + 1) * BL2
        m = dict(common)

        # phase-1 weights/input
        if pw == "c":
            Wih1 = np.zeros((1024, 768), np.float32)
            Wih1[:, :128] = np.asarray(inputs["c0_Wih"], np.float32)[d]
            Whh1 = np.asarray(inputs["c0_Whh"], np.float32)[d]
            b1 = (np.asarray(inputs["c0_bih"], np.float32)[d]
                  + np.asarray(inputs["c0_bhh"], np.float32)[d])
            ce = emb[char_ids[lo:hi]]  # (32, 512, 128)
            X = np.zeros((128, DK1, T, BL2), np.float32)
            X[:, 0] = ce.transpose(2, 1, 0)
            Wl1 = np.asarray(inputs["c1_Wih"], np.float32)[d]
            Whh2 = np.asarray(inputs["c1_Whh"], np.float32)[d]
            b2 = (np.asarray(inputs["c1_bih"], np.float32)[d]
                  + np.asarray(inputs["c1_bhh"], np.float32)[d])
        else:
            Wih1 = np.asarray(inputs["w0_Wih"], np.float32)[d]
            Whh1 = np.asarray(inputs["w0_Whh"], np.float32)[d]
            b1 = (np.asarray(inputs["w0_bih"], np.float32)[d]
                  + np.asarray(inputs["w0_bhh"], np.float32)[d])
            X = wemb[lo:hi].reshape(BL2, T, DK1, 128).transpose(3, 2, 1, 0)
            Wl1 = np.asarray(inputs["w1_Wih"], np.float32)[d]
            Whh2 = np.asarray(inputs["w1_Whh"], np.float32)[d]
            b2 = (np.asarray(inputs["w1_bih"], np.float32)[d]
                  + np.asarray(inputs["w1_bhh"], np.float32)[d])
        if d == 1:  # backward: reverse local time
            X = X[:, :, ::-1]
        m["x1"] = np.ascontiguousarray(X.reshape(128, DK1, T * BL2)).astype(np.float16)
        m["wih1"] = _wih_prep(Wih1, DK1)
        m["whh1"] = _wih_prep(Whh1, 2)
        m["bias1"] = np.ascontiguousarray(b1[_GPERM].reshape(8, 128).T).astype(np.float32)

        # phase-2 weights: columns [own(256) | peer(256)]
        if d == 1:
            Wl1 = Wl1[:, np.r_[256:512, 0:256]]
        m["wih2"] = _wih_prep(Wl1, DK2)
        m["whh2"] = _wih_prep(Whh2, 2)
        m["bias2"] = np.ascontiguousarray(b2[_GPERM].reshape(8, 128).T).astype(np.float32)
        # blend: f-core (d=0) picks AG region 1 (the b-core), b-core picks 0
        msk = np.zeros((128, 2), np.float32)
        msk[:, 1 - d] = 1.0
        m["masks"] = msk

        # phase-3 tags for this core's 8 sequences
        seqs3 = np.r_[4 * c: 4 * c + 4, 32 + 4 * c: 32 + 4 * c + 4]
        oh = (np.arange(K)[:, None, None] == tags[seqs3][None]).astype(np.float32)
        # (15, 8seq, 512t) -> (15, t, b)
        m["tagoneT"] = np.ascontiguousarray(
            oh.transpose(0, 2, 1).reshape(K, TB3)).astype(np.float16)
        in_maps.append(m)
    return in_maps, alpha


def kernel(**inputs):
    nc = _build_nc()
    in_maps, alpha = _make_in_maps(inputs)
    res = run_bass_kernel_spmd(nc, in_maps, core_ids=list(range(NC_N)))
    total = sum(float(res.results[c]["out"][0, 0]) for c in range(NC_N))
    total -= B * (T - 1) * np.log(alpha)
    return np.float32(total / B)


# revision 17
# speedup vs baseline: 4.7334x; 1.0973x over previous
"""BiLSTM dual-pathway + CRF NLL kernel for 8 Trainium2 NeuronCores.

Sharding: direction-parallel for the LSTM recurrences, data-parallel for the
classifier/CRF. Phase 1 runs the four layer-0 directions (char fwd/bwd, word
fwd/bwd) on 8 cores as (direction x batch-half), batch 32 per core, so the
recurrent matmuls run at N=32 instead of N=8 and the sequential chain count
drops from 4096 to 1024 steps. A pairwise AllGather exchanges the L0 hidden
states between fwd/bwd cores, phase 2 runs the four layer-1 directions the
same way, then an 8-rank AllToAll redistributes hidden states to a
data-parallel layout (8 sequences per core) for the classifier and CRF.

SPMD uniformity: every core runs the identical program. Backward directions
receive host-time-reversed inputs; reversed reads of peer hidden states are
fixed negative-stride APs, with host-permuted weight columns absorbing the
f/b role differences. Peer-region selection after the AllGather uses per-core
0/1 blend masks delivered as input data.

The CRF forward scan runs in probability space with a constant per-step
prescale alpha folded into the emission exponentials (corrected analytically
on the host), so each step is one resident-weight 15x15 matmul plus one
vector multiply -- no per-step exp/ln activation-table swaps.
"""

import sys

sys.path.insert(0, "/opt/trn_rl_repo")

import numpy as np

import concourse.bass as bass
import concourse.mybir as mybir
from concourse import bacc
from concourse.bass import ds
from concourse.tile import TileContext
from concourse.bass_utils import run_bass_kernel_spmd

F16 = mybir.dt.float16
F32 = mybir.dt.float32
AF = mybir.ActivationFunctionType

B, T, V, K = 64, 512, 40, 15
NC_N = 8
BL2 = 32            # batch per core in phases 1-2
BL3 = 8             # sequences per core in phase 3
TB3 = T * BL3       # 4096 classifier/CRF columns per core
DK1, DK2 = 6, 4     # input chunks for L0 (word=768, char padded) and L1 (512)
UNROLL = 16
HCOL = T + 1        # h buffer columns per sequence (col 0 = zero init)

_BUILD_CACHE = {}


def _emit_rec_block(nc, stagep, ps_rec, hc, cst, whh, xg_dram, tv, fodder):
    """Emit 16 recurrence steps for time block tv.

    fodder: list of (items, per_step) - closures emitting one PE-side quantum
    each (GEMM matmuls for other pipeline stages), dispensed between the
    step's recurrent matmuls and its vector/scalar chain so they execute in
    the PE's dependency-stall gaps.
    """
    CB = UNROLL * BL2
    xgs = stagep.tile([128, 8, CB], F16, tag="xgs")
    nc.sync.dma_start(out=xgs, in_=xg_dram[:, :, ds(tv * BL2, CB)])
    cursors = [[items, 0, per_step] for items, per_step in fodder]
    for j in range(UNROLL):
        psA = ps_rec.tile([128, 4, BL2], F32, tag="recpsA")
        psB = ps_rec.tile([128, 4, BL2], F32, tag="recpsB")
        # i,f gates (chunks 0-3) first into psA so the first g-add can
        # start while the o/g~ matmuls (psB) still run
        for m in (0, 1, 2, 3):
            for k in range(2):
                nc.tensor.matmul(psA[:, m], whh[:, k, m],
                                 hc[:, k, j], start=(k == 0), stop=(k == 1))
        for m in (6, 7, 4, 5):
            for k in range(2):
                nc.tensor.matmul(psB[:, m - 4], whh[:, k, m],
                                 hc[:, k, j], start=(k == 0), stop=(k == 1))
        for cur in cursors:
            for _ in range(cur[2]):
                if cur[1] < len(cur[0]):
                    cur[0][cur[1]]()
                    cur[1] += 1
        g = stagep.tile([128, 8, BL2], F32, tag="g")
        xsl = xgs[:, :, ds(j * BL2, BL2)]
        nc.vector.tensor_add(g[:, 0:4], psA, xsl[:, 0:4])
        sig = stagep.tile([128, 6, BL2], F32, tag="sig")
        nc.scalar.activation(sig[:, 0:4], g[:, 0:4], AF.Sigmoid)
        nc.vector.tensor_mul(cst, cst, sig[:, 2:4])
        nc.vector.tensor_add(g[:, 4:8], psB, xsl[:, 4:8])
        tgg = stagep.tile([128, 2, BL2], F32, tag="tgg")
        nc.scalar.activation(tgg, g[:, 6:8], AF.Tanh)
        tmp = stagep.tile([128, 2, BL2], F32, tag="tmpig")
        nc.vector.tensor_mul(tmp, sig[:, 0:2], tgg)
        nc.scalar.activation(sig[:, 4:6], g[:, 4:6], AF.Sigmoid)
        nc.vector.tensor_add(cst, cst, tmp)
        tch = stagep.tile([128, 2, BL2], F32, tag="tch")
        nc.scalar.activation(tch, cst, AF.Tanh)
        nc.vector.tensor_mul(hc[:, :, j + 1], sig[:, 4:6], tch)
    for cur in cursors:
        while cur[1] < len(cur[0]):
            cur[0][cur[1]]()
            cur[1] += 1


def _build_nc():
    if "nc" in _BUILD_CACHE:
        return _BUILD_CACHE["nc"]
    nc = bacc.Bacc(target_bir_lowering=False, num_devices=NC_N)

    # ---- external parameters -------------------------------------------------
    x1_ext = nc.declare_dram_parameter("x1", [128, DK1, T * BL2], F16, isOutput=False)
    wih1_ext = nc.declare_dram_parameter("wih1", [128, DK1 * 8 * 128], F16, isOutput=False)
    whh1_ext = nc.declare_dram_parameter("whh1", [128, 2 * 8 * 128], F16, isOutput=False)
    bias1_ext = nc.declare_dram_parameter("bias1", [128, 8], F32, isOutput=False)
    wih2_ext = nc.declare_dram_parameter("wih2", [128, DK2 * 8 * 128], F16, isOutput=False)
    whh2_ext = nc.declare_dram_parameter("whh2", [128, 2 * 8 * 128], F16, isOutput=False)
    bias2_ext = nc.declare_dram_parameter("bias2", [128, 8], F32, isOutput=False)
    masks_ext = nc.declare_dram_parameter("masks", [128, 2], F32, isOutput=False)
    cls1_ext = nc.declare_dram_parameter("cls1", [128, 8 * 4 * 128], F16, isOutput=False)
    clsb1_ext = nc.declare_dram_parameter("clsb1", [128, 4], F32, isOutput=False)
    cls2_ext = nc.declare_dram_parameter("cls2", [128, 4 * 15], F16, isOutput=False)
    clsb2_ext = nc.declare_dram_parameter("clsb2", [15, 1], F32, isOutput=False)
    mexp_ext = nc.declare_dram_parameter("mexp", [15, 15], F32, isOutput=False)
    transn_ext = nc.declare_dram_parameter("transn", [15, 15], F16, isOutput=False)
    start_ext = nc.declare_dram_parameter("crfstart", [15, 1], F32, isOutput=False)
    end_ext = nc.declare_dram_parameter("crfend", [15, 1], F32, isOutput=False)
    lna_ext = nc.declare_dram_parameter("lnalpha", [15, 1], F32, isOutput=False)
    tago_ext = nc.declare_dram_parameter("tagoneT", [15, TB3], F16, isOutput=False)
    out_ext = nc.declare_dram_parameter("out", [1, 1], F32, isOutput=True)

    # ---- internal DRAM -------------------------------------------------------
    HSZ = 2 * BL2 * T  # 32768 cols/partition of h (f16)
    xg1_dram = nc.dram_tensor("xg1", [128, 8, T * BL2], F16)
    xg2_dram = nc.dram_tensor("xg2", [128, 8, T * BL2], F16)
    hmine = nc.dram_tensor("hmine", [128, HSZ], F16)
    agout = nc.dram_tensor("agout", [2, 128, HSZ], F16)
    sendb = nc.dram_tensor("sendb", [8, 128, 2 * 4 * T], F16)
    recvb = nc.dram_tensor("recvb", [8, 128, 2 * 4 * T], F16)

    with TileContext(nc) as tc:
        with (
            tc.tile_pool(name="consts", bufs=1) as consts,
            tc.tile_pool(name="seqs", bufs=1) as seqs,
            tc.tile_pool(name="work", bufs=2) as work,
            tc.tile_pool(name="stage", bufs=2) as stagep,
            tc.tile_pool(name="ps_big", bufs=3, space="PSUM") as ps_big,
            tc.tile_pool(name="ps_rec", bufs=1, space="PSUM") as ps_rec,
            tc.tile_pool(name="ps_small", bufs=3, space="PSUM") as ps_small,
        ):
            # h buffer, seq-major: [128, chunk2, b32, T+1], reused by phases 1+2
            h_sb = seqs.tile([128, 2, BL2, HCOL], F16, tag="h_sb")
            hc = seqs.tile([128, 2, UNROLL + 1, BL2], F16, tag="hcomp")
            cst = seqs.tile([128, 2, BL2], F32, tag="cstate")
            masks = consts.tile([128, 2], F32, tag="masks")
            nc.sync.dma_start(out=masks, in_=masks_ext[:, :])

            # ================= PHASE 1 (L0) =================
            wih1 = consts.tile([128, DK1, 8, 128], F16, tag="wbig")
            nc.sync.dma_start(
                out=wih1,
                in_=wih1_ext.ap().rearrange("p (k m c) -> p k m c", k=DK1, m=8))
            whh1 = consts.tile([128, 2, 8, 128], F16, tag="whh")
            nc.sync.dma_start(
                out=whh1,
                in_=whh1_ext.ap().rearrange("p (k m c) -> p k m c", k=2, m=8))
            bias1 = consts.tile([128, 8], F32, tag="bias1")
            nc.sync.dma_start(out=bias1, in_=bias1_ext[:, :])
            # wih2/bias2 load now: the L1 input GEMM's slotA half runs
            # interleaved inside the phase-1 recurrence
            wih2 = consts.tile([128, DK2, 8, 128], F16, tag="wih2")
            nc.sync.dma_start(
                out=wih2,
                in_=wih2_ext.ap().rearrange("p (k m c) -> p k m c", k=DK2, m=8))
            bias2 = consts.tile([128, 8], F32, tag="bias2")
            nc.sync.dma_start(out=bias2, in_=bias2_ext[:, :])

            CB = UNROLL * BL2
            # phase-1 h layout in DRAM is t-major: (p, c, t, b)
            hm_view = hmine.ap().rearrange("p (c t b) -> p c t b", c=2, t=T)

            def g1_items(tv_lead):
                # xg1 block tv_lead: input DMA now, 48 matmul quanta
                xb = stagep.tile([128, DK1, CB], F16, tag="xb1")
                nc.sync.dma_start(out=xb, in_=x1_ext[:, :, ds(tv_lead * BL2, CB)])
                items, pss = [], {}
                for m in range(8):
                    for k in range(DK1):
                        def it(m=m, k=k):
                            if k == 0:
                                pss[m] = ps_big.tile([128, CB], F32, tag="gemmps",
                                                     name=f"g1ps{m}")
                            nc.tensor.matmul(pss[m], wih1[:, k, m], xb[:, k],
                                             start=(k == 0), stop=(k == DK1 - 1))
                            if k == DK1 - 1:
                                st = stagep.tile([128, CB], F16, tag="xgstage",
                                                 name=f"g1st{m}")
                                nc.scalar.activation(st, pss[m], AF.Identity,
                                                     bias=bias1[:, m: m + 1])
                                nc.sync.dma_start(
                                    out=xg1_dram[:, m, ds(tv_lead * BL2, CB)], in_=st)
                        items.append(it)
                return items

            def p1_end(tv):
                nc.sync.dma_start(out=hm_view[:, :, ds(tv, UNROLL)],
                                  in_=hc[:, :, 1: UNROLL + 1])
                nc.gpsimd.tensor_copy(hc[:, :, 0], hc[:, :, UNROLL])

            def rec1_block(tv, fodder):
                _emit_rec_block(nc, stagep, ps_rec, hc, cst, whh1, xg1_dram,
                                tv, fodder)
                p1_end(tv)

            nc.vector.memset(cst, 0.0)
            nc.vector.memset(hc[:, :, 0], 0.0)
            for blk in range(2):
                for it in g1_items(16 * blk):
                    it()
            rec1_block(0, [(g1_items(32), 3)])
            rec1_block(16, [(g1_items(48), 3)])
            with tc.For_i(32, 480, 16) as tv:
                items1 = g1_items(tv + 32)
                _emit_rec_block(nc, stagep, ps_rec, hc, cst, whh1, xg1_dram,
                                tv, [(items1, 3)])
                p1_end(tv)
            rec1_block(480, [])
            rec1_block(496, [])

            nc.gpsimd.collective_compute(
                "AllGather", mybir.AluOpType.bypass,
                replica_groups=[[0, 2], [1, 3], [4, 6], [5, 7]],
                ins=[hmine.ap()], outs=[agout.ap()],
            )

            # ================= PHASE 2 (L1) =================
            whh2 = consts.tile([128, 2, 8, 128], F16, tag="whh")
            nc.sync.dma_start(
                out=whh2,
                in_=whh2_ext.ap().rearrange("p (k m c) -> p k m c", k=2, m=8))

            def slot_ap(tensor_offset, tensor, c, tv, reverse):
                # [128, UNROLL*32] chunk c of a t-major h DRAM region
                # (p, c2, T, b32): a contiguous ascending t window; reversed
                # slots flip t on the SBUF side.
                if not reverse:
                    w = tv
                else:
                    w = (T - UNROLL) - tv
                off = tensor_offset + c * (T * BL2) + w * BL2
                return bass.AP(tensor=tensor, offset=off,
                               ap=[[HSZ, 128], [1, UNROLL * BL2]])

            def g2_items(tv_lead):
                # full xg2 block: slotA = own h fwd, slotB = peer h reversed
                sa = stagep.tile([128, 2, UNROLL, BL2], F16, tag="slotA")
                r0 = stagep.tile([128, 2, UNROLL, BL2], F16, tag="slotR0")
                r1 = stagep.tile([128, 2, UNROLL, BL2], F16, tag="slotR1")
                for c in range(2):
                    nc.sync.dma_start(out=sa[:, c],
                                      in_=slot_ap(0, hmine, c, tv_lead, False))
                    nc.sync.dma_start(out=r0[:, c],
                                      in_=slot_ap(0, agout, c, tv_lead, True))
                    nc.sync.dma_start(out=r1[:, c],
                                      in_=slot_ap(128 * HSZ, agout, c, tv_lead, True))
                sbr = stagep.tile([128, 2, UNROLL, BL2], F16, tag="slotBr")
                items, pss = [], {}

                def blend0():
                    nc.vector.tensor_scalar_mul(r0, r0, masks[:, 0:1])

                def blend1():
                    nc.vector.tensor_scalar_mul(r1, r1, masks[:, 1:2])

                def blend2():
                    # add, writing with the t window reversed
                    p_step = sbr.ap[0][0]
                    rev = bass.AP(tensor=sbr.tensor,
                                  offset=sbr.offset + (UNROLL - 1) * BL2,
                                  ap=[[p_step, 128], [UNROLL * BL2, 2],
                                      [-BL2, UNROLL], [1, BL2]])
                    nc.vector.tensor_add(rev, r0, r1)

                items += [blend0, blend1, blend2]

                def src_k(k):
                    t = sa if k < 2 else sbr
                    return t[:, k % 2].rearrange("p t b -> p (t b)")

                for m in range(8):
                    for k in range(DK2):
                        def it(m=m, k=k):
                            if k == 0:
                                pss[m] = ps_big.tile([128, CB], F32, tag="gemmps",
                                                     name=f"g2ps{m}")
                            nc.tensor.matmul(pss[m], wih2[:, k, m], src_k(k),
                                             start=(k == 0), stop=(k == DK2 - 1))
                            if k == DK2 - 1:
                                st = stagep.tile([128, CB], F16, tag="xgstage",
                                                 name=f"g2st{m}")
                                nc.scalar.activation(st, pss[m], AF.Identity,
                                                     bias=bias2[:, m: m + 1])
                                nc.sync.dma_start(
                                    out=xg2_dram[:, m, ds(tv_lead * BL2, CB)], in_=st)
                        items.append(it)
                return items

            def p2_end(tv):
                nc.gpsimd.tensor_copy(
                    h_sb[:, :, :, ds(tv + 1, UNROLL)],
                    hc[:, :, 1: UNROLL + 1].rearrange("p c t b -> p c b t"))
                nc.gpsimd.tensor_copy(hc[:, :, 0], hc[:, :, UNROLL])

            nc.vector.memset(cst, 0.0)
            nc.vector.memset(hc[:, :, 0], 0.0)
            for blk in (0, 1):
                for it in g2_items(16 * blk):
                    it()
            with tc.For_i(0, 480, 16) as tv:
                items = g2_items(tv + 32)
                _emit_rec_block(nc, stagep, ps_rec, hc, cst, whh2, xg2_dram,
                                tv, [(items, 3)])
                p2_end(tv)
            for blk in (30, 31):
                _emit_rec_block(nc, stagep, ps_rec, hc, cst, whh2, xg2_dram,
                                16 * blk, [])
                p2_end(16 * blk)

            # ---- AllToAll redistribution to data-parallel layout ----
            sb_view = sendb.ap().rearrange("j p (c b t) -> j p c b t", c=2, b=4)
            for j in range(8):
                for c in range(2):
                    nc.sync.dma_start(out=sb_view[j, :, c],
                                      in_=h_sb[:, c, ds(4 * j, 4), 1:HCOL])
            nc.gpsimd.collective_compute(
                "AllToAll", mybir.AluOpType.bypass,
                replica_groups=[list(range(8))],
                ins=[sendb.ap()], outs=[recvb.ap()],
            )

            # ================= PHASE 3: classifier + CRF =================
            cls1 = consts.tile([128, 8, 4, 128], F16, tag="wbig")
            nc.sync.dma_start(
                out=cls1,
                in_=cls1_ext.ap().rearrange("p (k m c) -> p k m c", k=8, m=4))
            clsb1 = consts.tile([128, 4], F32, tag="clsb1")
            nc.sync.dma_start(out=clsb1, in_=clsb1_ext[:, :])
            cls2 = consts.tile([128, 4, 15], F16, tag="cls2")
            nc.sync.dma_start(
                out=cls2, in_=cls2_ext.ap().rearrange("p (k j) -> p k j", k=4))
            clsb2 = consts.tile([15, 1], F32, tag="clsb2")
            nc.sync.dma_start(out=clsb2, in_=clsb2_ext[:, :])
            mexp = consts.tile([15, 15], F32, tag="mexp")
            nc.sync.dma_start(out=mexp, in_=mexp_ext[:, :])
            transn = consts.tile([15, 15], F16, tag="transn")
            nc.sync.dma_start(out=transn, in_=transn_ext[:, :])
            crfstart = consts.tile([15, 1], F32, tag="crfstart")
            nc.sync.dma_start(out=crfstart, in_=start_ext[:, :])
            crfend = consts.tile([15, 1], F32, tag="crfend")
            nc.sync.dma_start(out=crfend, in_=end_ext[:, :])
            lnalpha = consts.tile([15, 1], F32, tag="lnalpha")
            nc.sync.dma_start(out=lnalpha, in_=lna_ext[:, :])
            tago = consts.tile([15, TB3], F16, tag="tago")
            nc.sync.dma_start(out=tago, in_=tago_ext[:, :])

            logits = seqs.tile([15, TB3], F32, tag="logits")

            NT = 64  # t-steps per classifier n-tile (NT*BL3 = 512 cols)
            SHSZ = 128 * 2 * 4 * T  # elements per recv shard

            def comb_ap(kk, half, ns):
                # [128, 4, NT]: dir kk//2, chunk kk%2, half-shard, n-tile ns;
                # always an ascending t window (reversal done in the rhs view)
                d, c = kk // 2, kk % 2
                rev = d in (1, 3)  # c1b, w1b stored time-reversed
                base = (2 * d + half) * SHSZ + c * (4 * T)
                if not rev:
                    off = base + ns * NT
                else:
                    off = base + (T - NT) - ns * NT
                return bass.AP(tensor=recvb, offset=off,
                               ap=[[2 * 4 * T, 128], [T, 4], [1, NT]])

            for ns in range(8):
                comb = stagep.tile([128, 8, BL3, NT], F16, tag="comb", bufs=2)
                for kk in range(8):
                    for half in range(2):
                        nc.sync.dma_start(out=comb[:, kk, ds(4 * half, 4)],
                                          in_=comb_ap(kk, half, ns))
                hmt = []
                for m in range(4):
                    ps = ps_big.tile([128, NT * BL3], F32, tag="gemmps")
                    for kk in range(8):
                        if kk // 2 in (1, 3):
                            p_step = comb.ap[0][0]
                            off = (comb.offset + kk * BL3 * NT + (NT - 1))
                            rhs = bass.AP(tensor=comb.tensor, offset=off,
                                          ap=[[p_step, 128], [-1, NT],
                                              [NT, BL3]])
                        else:
                            rhs = comb[:, kk].rearrange("p b t -> p t b")
                        nc.tensor.matmul(ps, cls1[:, kk, m], rhs,
                                         start=(kk == 0), stop=(kk == 7))
                    hm = stagep.tile([128, NT * BL3], F16, tag="hm", bufs=4,
                                     name=f"hm{m}")
                    nc.scalar.activation(hm, ps, AF.Relu, bias=clsb1[:, m: m + 1])
                    hmt.append(hm)
                ps2 = ps_small.tile([15, NT * BL3], F32, tag="small")
                for m in range(4):
                    nc.tensor.matmul(ps2, cls2[:, m], hmt[m],
                                     start=(m == 0), stop=(m == 3))
                nc.vector.tensor_scalar_add(
                    logits[:, ds(ns * NT * BL3, NT * BL3)], ps2, clsb2)

            # fold CRF start/end into first/last emission columns
            nc.vector.tensor_scalar_add(logits[:, 0:BL3], logits[:, 0:BL3], crfstart)
            nc.vector.tensor_scalar_add(logits[:, TB3 - BL3: TB3],
                                        logits[:, TB3 - BL3: TB3], crfend)

            # ---- CRF numerator ----
            racc = work.tile([15, 16], F32, tag="racc")
            nc.vector.memset(racc, 0.0)
            for ns in range(8):
                pre = stagep.tile([15, 512], F32, tag="prodns")
                nc.vector.tensor_mul(pre, logits[:, ds(ns * 512, 512)],
                                     tago[:, ds(ns * 512, 512)])
                nc.vector.tensor_reduce(racc[:, 8 + ns: 9 + ns], pre,
                                        axis=mybir.AxisListType.X,
                                        op=mybir.AluOpType.add)
                psv = ps_small.tile([15, 512], F32, tag="small")
                nc.tensor.matmul(psv, transn, tago[:, ds(ns * 512, 512)],
                                 start=True, stop=True)
                w = 512 if ns < 7 else 512 - BL3
                pr = stagep.tile([15, 512], F32, tag="prodns")
                nc.vector.tensor_mul(pr[:, :w], psv[:, :w],
                                     tago[:, ds(ns * 512 + BL3, w)])
                nc.vector.tensor_reduce(racc[:, ns: ns + 1], pr[:, :w],
                                        axis=mybir.AxisListType.X,
                                        op=mybir.AluOpType.add)
            nv = stagep.tile([15, 1], F32, tag="nv")
            nc.vector.tensor_reduce(nv, racc, axis=mybir.AxisListType.X,
                                    op=mybir.AluOpType.add)
            ones15 = consts.tile([15, 1], F32, tag="ones15")
            nc.vector.memset(ones15, 1.0)
            psn = ps_small.tile([1, 1], F32, tag="small")
            nc.tensor.matmul(psn, ones15, nv, start=True, stop=True)
            num11 = work.tile([1, 1], F32, tag="num11")
            nc.vector.tensor_copy(num11, psn)

            # ---- CRF forward scan, probability space ----
            # p0 first, then E = alpha*exp(logits) computed in place over
            # logits (last col already includes e^end)
            p = seqs.tile([15, BL3], F32, tag="pvec")
            nc.scalar.activation(p, logits[:, 0:BL3], AF.Exp)
            E = logits
            nc.scalar.activation(E, logits, AF.Exp, bias=lnalpha)

            def crf_step(t_col_ap):
                z = ps_small.tile([15, BL3], F32, tag="small")
                nc.tensor.matmul(z, mexp, p, start=True, stop=True)
                nc.vector.tensor_mul(p, z, t_col_ap)

            for t in range(1, 16):
                crf_step(E[:, t * BL3: (t + 1) * BL3])
            with tc.For_i(0, 496, UNROLL) as tv:
                for j in range(UNROLL):
                    crf_step(E[:, ds((16 + j) * BL3 + tv * BL3, BL3)])

            # ---- denominator + output ----
            psd = ps_small.tile([1, BL3], F32, tag="small")
            nc.tensor.matmul(psd, ones15, p, start=True, stop=True)
            ln8 = stagep.tile([1, BL3], F32, tag="ln8")
            nc.scalar.activation(ln8, psd, AF.Ln)
            den11 = work.tile([1, 1], F32, tag="den11")
            nc.vector.tensor_reduce(den11, ln8, axis=mybir.AxisListType.X,
                                    op=mybir.AluOpType.add)
            res = work.tile([1, 1], F32, tag="res")
            nc.vector.tensor_sub(res, den11, num11)
            nc.sync.dma_start(out=out_ext[:, :], in_=res)

    nc.finalize()
    _BUILD_CACHE["nc"] = nc
    return nc


# ---- host-side input prep ---------------------------------------------------

# gate perm [i(256), f(256), g(256), o(256)] -> [i, f, o, g~]
_GPERM = np.concatenate([np.arange(0, 512), np.arange(768, 1024), np.arange(512, 768)])

# core c -> (pathway, direction, half): 0..3 char f/f/b/b, 4..7 word
_ROLES = [("c", 0, 0), ("c", 0, 1), ("c", 1, 0), ("c", 1, 1),
          ("w", 0, 0), ("w", 0, 1), ("w", 1, 0), ("w", 1, 1)]


def _wih_prep(W, dk_n):
    Wp = W[_GPERM]
    return np.ascontiguousarray(
        Wp.reshape(8, 128, dk_n, 128).transpose(3, 2, 0, 1).reshape(128, dk_n * 8 * 128)
    ).astype(np.float16)


def _make_in_maps(inputs):
    char_ids = np.asarray(inputs["char_ids"])
    tags = np.asarray(inputs["tags"])
    wemb = np.asarray(inputs["word_embeddings"], np.float32)
    emb = np.asarray(inputs["char_emb_table"], np.float32)
    trans = np.asarray(inputs["crf_trans"], np.float32)

    alpha = 1.0 / (15.0 * float(np.exp(trans).mean()))
    common = {}
    w1 = np.asarray(inputs["cls_w1"], np.float32)
    common["cls1"] = np.ascontiguousarray(
        w1.reshape(4, 128, 8, 128).transpose(3, 2, 0, 1).reshape(128, 8 * 4 * 128)
    ).astype(np.float16)
    common["clsb1"] = np.ascontiguousarray(
        np.asarray(inputs["cls_b1"], np.float32).reshape(4, 128).T).astype(np.float32)
    w2 = np.asarray(inputs["cls_w2"], np.float32)
    common["cls2"] = np.ascontiguousarray(
        w2.reshape(15, 4, 128).transpose(2, 1, 0).reshape(128, 4 * 15)).astype(np.float16)
    common["clsb2"] = np.asarray(inputs["cls_b2"], np.float32).reshape(15, 1).copy()
    common["mexp"] = np.exp(trans).astype(np.float32)
    common["transn"] = trans.astype(np.float16)
    common["crfstart"] = np.asarray(inputs["crf_start"], np.float32).reshape(15, 1).copy()
    common["crfend"] = np.asarray(inputs["crf_end"], np.float32).reshape(15, 1).copy()
    common["lnalpha"] = np.full((15, 1), np.log(alpha), np.float32)

    in_maps = []
    for c in range(NC_N):
        pw, d, hf = _ROLES[c]
        lo, hi = hf * BL2, (hf + 1) * BL2
        m = dict(common)

        # phase-1 weights/input
        if pw == "c":
            Wih1 = np.zeros((1024, 768), np.float32)
            Wih1[:, :128] = np.asarray(inputs["c0_Wih"], np.float32)[d]
            Whh1 = np.asarray(inputs["c0_Whh"], np.float32)[d]
            b1 = (np.asarray(inputs["c0_bih"], np.float32)[d]
                  + np.asarray(inputs["c0_bhh"], np.float32)[d])
            ce = emb[char_ids[lo:hi]]  # (32, 512, 128)
            X = np.zeros((128, DK1, T, BL2), np.float32)
            X[:, 0] = ce.transpose(2, 1, 0)
            Wl1 = np.asarray(inputs["c1_Wih"], np.float32)[d]
            Whh2 = np.asarray(inputs["c1_Whh"], np.float32)[d]
            b2 = (np.asarray(inputs["c1_bih"], np.float32)[d]
                  + np.asarray(inputs["c1_bhh"], np.float32)[d])
        else:
            Wih1 = np.asarray(inputs["w0_Wih"], np.float32)[d]
            Whh1 = np.asarray(inputs["w0_Whh"], np.float32)[d]
            b1 = (np.asarray(inputs["w0_bih"], np.float32)[d]
                  + np.asarray(inputs["w0_bhh"], np.float32)[d])
            X = wemb[lo:hi].reshape(BL2, T, DK1, 128).transpose(3, 2, 1, 0)
            Wl1 = np.asarray(inputs["w1_Wih"], np.float32)[d]
            Whh2 = np.asarray(inputs["w1_Whh"], np.float32)[d]
            b2 = (np.asarray(inputs["w1_bih"], np.float32)[d]
                  + np.asarray(inputs["w1_bhh"], np.float32)[d])
        if d == 1:  # backward: reverse local time
            X = X[:, :, ::-1]
        m["x1"] = np.ascontiguousarray(X.reshape(128, DK1, T * BL2)).astype(np.float16)
        m["wih1"] = _wih_prep(Wih1, DK1)
        m["whh1"] = _wih_prep(Whh1, 2)
        m["bias1"] = np.ascontiguousarray(b1[_GPERM].reshape(8, 128).T).astype(np.float32)

        # phase-2 weights: columns [own(256) | peer(256)]
        if d == 1:
            Wl1 = Wl1[:, np.r_[256:512, 0:256]]
        m["wih2"] = _wih_prep(Wl1, DK2)
        m["whh2"] = _wih_prep(Whh2, 2)
        m["bias2"] = np.ascontiguousarray(b2[_GPERM].reshape(8, 128).T).astype(np.float32)
        # blend: f-core (d=0) picks AG region 1 (the b-core), b-core picks 0
        msk = np.zeros((128, 2), np.float32)
        msk[:, 1 - d] = 1.0
        m["masks"] = msk

        # phase-3 tags for this core's 8 sequences
        seqs3 = np.r_[4 * c: 4 * c + 4, 32 + 4 * c: 32 + 4 * c + 4]
        oh = (np.arange(K)[:, None, None] == tags[seqs3][None]).astype(np.float32)
        # (15, 8seq, 512t) -> (15, t, b)
        m["tagoneT"] = np.ascontiguousarray(
            oh.transpose(0, 2, 1).reshape(K, TB3)).astype(np.float16)
        in_maps.append(m)
    return in_maps, alpha


def kernel(**inputs):
    nc = _build_nc()
    in_maps, alpha = _make_in_maps(inputs)
    res = run_bass_kernel_spmd(nc, in_maps, core_ids=list(range(NC_N)))
    total = sum(float(res.results[c]["out"][0, 0]) for c in range(NC_N))
    total -= B * (T - 1) * np.log(alpha)
    return np.float32(total / B)
